# revision 29
# baseline (speedup 1.0000x reference)
"""Trainium2 Bass kernel for nn_MultiHeadAttention_72189810312078.

Computation (per token): qkv = x @ w_qkv.T + b_qkv; per-token attention over
the 16 heads with 16x16 score matrices; out = attn_out @ w_out.T + b_out.

Strategy: data-parallel over 8 NeuronCores (8192 tokens each). Host
pre-transposes x to xT [1024, N] so the channel (contraction) dim lands on
SBUF partitions. All matmul operands are float32r (fp32 storage, ~11-bit
mantissa on the PE): f32r matmuls are self-weight-loading (no standalone
LDWEIGHTS instructions, which serialize on this backend) and run at 1
cycle/row when the moving operand is >= 256 wide. Every matmul in the kernel
is structured to hit that fast path:

  1) qkvT projection (per 256-token superblock): 24 feature-chunk matmuls
     (moving=256), K=1024 accumulated in PSUM; bias fused into the
     PSUM->staging eviction (DVE tensor_scalar / ScalarE Identity+bias),
     scattered to Q/K/V staging [d, (group, headslot, t)].
  2) Attention on PAIRS of 8-token groups so moving=256: per pair, two
     scores matmuls K_g^T @ [Q_gA|Q_gB] (the cross-group half is garbage);
     exp on ScalarE writes only the valid halves of persistent es tiles
     whose cross-group halves were zeroed once at startup; in-group
     cross-token mask applied multiplicatively on GpSimd.
  3) attnV flipped and paired: psF[0:128, 256] = sum_X V8_X^T @ es_X where
     V8 [128 rows=(hk,t), 128 cols = 64 V^T cols + 64 constant ones cols];
     the ones columns replicate the per-(head,token) softmax denominator
     row across PSUM partitions 64:128 for free. V^T comes from a PE
     transpose (f32r, 1.5 cyc/row). DVE reciprocal + tensor_mul normalize
     during the S2 pack, which reads psF feature-major directly (no output
     transpose needed).
  4) out-projection per 128-token stage: 16 matmuls (moving=512) against
     host-permuted w_out rows, deferred one batch so the next stage's
     scores cover the S2 dependency; bias added on GpSimd during eviction.

The software pipeline overlaps attention of superblock k with the qkv
projection of superblock k+1 (ping-pong staging halves); projection chunks
are front-loaded 4-per-batch so staging completes a batch early.

KMODE: "f32r" (default) as above; "bf16"/"f32" legacy modes kept for A/B.
PSUM accumulation is fp32 always. Max rel err vs fp32 reference ~4e-4.
"""

import os
import sys
from contextlib import ExitStack, nullcontext

sys.path.insert(0, "/opt/trn_rl_repo")

import numpy as np
import ml_dtypes

import concourse.bass as bass  # noqa: E402
import concourse.bacc as bacc  # noqa: E402
import concourse.tile as tile  # noqa: E402
from concourse import mybir  # noqa: E402
from concourse.bass_utils import run_bass_kernel_spmd  # noqa: E402
from concourse.masks import make_identity  # noqa: E402

F32 = mybir.dt.float32
F32R = mybir.dt.float32r
BF16 = mybir.dt.bfloat16

N_CORES = 8
H, D, C = 16, 64, 1024
SB = 256   # tokens per superblock (projection moving dim)
SS = 128   # tokens per attention sub-stage / out-projection block
NG = SB // 8   # token groups per superblock (32)

KMODE = os.environ.get("KMODE", "f32r")
GPS = int(os.environ.get("GPS", "3"))  # bitmask: 1=memset, 2=mask-mul, 4=bias
Exp = mybir.ActivationFunctionType.Exp
Copy = mybir.ActivationFunctionType.Copy
Ident = mybir.ActivationFunctionType.Identity


def _dtypes(mode):
    """-> (WD projection-weight dtype, SD attention staging, AD S2)."""
    if mode == "bf16":
        return BF16, BF16, BF16
    if mode == "f32r":
        return F32R, F32R, F32R
    return F32, F32, F32


XBF = os.environ.get("XBF", "0") == "1"   # x streamed in bf16 (moving operand)


def build(tok, mode=KMODE, static_loop=False):
    WD, SD, AD = _dtypes(mode)

    nc = bacc.Bacc("TRN2", target_bir_lowering=False, debug=False,
                   enable_asserts=True, num_devices=N_CORES)
    xT_d = nc.dram_tensor("xT", [C, tok], WD, kind="ExternalInput").ap()
    wqkvT_d = nc.dram_tensor("wqkvT", [C, 3 * C], WD, kind="ExternalInput").ap()
    woutT_d = nc.dram_tensor("woutT", [C, C], WD, kind="ExternalInput").ap()
    bcols_d = nc.dram_tensor("bcols", [128, 24], F32, kind="ExternalInput").ap()
    borep_d = nc.dram_tensor("borep", [128, C], F32, kind="ExternalInput").ap()
    maskB_d = nc.dram_tensor("maskB", [128, 512], SD, kind="ExternalInput").ap()
    out_d = nc.dram_tensor("out", [tok, C], F32, kind="ExternalOutput").ap()

    with tile.TileContext(nc) as tc, ExitStack() as ctx:
        consts = ctx.enter_context(tc.tile_pool(name="consts", bufs=1))
        xin = ctx.enter_context(tc.tile_pool(name="xin", bufs=2))
        stag = ctx.enter_context(tc.tile_pool(name="stag", bufs=1))
        smx = ctx.enter_context(tc.tile_pool(name="smx", bufs=2))
        s2p = ctx.enter_context(tc.tile_pool(name="s2p", bufs=2))
        outp = ctx.enter_context(tc.tile_pool(name="outp", bufs=2))
        psA = ctx.enter_context(tc.tile_pool(name="psA", bufs=2, space="PSUM"))
        psSp = ctx.enter_context(tc.tile_pool(name="psSp", bufs=2, space="PSUM"))
        psVp = ctx.enter_context(tc.tile_pool(name="psVp", bufs=1, space="PSUM"))
        psC2p = ctx.enter_context(tc.tile_pool(name="psC2p", bufs=1, space="PSUM"))
        psTp = ctx.enter_context(tc.tile_pool(name="psTp", bufs=1, space="PSUM"))
        psOp = ctx.enter_context(tc.tile_pool(name="psOp", bufs=1, space="PSUM"))

        # ---- constants ----
        wq_sb = consts.tile([128, 8, 3 * C], WD)
        nc.sync.dma_start(out=wq_sb, in_=wqkvT_d.rearrange("(ci p) f -> p ci f", p=128))
        wo_sb = consts.tile([128, 8, C], WD)
        nc.sync.dma_start(out=wo_sb, in_=woutT_d.rearrange("(ci p) f -> p ci f", p=128))
        bcols_sb = consts.tile([128, 24], F32)
        nc.sync.dma_start(out=bcols_sb, in_=bcols_d)
        borep_sb = consts.tile([128, C], F32)
        nc.sync.dma_start(out=borep_sb, in_=borep_d)
        maskB_sb = consts.tile([128, 512], SD)
        nc.sync.dma_start(out=maskB_sb, in_=maskB_d)
        idq = consts.tile([128, 128], F32)
        make_identity(nc, idq)
        if SD is F32:
            idS = idq
        else:
            idS = consts.tile([128, 128], SD)
            nc.vector.tensor_copy(idS, idq)
        if AD is F32:
            idr = idq
        elif AD is SD:
            idr = idS
        else:
            idr = consts.tile([128, 128], AD)
            nc.vector.tensor_copy(idr, idq)

        ecnt = 0  # evict-engine round robin

        def evict_copy(dst, src):
            nonlocal ecnt
            if ecnt % 2 == 0:
                nc.vector.tensor_copy(dst, src)
            else:
                nc.scalar.copy(dst, src)
            ecnt += 1

        xT_r = xT_d.rearrange("(ci p) t -> p ci t", p=128)
        if static_loop:
            loop_iter = [(nullcontext(iv), iv) for iv in range(0, tok, SB)]
        else:
            fc = tc.For_i(0, tok, SB,
                          hint_engines=(mybir.EngineType.PE,
                                        mybir.EngineType.DVE))
            loop_iter = [(fc, None)]
        for _ctx, _iv in loop_iter:
          with _ctx as _cv:
            iv = _iv if _iv is not None else _cv
            x_sb = xin.tile([128, 8, SB], WD)
            nc.sync.dma_start(out=x_sb, in_=xT_r[:, :, bass.ds(iv, SB)])

            # staging: T1 rows 64:128 = Q; T2 rows 64:128 = K, rows 0:64 = V
            T1 = stag.tile([128, NG, 16, 8], SD, name="T1")
            T2 = stag.tile([128, NG, 16, 8], SD, name="T2")

            # ---- qkv projection + scatter-evict (bias fused / on gpsimd) ----
            for co in range(24):
                psC1 = psA.tile([128, SB], F32)
                for ci in range(8):
                    nc.tensor.matmul(psC1, wq_sb[:, ci, co * 128:(co + 1) * 128],
                                     x_sb[:, ci, :], start=(ci == 0),
                                     stop=(ci == 7))
                kind, c = co // 8, co % 8
                for dlt in range(2):
                    src = psC1[64 * dlt:64 * dlt + 64, :].rearrange(
                        "p (g t) -> p g t", g=NG)
                    hslot = 2 * c + dlt
                    if kind == 0:
                        dst = T1[64:128, :, hslot, :]
                    elif kind == 1:
                        dst = T2[64:128, :, hslot, :]
                    else:
                        dst = T2[0:64, :, hslot, :]
                    bias = bcols_sb[64 * dlt:64 * dlt + 64, co:co + 1]
                    if dlt == 0:
                        # DVE evict with fused bias add
                        nc.vector.tensor_scalar_add(dst, src, bias)
                    elif GPS & 4:
                        # ACT plain evict, bias added SBUF-side on idle gpsimd
                        nc.scalar.copy(dst, src)
                        nc.gpsimd.tensor_scalar_add(dst, dst, bias)
                    else:
                        nc.vector.tensor_scalar_add(dst, src, bias)

            # ---- attention (8 batches of 4 groups) + out-proj per 128 tok ----
            for iss in range(2):
                S2 = s2p.tile([128, 8, SS], AD)
                for b4 in range(4 * iss, 4 * iss + 4):
                    psS = psSp.tile([128, 512], F32)
                    psV = psVp.tile([128, 4, 64], SD)
                    for j in range(4):
                        g = 4 * b4 + j
                        nc.tensor.matmul(psS[:, 128 * j:128 * j + 128],
                                         T2[64:128, g, :, :], T1[64:128, g, :, :],
                                         start=True, stop=True)
                        nc.tensor.transpose(psV[:, j, :], T2[0:64, g, :, :],
                                            idS[0:64, 0:64])
                    es4 = smx.tile([128, 512], SD)
                    nc.scalar.activation(es4, psS, Exp, scale=0.125)
                    if GPS & 2:
                        nc.gpsimd.tensor_mul(es4, es4, maskB_sb)
                    else:
                        nc.vector.tensor_mul(es4, es4, maskB_sb)
                    V8sb = smx.tile([128, 4, 66], SD)
                    nc.scalar.copy(V8sb[:, :, 0:64], psV)
                    if GPS & 1:
                        nc.gpsimd.memset(V8sb[:, :, 64:65], 1.0)
                    else:
                        nc.vector.memset(V8sb[:, :, 64:65], 1.0)
                    psC2 = psC2p.tile([128, 4, 66], F32)
                    for j in range(4):
                        nc.tensor.matmul(psC2[:, j, 0:65],
                                         es4[:, 128 * j:128 * j + 128],
                                         V8sb[:, j, 0:65], start=True, stop=True)
                    rec4 = smx.tile([128, 4], F32)
                    nc.vector.reciprocal(rec4, psC2[:, :, 64:65])
                    attno = smx.tile([128, 4, 64], AD)
                    for j in range(4):
                        if j % 2 == 0:
                            nc.scalar.activation(attno[:, j, :], psC2[:, j, 0:64],
                                                 Copy, scale=rec4[:, j:j + 1])
                        else:
                            nc.vector.tensor_scalar_mul(attno[:, j, :],
                                                        psC2[:, j, 0:64],
                                                        rec4[:, j:j + 1])
                    psT = psTp.tile([64, 4, 128], AD)
                    for j in range(4):
                        nc.tensor.transpose(psT[:, j, :], attno[:, j, :], idr)
                    # S2 pack: head slots 8*dlt..8*dlt+7 -> S2 rows 64*dlt+d
                    for dlt in range(2):
                        src = psT[:, :, 64 * dlt:64 * dlt + 64].rearrange(
                            "p j (h t) -> p j h t", h=8)
                        dst = S2[64 * dlt:64 * dlt + 64].rearrange(
                            "p c (gb gj t) -> p gj c gb t", gb=4, gj=4)[
                                :, :, :, b4 % 4, :]
                        evict_copy(dst, src)

                # ---- out-projection for this 128-token block ----
                outsb = outp.tile([128, C], F32)
                for nh in range(2):
                    psO = psOp.tile([128, 512], F32)
                    for c in range(8):
                        nc.tensor.matmul(psO, S2[:, c, :],
                                         wo_sb[:, c, 512 * nh:512 * nh + 512],
                                         start=(c == 0), stop=(c == 7))
                    nc.vector.tensor_add(outsb[:, 512 * nh:512 * nh + 512], psO,
                                         borep_sb[:, 512 * nh:512 * nh + 512])
                nc.sync.dma_start(out=out_d[bass.ds(iv + SS * iss, SS), :],
                                  in_=outsb)

    nc.compile()
    return nc


def build_pipe(tok, mode=KMODE, static_loop=False, reps=1):
    """Software-pipelined build v3: all-f32r matmuls (self-loading weights, no
    standalone LDWEIGHTS), attention via group-PAIR matmuls so every PE op has
    a 256-wide moving operand (f32r fast path):
      - scores: per pair (gA,gB), two matmuls K_g^T @ [Q_gA|Q_gB] (256 moving)
      - exp on ScalarE -> es (bf16); pair mask (kills cross-group and
        cross-token terms) on GpSimd
      - attnV flipped: psF[d(+Z row), QpairCols] = sum_X V8_X^T @ es_X with the
        V8 ones-column producing the softmax denominator row; S2 packed
        directly from psF (no output transpose)
      - denominators: Z row gathered to zbuf, PE-transposed to per-token
        column, reciprocal on DVE, applied as per-partition scale during the
        out-projection eviction on ScalarE.
    Projection of superblock k+1 overlaps attention of superblock k."""
    WD, SD, AD = _dtypes(mode)
    XD = BF16 if XBF else WD

    nc = bacc.Bacc("TRN2", target_bir_lowering=False, debug=False,
                   enable_asserts=True, num_devices=N_CORES)
    xT_d = nc.dram_tensor("xT", [C, tok], XD, kind="ExternalInput").ap()
    wqkvT_d = nc.dram_tensor("wqkvT", [C, 3 * C], WD, kind="ExternalInput").ap()
    woutT_d = nc.dram_tensor("woutT", [C, C], WD, kind="ExternalInput").ap()
    bcols_d = nc.dram_tensor("bcols", [128, 24], F32, kind="ExternalInput").ap()
    borep_d = nc.dram_tensor("borep", [128, C], BF16, kind="ExternalInput").ap()
    maskP_d = nc.dram_tensor("maskP", [128, 128], BF16, kind="ExternalInput").ap()
    out_d = nc.dram_tensor("out", [tok, C], F32, kind="ExternalOutput").ap()

    PB = [int(v) for v in os.environ.get("PB", "1,2,2,2,2,2").split(",")]
    PRJ = int(os.environ.get("PRJ", "3"))
    with tile.TileContext(nc) as tc, ExitStack() as ctx:
        consts = ctx.enter_context(tc.tile_pool(name="consts", bufs=1))
        xin = ctx.enter_context(tc.tile_pool(name="xin", bufs=PB[0]))
        stag = ctx.enter_context(tc.tile_pool(name="stag", bufs=1))
        smx = ctx.enter_context(tc.tile_pool(name="smx", bufs=PB[1]))
        s2p = ctx.enter_context(tc.tile_pool(name="s2p", bufs=PB[2]))
        outp = ctx.enter_context(tc.tile_pool(name="outp", bufs=PB[3]))
        psA = ctx.enter_context(tc.tile_pool(name="psA", bufs=PB[4], space="PSUM"))
        psSp = ctx.enter_context(tc.tile_pool(name="psSp", bufs=PB[5], space="PSUM"))
        psVp = ctx.enter_context(tc.tile_pool(name="psVp", bufs=1, space="PSUM"))
        psFp = ctx.enter_context(tc.tile_pool(name="psFp", bufs=2, space="PSUM"))
        psOp = ctx.enter_context(tc.tile_pool(name="psOp", bufs=1, space="PSUM"))

        wq_sb = consts.tile([128, 8, 3 * C], WD)
        nc.sync.dma_start(out=wq_sb, in_=wqkvT_d.rearrange("(ci p) f -> p ci f", p=128))
        wo_sb = consts.tile([128, 8, C], WD)
        nc.sync.dma_start(out=wo_sb, in_=woutT_d.rearrange("(ci p) f -> p ci f", p=128))
        bcols_sb = consts.tile([128, 24], F32)
        nc.sync.dma_start(out=bcols_sb, in_=bcols_d)
        borep_sb = consts.tile([128, C], BF16)
        nc.sync.dma_start(out=borep_sb, in_=borep_d)
        maskT_sb = consts.tile([128, 128], BF16)
        nc.sync.dma_start(out=maskT_sb, in_=maskP_d)
        idq = consts.tile([128, 128], F32)
        make_identity(nc, idq)
        if SD is F32:
            idS = idq
        else:
            idS = consts.tile([128, 128], SD)
            nc.vector.tensor_copy(idS, idq)

        # persistent ping-pong staging (half-set hb=0: Q/K upper, V lower)
        Q_AB = stag.tile([128, NG, 16, 8], SD, name="Q_AB")
        K_AB = stag.tile([128, NG, 16, 8], SD, name="K_AB")
        V_AB = stag.tile([128, NG, 16, 8], SD, name="V_AB")
        # persistent es pair tiles [128, X, gp, 16, 8]; the gp != X (cross
        # group) halves are zeroed once here and never written again, so the
        # attnV pair matmuls read zeros there without any recurring masking.
        es_pp = [stag.tile([128, 2, 2, 16, 8], SD, name=f"es{i}")
                 for i in range(2)]
        zeroC = consts.tile([128, 128], BF16)
        nc.vector.memset(zeroC, 0.0)
        onesC = consts.tile([128, 64], BF16)
        nc.vector.memset(onesC, 1.0)
        for e in es_pp:
            nc.vector.tensor_copy(e[:, 0, 1].rearrange("p h t -> p (h t)"), zeroC)
            nc.vector.tensor_copy(e[:, 1, 0].rearrange("p h t -> p (h t)"), zeroC)
        # persistent V8 pair tiles [128, X, 128]: cols 0:64 = V^T (rewritten
        # each pair), cols 64:128 = constant ones so the attnV matmul output
        # rows 64:128 replicate the softmax-denominator row across partitions
        v8_pp = [stag.tile([128, 2, 128], SD, name=f"v8_{i}") for i in range(2)]
        for e in v8_pp:
            nc.vector.tensor_copy(e[:, 0, 64:128], onesC)
            nc.vector.tensor_copy(e[:, 1, 64:128], onesC)

        xT_r = xT_d.rearrange("(ci p) t -> p ci t", p=128)

        def emit_xload(piv):
            x_sb = xin.tile([128, 8, SB], XD)
            nc.sync.dma_start(out=x_sb, in_=xT_r[:, :, bass.ds(piv, SB)])
            return x_sb

        def qk_half(T, hb):
            return T[64 * (1 - hb):64 * (1 - hb) + 64]

        def v_half(hb):
            return V_AB[64 * hb:64 * hb + 64]

        def emit_proj_chunk(x_sb, co, hb):
            psC1 = psA.tile([128, SB], F32)
            for ci in range(8):
                nc.tensor.matmul(psC1, wq_sb[:, ci, co * 128:(co + 1) * 128],
                                 x_sb[:, ci, :], start=(ci == 0), stop=(ci == 7))
            kind, c = co // 8, co % 8
            for dlt in range(2):
                src = psC1[64 * dlt:64 * dlt + 64, :].rearrange(
                    "p (g t) -> p g t", g=NG)
                hslot = 2 * c + dlt
                if kind == 0:
                    dst = qk_half(Q_AB, hb)[:, :, hslot, :]
                elif kind == 1:
                    dst = qk_half(K_AB, hb)[:, :, hslot, :]
                else:
                    dst = v_half(hb)[:, :, hslot, :]
                bias = bcols_sb[64 * dlt:64 * dlt + 64, co:co + 1]
                if (co + dlt) % 2 == 0:
                    nc.vector.tensor_scalar_add(dst, src, bias)
                else:
                    nc.scalar.activation(dst, src, Ident, bias=bias)

        def emit_attn_batch1(b4, hb):
            """scores (pair matmuls) + V transposes + exp + pair-mask for the
            two pairs of batch b4 (groups 4*b4..4*b4+3)."""
            vb = 64 * hb
            pend = []
            for p in range(2):
                gA = 4 * b4 + 2 * p
                psS = psSp.tile([128, 2, 256], F32)
                psV = psVp.tile([128, 2, 64], SD)
                qpair = qk_half(Q_AB, hb)[:, gA:gA + 2, :, :]
                for X in range(2):
                    nc.tensor.matmul(psS[:, X, :],
                                     qk_half(K_AB, hb)[:, gA + X, :, :],
                                     qpair, start=True, stop=True)
                for X in range(2):
                    nc.tensor.transpose(psV[:, X, :], v_half(hb)[:, gA + X, :, :],
                                        idS[vb:vb + 64, vb:vb + 64])
                es = es_pp[p]
                for X in range(2):
                    nc.scalar.activation(es[:, X, X],
                                         psS[:, X, 128 * X:128 * X + 128],
                                         Exp, scale=0.125)
                    nc.gpsimd.tensor_mul(es[:, X, X], es[:, X, X], maskT_sb)
                V8sb = v8_pp[p]
                if p == 0:
                    nc.vector.tensor_copy(V8sb[:, :, 0:64], psV)
                else:
                    nc.scalar.copy(V8sb[:, :, 0:64], psV)
                pend.append((es, V8sb))
            return pend

        def emit_attn_batch2(b4, pend, S2, pairs=(0, 1)):
            for p in pairs:
                es, V8sb = pend[p]
                psF = psFp.tile([128, 2, 16, 8], F32)
                esf = es.rearrange("q x gp h t -> q x (gp h t)")
                for X in range(2):
                    nc.tensor.matmul(psF, V8sb[:, X, :], esf[:, X, :],
                                     start=(X == 0), stop=(X == 1))
                # psF rows 64:128 hold the per-(head, token) softmax
                # denominator row replicated by the ones columns of V8
                rZB = smx.tile([64, 2, 16, 8], F32, name="rZB")
                nc.vector.reciprocal(rZB, psF[64:128])
                for dlt in range(2):
                    src = psF[0:64].rearrange("p gp h t -> p h gp t")[
                        :, 8 * dlt:8 * dlt + 8, :, :]
                    rzs = rZB.rearrange("p gp h t -> p h gp t")[
                        :, 8 * dlt:8 * dlt + 8, :, :]
                    dst = S2[64 * dlt:64 * dlt + 64].rearrange(
                        "p c (gb pp gp t) -> p c gb pp gp t",
                        gb=4, pp=2, gp=2)[:, :, b4 % 4, p, :, :]
                    nc.vector.tensor_mul(dst, src, rzs)

        def emit_outproj(S2, oiv, iss):
            for nh in range(2):
                psO = psOp.tile([128, 512], F32)
                for c in range(8):
                    nc.tensor.matmul(psO, S2[:, c, :],
                                     wo_sb[:, c, 512 * nh:512 * nh + 512],
                                     start=(c == 0), stop=(c == 7))
                outsb = outp.tile([128, 512], F32, name="outsb")
                nc.scalar.copy(outsb, psO)
                nc.gpsimd.tensor_add(outsb, outsb,
                                     borep_sb[:, 512 * nh:512 * nh + 512])
                nc.sync.dma_start(
                    out=out_d[bass.ds(oiv + SS * iss, SS),
                              bass.ds(512 * nh, 512)], in_=outsb)

        def emit_part(attn_oiv, attn_hb, proj_piv, proj_hb):
            """Weave attention of one superblock with projection of another.
            Either may be None (prologue/epilogue)."""
            x_sb = emit_xload(proj_piv) if proj_piv is not None else None
            S2 = None
            dpo = None   # deferred out-projection (S2, iss)
            for b4 in range(8):
                if attn_oiv is not None:
                    if b4 % 4 == 0:
                        S2 = s2p.tile([128, 8, SS], AD, name="S2")
                    pend = emit_attn_batch1(b4, attn_hb)
                    if dpo is not None:
                        emit_outproj(dpo[0], attn_oiv, dpo[1])
                        dpo = None
                if x_sb is not None:
                    for co in range(PRJ * b4, min(PRJ * b4 + PRJ, 24)):
                        emit_proj_chunk(x_sb, co, proj_hb)
                if attn_oiv is not None:
                    emit_attn_batch2(b4, pend, S2)
                    if b4 % 4 == 3:
                        dpo = (S2, b4 // 4)
            if dpo is not None:
                emit_outproj(dpo[0], attn_oiv, dpo[1])

        assert tok % (2 * SB) == 0 and tok >= 2 * SB
        emit_part(None, None, 0, 0)                      # prologue: proj sb0 -> A
        if tok > 2 * SB and static_loop:
            for iv in range(0, tok - 2 * SB, 2 * SB):
                emit_part(iv, 0, iv + SB, 1)             # attn A, proj -> B
                emit_part(iv + SB, 1, iv + 2 * SB, 0)    # attn B, proj -> A
        elif tok > 2 * SB and reps == 1:
            with tc.For_i(0, tok - 2 * SB, 2 * SB,
                          hint_engines=(mybir.EngineType.PE, mybir.EngineType.DVE,
                                        mybir.EngineType.Activation)) as iv:
                emit_part(iv, 0, iv + SB, 1)             # attn A, proj -> B
                emit_part(iv + SB, 1, iv + 2 * SB, 0)    # attn B, proj -> A
        elif tok > 2 * SB:
            with tc.For_i(0, reps, 1) as _rep:
                with tc.For_i(0, tok - 2 * SB, 2 * SB,
                              hint_engines=(mybir.EngineType.PE,
                                            mybir.EngineType.DVE,
                                            mybir.EngineType.Activation)) as iv:
                    emit_part(iv, 0, iv + SB, 1)         # attn A, proj -> B
                    emit_part(iv + SB, 1, iv + 2 * SB, 0)  # attn B, proj -> A
        last = tok - 2 * SB
        emit_part(last, 0, tok - SB, 1)                  # attn A, proj last -> B
        emit_part(tok - SB, 1, None, None)               # attn B
    nc.compile()
    return nc


def build_pipe4(tok, mode=KMODE, static_loop=False, reps=1):
    """v4: like build_pipe (v3) but
      - PSUM bank remap (dep tracking is bank-granular, 8 banks):
        psA 3 bufs (proj, deeper pipelining; v3 had 2), psSp 2, pvf 2
        (V^T transpose target + attnV psF SHARE one 1.5KB tile per pair --
        their accesses are a sequential chain within the pair so the shared
        bank adds no serialization), psOp 1.
      - exp and mask fused across the pair with a strided diag AP
        ([128, 2(stride 384), 128]): ONE ScalarE exp + ONE GpSimd mask-mul
        per pair instead of 2+2.
      - out-proj eviction fused: GpSimd tensor_add(outsb, psO, borep) reads
        PSUM directly (drops the separate ScalarE copy).
    """
    WD, SD, AD = _dtypes(mode)
    XD = BF16 if XBF else WD

    nc = bacc.Bacc("TRN2", target_bir_lowering=False, debug=False,
                   enable_asserts=True, num_devices=N_CORES)
    xT_d = nc.dram_tensor("xT", [C, tok], XD, kind="ExternalInput").ap()
    wqkvT_d = nc.dram_tensor("wqkvT", [C, 3 * C], WD, kind="ExternalInput").ap()
    woutT_d = nc.dram_tensor("woutT", [C, C], WD, kind="ExternalInput").ap()
    bcols_d = nc.dram_tensor("bcols", [128, 24], F32, kind="ExternalInput").ap()
    borep_d = nc.dram_tensor("borep", [128, C], BF16, kind="ExternalInput").ap()
    maskP_d = nc.dram_tensor("maskP", [128, 128], BF16, kind="ExternalInput").ap()
    out_d = nc.dram_tensor("out", [tok, C], F32, kind="ExternalOutput").ap()

    PRJ = int(os.environ.get("PRJ", "3"))
    NSLA = int(os.environ.get("NSLA", "3"))   # proj PSUM bufs
    XBUF = int(os.environ.get("XBUF", "2" if XBF else "1"))
    with tile.TileContext(nc) as tc, ExitStack() as ctx:
        consts = ctx.enter_context(tc.tile_pool(name="consts", bufs=1))
        xin = ctx.enter_context(tc.tile_pool(name="xin", bufs=XBUF))
        stag = ctx.enter_context(tc.tile_pool(name="stag", bufs=1))
        smx = ctx.enter_context(tc.tile_pool(name="smx", bufs=2))
        s2p = ctx.enter_context(tc.tile_pool(name="s2p", bufs=2))
        outp = ctx.enter_context(tc.tile_pool(name="outp", bufs=2))
        psA = ctx.enter_context(tc.tile_pool(name="psA", bufs=NSLA, space="PSUM"))
        psSp = ctx.enter_context(tc.tile_pool(name="psSp", bufs=2, space="PSUM"))
        pvf = ctx.enter_context(tc.tile_pool(name="pvf", bufs=2, space="PSUM"))
        psOp = ctx.enter_context(tc.tile_pool(name="psOp", bufs=1, space="PSUM"))

        wq_sb = consts.tile([128, 8, 3 * C], WD)
        nc.sync.dma_start(out=wq_sb, in_=wqkvT_d.rearrange("(ci p) f -> p ci f", p=128))
        wo_sb = consts.tile([128, 8, C], WD)
        nc.sync.dma_start(out=wo_sb, in_=woutT_d.rearrange("(ci p) f -> p ci f", p=128))
        bcols_sb = consts.tile([128, 24], F32)
        nc.sync.dma_start(out=bcols_sb, in_=bcols_d)
        borep_sb = consts.tile([128, C], BF16)
        nc.sync.dma_start(out=borep_sb, in_=borep_d)
        maskT_sb = consts.tile([128, 128], BF16)
        nc.sync.dma_start(out=maskT_sb, in_=maskP_d)
        maskT2 = maskT_sb.unsqueeze(1).broadcast_to([128, 2, 128])
        idq = consts.tile([128, 128], F32)
        make_identity(nc, idq)
        if SD is F32:
            idS = idq
        else:
            idS = consts.tile([128, 128], SD)
            nc.vector.tensor_copy(idS, idq)

        # persistent ping-pong staging (half-set hb=0: Q/K upper, V lower)
        Q_AB = stag.tile([128, NG, 16, 8], SD, name="Q_AB")
        K_AB = stag.tile([128, NG, 16, 8], SD, name="K_AB")
        V_AB = stag.tile([128, NG, 16, 8], SD, name="V_AB")
        # persistent es pair tiles [128, X, gp, 16, 8]; cross (gp != X) halves
        # zeroed once, never rewritten -> attnV pair matmuls read zeros there.
        es_pp = [stag.tile([128, 2, 2, 16, 8], SD, name=f"es{i}")
                 for i in range(2)]
        zeroC = consts.tile([128, 128], BF16)
        nc.vector.memset(zeroC, 0.0)
        onesC = consts.tile([128, 64], BF16)
        nc.vector.memset(onesC, 1.0)
        for e in es_pp:
            nc.vector.tensor_copy(e[:, 0, 1].rearrange("p h t -> p (h t)"), zeroC)
            nc.vector.tensor_copy(e[:, 1, 0].rearrange("p h t -> p (h t)"), zeroC)
        # persistent V8 pair tiles [128, X, 128]: cols 0:64 = V^T (rewritten
        # each pair), cols 64:128 = ones -> attnV rows 64:128 = softmax denom
        v8_pp = [stag.tile([128, 2, 128], SD, name=f"v8_{i}") for i in range(2)]
        for e in v8_pp:
            nc.vector.tensor_copy(e[:, 0, 64:128], onesC)
            nc.vector.tensor_copy(e[:, 1, 64:128], onesC)

        xT_r = xT_d.rearrange("(ci p) t -> p ci t", p=128)

        def emit_xload(piv):
            x_sb = xin.tile([128, 8, SB], XD)
            nc.sync.dma_start(out=x_sb, in_=xT_r[:, :, bass.ds(piv, SB)])
            return x_sb

        def qk_half(T, hb):
            return T[64 * (1 - hb):64 * (1 - hb) + 64]

        def v_half(hb):
            return V_AB[64 * hb:64 * hb + 64]

        def emit_proj_chunk(x_sb, co, hb):
            psC1 = psA.tile([128, SB], F32)
            for ci in range(8):
                nc.tensor.matmul(psC1, wq_sb[:, ci, co * 128:(co + 1) * 128],
                                 x_sb[:, ci, :], start=(ci == 0), stop=(ci == 7))
            kind, c = co // 8, co % 8
            for dlt in range(2):
                src = psC1[64 * dlt:64 * dlt + 64, :].rearrange(
                    "p (g t) -> p g t", g=NG)
                hslot = 2 * c + dlt
                if kind == 0:
                    dst = qk_half(Q_AB, hb)[:, :, hslot, :]
                elif kind == 1:
                    dst = qk_half(K_AB, hb)[:, :, hslot, :]
                else:
                    dst = v_half(hb)[:, :, hslot, :]
                bias = bcols_sb[64 * dlt:64 * dlt + 64, co:co + 1]
                if (co + dlt) % 2 == 0:
                    nc.vector.tensor_scalar_add(dst, src, bias)
                else:
                    nc.scalar.activation(dst, src, Ident, bias=bias)

        def diag2(ap4):
            """[128, 2, 256]-ish -> diag blocks [128, 2 (stride 384), 128]."""
            flat = ap4.rearrange("p x c -> p (x c)")
            return flat.rearrange("p (q r) -> p q r", q=4)[:, ::3, :]

        def emit_attn_batch1(b4, hb):
            """scores (pair matmuls) + V transposes + fused exp + pair-mask
            for the two pairs of batch b4 (groups 4*b4..4*b4+3)."""
            vb = 64 * hb
            pend = []
            for p in range(2):
                gA = 4 * b4 + 2 * p
                psS = psSp.tile([128, 2, 256], F32)
                vft = pvf.tile([128, 384], F32, name="vf")
                psV = vft[:, 256:384].rearrange("p (x v) -> p x v", x=2)
                if SD is not F32:
                    psV = psV.bitcast(SD)
                qpair = qk_half(Q_AB, hb)[:, gA:gA + 2, :, :]
                for X in range(2):
                    nc.tensor.matmul(psS[:, X, :],
                                     qk_half(K_AB, hb)[:, gA + X, :, :],
                                     qpair, start=True, stop=True)
                for X in range(2):
                    nc.tensor.transpose(psV[:, X, :], v_half(hb)[:, gA + X, :, :],
                                        idS[vb:vb + 64, vb:vb + 64])
                es = es_pp[p]
                es_diag = diag2(es.rearrange("p x g h t -> p x (g h t)"))
                nc.scalar.activation(es_diag, diag2(psS), Exp, scale=0.125)
                nc.gpsimd.tensor_mul(es_diag, es_diag, maskT2)
                V8sb = v8_pp[p]
                if p == 0:
                    nc.vector.tensor_copy(V8sb[:, :, 0:64], psV)
                else:
                    nc.scalar.copy(V8sb[:, :, 0:64], psV)
                pend.append((es, V8sb, vft))
            return pend

        def emit_attn_batch2(b4, pend, S2, pairs=(0, 1)):
            for p in pairs:
                es, V8sb, vft = pend[p]
                psF = vft[:, 0:256].rearrange("p (g h t) -> p g h t", g=2, h=16)
                esf = es.rearrange("q x gp h t -> q x (gp h t)")
                for X in range(2):
                    nc.tensor.matmul(psF, V8sb[:, X, :], esf[:, X, :],
                                     start=(X == 0), stop=(X == 1))
                # psF rows 64:128: per-(head, token) softmax denominator
                rZB = smx.tile([64, 2, 16, 8], F32, name="rZB")
                nc.vector.reciprocal(rZB, psF[64:128])
                for dlt in range(2):
                    src = psF[0:64].rearrange("p gp h t -> p h gp t")[
                        :, 8 * dlt:8 * dlt + 8, :, :]
                    rzs = rZB.rearrange("p gp h t -> p h gp t")[
                        :, 8 * dlt:8 * dlt + 8, :, :]
                    dst = S2[64 * dlt:64 * dlt + 64].rearrange(
                        "p c (gb pp gp t) -> p c gb pp gp t",
                        gb=4, pp=2, gp=2)[:, :, b4 % 4, p, :, :]
                    nc.vector.tensor_mul(dst, src, rzs)

        def emit_outproj(S2, oiv, iss):
            for nh in range(2):
                psO = psOp.tile([128, 512], F32)
                for c in range(8):
                    nc.tensor.matmul(psO, S2[:, c, :],
                                     wo_sb[:, c, 512 * nh:512 * nh + 512],
                                     start=(c == 0), stop=(c == 7))
                outsb = outp.tile([128, 512], F32, name="outsb")
                nc.scalar.copy(outsb, psO)
                nc.gpsimd.tensor_add(outsb, outsb,
                                     borep_sb[:, 512 * nh:512 * nh + 512])
                nc.sync.dma_start(
                    out=out_d[bass.ds(oiv + SS * iss, SS),
                              bass.ds(512 * nh, 512)], in_=outsb)

        def emit_part(attn_oiv, attn_hb, proj_piv, proj_hb):
            """Weave attention of one superblock with projection of another."""
            x_sb = emit_xload(proj_piv) if proj_piv is not None else None
            S2 = None
            dpo = None   # deferred out-projection (S2, iss)
            for b4 in range(8):
                if attn_oiv is not None:
                    if b4 % 4 == 0:
                        S2 = s2p.tile([128, 8, SS], AD, name="S2")
                    pend = emit_attn_batch1(b4, attn_hb)
                    if dpo is not None:
                        emit_outproj(dpo[0], attn_oiv, dpo[1])
                        dpo = None
                if x_sb is not None:
                    for co in range(PRJ * b4, min(PRJ * b4 + PRJ, 24)):
                        emit_proj_chunk(x_sb, co, proj_hb)
                if attn_oiv is not None:
                    emit_attn_batch2(b4, pend, S2)
                    if b4 % 4 == 3:
                        dpo = (S2, b4 // 4)
            if dpo is not None:
                emit_outproj(dpo[0], attn_oiv, dpo[1])

        assert tok % (2 * SB) == 0 and tok >= 2 * SB
        emit_part(None, None, 0, 0)                      # prologue: proj sb0 -> A
        if tok > 2 * SB and static_loop:
            for iv in range(0, tok - 2 * SB, 2 * SB):
                emit_part(iv, 0, iv + SB, 1)             # attn A, proj -> B
                emit_part(iv + SB, 1, iv + 2 * SB, 0)    # attn B, proj -> A
        elif tok > 2 * SB and reps == 1:
            with tc.For_i(0, tok - 2 * SB, 2 * SB,
                          hint_engines=(mybir.EngineType.PE, mybir.EngineType.DVE,
                                        mybir.EngineType.Activation)) as iv:
                emit_part(iv, 0, iv + SB, 1)             # attn A, proj -> B
                emit_part(iv + SB, 1, iv + 2 * SB, 0)    # attn B, proj -> A
        elif tok > 2 * SB:
            with tc.For_i(0, reps, 1) as _rep:
                with tc.For_i(0, tok - 2 * SB, 2 * SB,
                              hint_engines=(mybir.EngineType.PE,
                                            mybir.EngineType.DVE,
                                            mybir.EngineType.Activation)) as iv:
                    emit_part(iv, 0, iv + SB, 1)         # attn A, proj -> B
                    emit_part(iv + SB, 1, iv + 2 * SB, 0)  # attn B, proj -> A
        last = tok - 2 * SB
        emit_part(last, 0, tok - SB, 1)                  # attn A, proj last -> B
        emit_part(tok - SB, 1, None, None)               # attn B
    nc.compile()
    return nc


def build_pipe5(tok, mode="bf16", static_loop=False, reps=1):
    """v5: all-bf16 + K=128-everywhere matmul shapes (HW-measured: K=64
    matmuls run ~2x slower per output column; bf16 streams beat f32r and
    LDWEIGHTS does NOT serialize on this backend):

      - everything (x, weights, staging, es, S2) in bf16; PSUM f32.
      - scores via BLOCK-DIAG pairs: K staged with even groups' d-dim on
        partitions 0:64 and odd groups' on 64:128 (K_bd[128, pr, 16, 8]);
        Q staged zero-padded block-diag (Q_bd[128, pr, 2, 16, 8], the
        off-diagonal partition halves zeroed once at startup). ONE matmul
        per pair: psS[128, 2*128] = K_bd[:,pr]^T @ Q_bd[:,pr], K=128,
        moving 256 -> both groups' score blocks, no garbage columns.
      - exp: ONE ScalarE activation [128,256] psS -> es_pair bf16 (dense,
        no diag APs); ONE GpSimd mask-mul (broadcast [128,2,128]).
      - attnV per GROUP (K=128, N=128): psF_g = V8_g^T @ es_g with the
        V8 ones-columns producing the softmax denominator rows; psF pair
        halves live side by side in the pvf tile so the pair-granular
        reciprocal + S2 pack from v4 are unchanged.
      - PSUM banks: psA x3 (1 bank ea), psS x2, pvf x2 (psF pair + psV
        transposes share a 1.25KB tile), psO x1 = 8.
      - Q/K projection evictions split even/odd groups (partition-shifted
        writes, 4 small instrs per chunk instead of 2).
    """
    WD = SD = AD = BF16

    nc = bacc.Bacc("TRN2", target_bir_lowering=False, debug=False,
                   enable_asserts=True, num_devices=N_CORES)
    xT_d = nc.dram_tensor("xT", [C, tok], WD, kind="ExternalInput").ap()
    wqkvT_d = nc.dram_tensor("wqkvT", [C, 3 * C], WD, kind="ExternalInput").ap()
    woutT_d = nc.dram_tensor("woutT", [C, C], WD, kind="ExternalInput").ap()
    bcols_d = nc.dram_tensor("bcols", [128, 24], F32, kind="ExternalInput").ap()
    borep_d = nc.dram_tensor("borep", [128, C], BF16, kind="ExternalInput").ap()
    maskP_d = nc.dram_tensor("maskP", [128, 128], BF16, kind="ExternalInput").ap()
    out_d = nc.dram_tensor("out", [tok, C], F32, kind="ExternalOutput").ap()

    PRJ = int(os.environ.get("PRJ", "3"))
    NSLA = int(os.environ.get("NSLA", "3"))
    NPAIR = NG // 2  # 16 pairs per superblock
    with tile.TileContext(nc) as tc, ExitStack() as ctx:
        consts = ctx.enter_context(tc.tile_pool(name="consts", bufs=1))
        xin = ctx.enter_context(tc.tile_pool(name="xin", bufs=2))
        stag = ctx.enter_context(tc.tile_pool(name="stag", bufs=1))
        smx = ctx.enter_context(tc.tile_pool(name="smx", bufs=3))
        s2p = ctx.enter_context(tc.tile_pool(name="s2p", bufs=2))
        outp = ctx.enter_context(tc.tile_pool(name="outp", bufs=2))
        psA = ctx.enter_context(tc.tile_pool(name="psA", bufs=NSLA, space="PSUM"))
        psSp = ctx.enter_context(tc.tile_pool(name="psSp", bufs=2, space="PSUM"))
        pvf = ctx.enter_context(tc.tile_pool(name="pvf", bufs=2, space="PSUM"))
        psOp = ctx.enter_context(tc.tile_pool(name="psOp", bufs=1, space="PSUM"))

        wq_sb = consts.tile([128, 8, 3 * C], WD)
        nc.sync.dma_start(out=wq_sb, in_=wqkvT_d.rearrange("(ci p) f -> p ci f", p=128))
        wo_sb = consts.tile([128, 8, C], WD)
        nc.sync.dma_start(out=wo_sb, in_=woutT_d.rearrange("(ci p) f -> p ci f", p=128))
        bcols_sb = consts.tile([128, 24], F32)
        nc.sync.dma_start(out=bcols_sb, in_=bcols_d)
        borep_sb = consts.tile([128, C], BF16)
        nc.sync.dma_start(out=borep_sb, in_=borep_d)
        maskT_sb = consts.tile([128, 128], BF16)
        nc.sync.dma_start(out=maskT_sb, in_=maskP_d)
        maskT2 = maskT_sb.unsqueeze(1).broadcast_to([128, 2, 128])
        idq = consts.tile([128, 128], F32)
        make_identity(nc, idq)
        idS = consts.tile([128, 128], SD)
        nc.vector.tensor_copy(idS, idq)

        # staging: block-diag K/Q per ping-pong half (full 128 partitions),
        # V keeps the half-partition ping-pong of v3/v4.
        K_bd = [stag.tile([128, NPAIR, 16, 8], SD, name=f"Kbd{i}")
                for i in range(2)]
        Q_bd = [stag.tile([128, NPAIR, 2, 16, 8], SD, name=f"Qbd{i}")
                for i in range(2)]
        V_AB = stag.tile([128, NG, 16, 8], SD, name="V_AB")
        zeroC = consts.tile([128, 128], BF16)
        nc.vector.memset(zeroC, 0.0)
        onesC = consts.tile([128, 64], BF16)
        nc.vector.memset(onesC, 1.0)
        # zero the off-diagonal Q halves once (never rewritten)
        for qb in Q_bd:
            for par in range(2):
                z = qb[64 * (1 - par):64 * (1 - par) + 64, :, par]
                nc.vector.memset(z, 0.0)
        v8_pp = [stag.tile([128, 2, 128], SD, name=f"v8_{i}") for i in range(2)]
        for e in v8_pp:
            nc.vector.tensor_copy(e[:, 0, 64:128], onesC)
            nc.vector.tensor_copy(e[:, 1, 64:128], onesC)

        xT_r = xT_d.rearrange("(ci p) t -> p ci t", p=128)

        def emit_xload(piv):
            x_sb = xin.tile([128, 8, SB], WD)
            nc.sync.dma_start(out=x_sb, in_=xT_r[:, :, bass.ds(piv, SB)])
            return x_sb

        def v_half(hb):
            return V_AB[64 * hb:64 * hb + 64]

        ECNT = [0]

        def evict(dst, src, bias):
            if ECNT[0] % 2 == 0:
                nc.vector.tensor_scalar_add(dst, src, bias)
            else:
                nc.scalar.activation(dst, src, Ident, bias=bias)
            ECNT[0] += 1

        def emit_proj_chunk(x_sb, co, hb):
            psC1 = psA.tile([128, SB], F32)
            for ci in range(8):
                nc.tensor.matmul(psC1, wq_sb[:, ci, co * 128:(co + 1) * 128],
                                 x_sb[:, ci, :], start=(ci == 0), stop=(ci == 7))
            kind, c = co // 8, co % 8
            hslot = 2 * c  # +dlt below
            for dlt in range(2):
                bias = bcols_sb[64 * dlt:64 * dlt + 64, co:co + 1]
                srcg = psC1[64 * dlt:64 * dlt + 64, :].rearrange(
                    "p (pr par t) -> p pr par t", par=2, t=8)
                if kind == 2:
                    src = psC1[64 * dlt:64 * dlt + 64, :].rearrange(
                        "p (g t) -> p g t", g=NG)
                    evict(v_half(hb)[:, :, hslot + dlt, :], src, bias)
                    continue
                for par in range(2):
                    src = srcg[:, :, par, :]
                    if kind == 0:
                        dst = Q_bd[hb][64 * par:64 * par + 64, :, par,
                                       hslot + dlt, :]
                    else:
                        dst = K_bd[hb][64 * par:64 * par + 64, :,
                                       hslot + dlt, :]
                    evict(dst, src, bias)

        def emit_attn_batch1(b4, hb):
            """block-diag pair scores + V^T transposes + fused exp/mask."""
            vb = 64 * hb
            pend = []
            for p in range(2):
                pr = 2 * b4 + p
                gA = 4 * b4 + 2 * p
                psS = psSp.tile([128, 256], F32)
                vft = pvf.tile([128, 320], F32, name="vf")
                psV = vft[:, 256:320].bitcast(SD).rearrange(
                    "p (x v) -> p x v", x=2)
                nc.tensor.matmul(psS, K_bd[hb][:, pr], Q_bd[hb][:, pr],
                                 start=True, stop=True)
                for X in range(2):
                    nc.tensor.transpose(psV[:, X, :], v_half(hb)[:, gA + X, :, :],
                                        idS[vb:vb + 64, vb:vb + 64])
                es = smx.tile([128, 2, 128], SD, name="es")
                nc.scalar.activation(es, psS.rearrange("p (x c) -> p x c", x=2),
                                     Exp, scale=0.125)
                nc.gpsimd.tensor_mul(es, es, maskT2)
                V8sb = v8_pp[p]
                if p == 0:
                    nc.vector.tensor_copy(V8sb[:, :, 0:64], psV)
                else:
                    nc.scalar.copy(V8sb[:, :, 0:64], psV)
                pend.append((es, V8sb, vft))
            return pend

        def emit_attn_batch2(b4, pend, S2, pairs=(0, 1)):
            for p in pairs:
                es, V8sb, vft = pend[p]
                psF = vft[:, 0:256].rearrange("p (g h t) -> p g h t", g=2, h=16)
                for X in range(2):
                    nc.tensor.matmul(psF[:, X], V8sb[:, X, :], es[:, X, :],
                                     start=True, stop=True)
                rZB = smx.tile([64, 2, 16, 8], F32, name="rZB")
                nc.vector.reciprocal(rZB, psF[64:128])
                for dlt in range(2):
                    src = psF[0:64].rearrange("p gp h t -> p h gp t")[
                        :, 8 * dlt:8 * dlt + 8, :, :]
                    rzs = rZB.rearrange("p gp h t -> p h gp t")[
                        :, 8 * dlt:8 * dlt + 8, :, :]
                    dst = S2[64 * dlt:64 * dlt + 64].rearrange(
                        "p c (gb pp gp t) -> p c gb pp gp t",
                        gb=4, pp=2, gp=2)[:, :, b4 % 4, p, :, :]
                    nc.vector.tensor_mul(dst, src, rzs)

        def emit_outproj(S2, oiv, iss):
            for nh in range(2):
                psO = psOp.tile([128, 512], F32)
                for c in range(8):
                    nc.tensor.matmul(psO, S2[:, c, :],
                                     wo_sb[:, c, 512 * nh:512 * nh + 512],
                                     start=(c == 0), stop=(c == 7))
                outsb = outp.tile([128, 512], F32, name="outsb")
                nc.scalar.copy(outsb, psO)
                nc.gpsimd.tensor_add(outsb, outsb,
                                     borep_sb[:, 512 * nh:512 * nh + 512])
                nc.sync.dma_start(
                    out=out_d[bass.ds(oiv + SS * iss, SS),
                              bass.ds(512 * nh, 512)], in_=outsb)

        def emit_part(attn_oiv, attn_hb, proj_piv, proj_hb):
            x_sb = emit_xload(proj_piv) if proj_piv is not None else None
            S2 = None
            dpo = None
            for b4 in range(8):
                if attn_oiv is not None:
                    if b4 % 4 == 0:
                        S2 = s2p.tile([128, 8, SS], AD, name="S2")
                    pend = emit_attn_batch1(b4, attn_hb)
                    if dpo is not None:
                        emit_outproj(dpo[0], attn_oiv, dpo[1])
                        dpo = None
                if x_sb is not None:
                    for co in range(PRJ * b4, min(PRJ * b4 + PRJ, 24)):
                        emit_proj_chunk(x_sb, co, proj_hb)
                if attn_oiv is not None:
                    emit_attn_batch2(b4, pend, S2)
                    if b4 % 4 == 3:
                        dpo = (S2, b4 // 4)
            if dpo is not None:
                emit_outproj(dpo[0], attn_oiv, dpo[1])

        assert tok % (2 * SB) == 0 and tok >= 2 * SB
        emit_part(None, None, 0, 0)
        if tok > 2 * SB and static_loop:
            for iv in range(0, tok - 2 * SB, 2 * SB):
                emit_part(iv, 0, iv + SB, 1)
                emit_part(iv + SB, 1, iv + 2 * SB, 0)
        elif tok > 2 * SB and reps == 1:
            with tc.For_i(0, tok - 2 * SB, 2 * SB,
                          hint_engines=(mybir.EngineType.PE, mybir.EngineType.DVE,
                                        mybir.EngineType.Activation)) as iv:
                emit_part(iv, 0, iv + SB, 1)
                emit_part(iv + SB, 1, iv + 2 * SB, 0)
        elif tok > 2 * SB:
            with tc.For_i(0, reps, 1) as _rep:
                with tc.For_i(0, tok - 2 * SB, 2 * SB,
                              hint_engines=(mybir.EngineType.PE,
                                            mybir.EngineType.DVE,
                                            mybir.EngineType.Activation)) as iv:
                    emit_part(iv, 0, iv + SB, 1)
                    emit_part(iv + SB, 1, iv + 2 * SB, 0)
        last = tok - 2 * SB
        emit_part(last, 0, tok - SB, 1)
        emit_part(tok - SB, 1, None, None)
    nc.compile()
    return nc


def build_pipe6(tok, mode="bf16", static_loop=False, reps=1):
    """v6: v5's all-bf16 + K=128 shapes, restructured to minimize instruction
    count (HW shows ~100ns-class per-instruction sync/sequencer overhead that
    the cost model underestimates):
      - SB=512 token superblocks: projection matmuls go 512-wide (same
        per-column rate, HALF the instruction + LDWEIGHTS count), evictions
        double in size and halve in count.
      - attention in QUADS (4 groups): ONE exp [128,512], ONE mask-mul,
        ONE V^T->SBUF copy, ONE reciprocal per quad; 2 block-diag scores
        matmuls, 4 transposes, 4 attnV matmuls, 2 S2-pack muls.
      - PSUM: psA [128,512] x3 (shared by projection chunks AND the
        out-projection), psS-quad [128,2,256] x2, psF-quad [128,4,16,8] x2,
        psV-quad [128,4,64]bf16 x1 = 8 banks.
    """
    del mode
    SB6, SS6 = 512, 128
    NG6 = SB6 // 8          # 64 groups
    NPAIR6 = NG6 // 2       # 32 pairs
    SD = BF16

    nc = bacc.Bacc("TRN2", target_bir_lowering=False, debug=False,
                   enable_asserts=True, num_devices=N_CORES)
    xT_d = nc.dram_tensor("xT", [C, tok], SD, kind="ExternalInput").ap()
    wqkvT_d = nc.dram_tensor("wqkvT", [C, 3 * C], SD, kind="ExternalInput").ap()
    woutT_d = nc.dram_tensor("woutT", [C, C], SD, kind="ExternalInput").ap()
    bcols_d = nc.dram_tensor("bcols", [128, 24], F32, kind="ExternalInput").ap()
    borep_d = nc.dram_tensor("borep", [128, C], BF16, kind="ExternalInput").ap()
    maskP_d = nc.dram_tensor("maskP", [128, 128], BF16, kind="ExternalInput").ap()
    out_d = nc.dram_tensor("out", [tok, C], F32, kind="ExternalOutput").ap()

    NSLA = int(os.environ.get("NSLA", "2"))
    with tile.TileContext(nc) as tc, ExitStack() as ctx:
        consts = ctx.enter_context(tc.tile_pool(name="consts", bufs=1))
        xin = ctx.enter_context(tc.tile_pool(name="xin", bufs=2))
        stag = ctx.enter_context(tc.tile_pool(name="stag", bufs=1))
        smx = ctx.enter_context(tc.tile_pool(name="smx", bufs=3))
        s2p = ctx.enter_context(tc.tile_pool(name="s2p", bufs=2))
        outp = ctx.enter_context(tc.tile_pool(name="outp", bufs=2))
        psA = ctx.enter_context(tc.tile_pool(name="psA", bufs=NSLA, space="PSUM"))
        psSp = ctx.enter_context(tc.tile_pool(
            name="psSp", bufs=int(os.environ.get("NSLS", "2")), space="PSUM"))
        psFp = ctx.enter_context(tc.tile_pool(
            name="psFp", bufs=int(os.environ.get("NSLF", "1")), space="PSUM"))
        psVp = ctx.enter_context(tc.tile_pool(
            name="psVp", bufs=int(os.environ.get("NSLV", "2")), space="PSUM"))
        SHWO = os.environ.get("SHWO", "0") == "1"  # outproj shares psA pool
        psOp = None if SHWO else ctx.enter_context(
            tc.tile_pool(name="psOp", bufs=1, space="PSUM"))

        wq_sb = consts.tile([128, 8, 3 * C], SD)
        nc.sync.dma_start(out=wq_sb, in_=wqkvT_d.rearrange("(ci p) f -> p ci f", p=128))
        wo_sb = consts.tile([128, 8, C], SD)
        nc.sync.dma_start(out=wo_sb, in_=woutT_d.rearrange("(ci p) f -> p ci f", p=128))
        bcols_sb = consts.tile([128, 24], F32)
        nc.sync.dma_start(out=bcols_sb, in_=bcols_d)
        borep_sb = consts.tile([128, C], BF16)
        nc.sync.dma_start(out=borep_sb, in_=borep_d)
        maskT_sb = consts.tile([128, 128], BF16)
        nc.sync.dma_start(out=maskT_sb, in_=maskP_d)
        maskT4 = maskT_sb.unsqueeze(1).broadcast_to([128, 4, 128])
        idq = consts.tile([128, 128], F32)
        make_identity(nc, idq)
        idS = consts.tile([128, 128], SD)
        nc.vector.tensor_copy(idS, idq)

        K_bd = [stag.tile([128, NPAIR6, 16, 8], SD, name=f"Kbd{i}")
                for i in range(2)]
        Q_bd = [stag.tile([128, NPAIR6, 2, 16, 8], SD, name=f"Qbd{i}")
                for i in range(2)]
        V_AB = stag.tile([128, NG6, 16, 8], SD, name="V_AB")
        onesC = consts.tile([128, 64], BF16)
        nc.vector.memset(onesC, 1.0)
        for qb in Q_bd:
            for par in range(2):
                nc.vector.memset(qb[64 * (1 - par):64 * (1 - par) + 64, :, par],
                                 0.0)
        # persistent V8 quad tiles [128, 4(g), 128]: cols 64:128 ones
        v8q = [stag.tile([128, 4, 128], SD, name=f"v8q{i}") for i in range(2)]
        for e in v8q:
            for g in range(4):
                nc.vector.tensor_copy(e[:, g, 64:128], onesC)

        xT_r = xT_d.rearrange("(ci p) t -> p ci t", p=128)

        def emit_xload(piv):
            x_sb = xin.tile([128, 8, SB6], SD)
            nc.sync.dma_start(out=x_sb, in_=xT_r[:, :, bass.ds(piv, SB6)])
            return x_sb

        def v_half(hb):
            return V_AB[64 * hb:64 * hb + 64]

        ECNT = [0]

        def evict(dst, src, bias):
            if ECNT[0] % 2 == 0:
                nc.vector.tensor_scalar_add(dst, src, bias)
            else:
                nc.scalar.activation(dst, src, Ident, bias=bias)
            ECNT[0] += 1

        def emit_proj_chunk(x_sb, co, hb):
            psC1 = psA.tile([128, SB6], F32, name="pa")
            for ci in range(8):
                nc.tensor.matmul(psC1, wq_sb[:, ci, co * 128:(co + 1) * 128],
                                 x_sb[:, ci, :], start=(ci == 0), stop=(ci == 7))
            kind, c = co // 8, co % 8
            hslot = 2 * c
            for dlt in range(2):
                bias = bcols_sb[64 * dlt:64 * dlt + 64, co:co + 1]
                if kind == 2:
                    src = psC1[64 * dlt:64 * dlt + 64, :].rearrange(
                        "p (g t) -> p g t", g=NG6)
                    evict(v_half(hb)[:, :, hslot + dlt, :], src, bias)
                    continue
                srcg = psC1[64 * dlt:64 * dlt + 64, :].rearrange(
                    "p (pr par t) -> p pr par t", par=2, t=8)
                for par in range(2):
                    src = srcg[:, :, par, :]
                    if kind == 0:
                        dst = Q_bd[hb][64 * par:64 * par + 64, :, par,
                                       hslot + dlt, :]
                    else:
                        dst = K_bd[hb][64 * par:64 * par + 64, :,
                                       hslot + dlt, :]
                    evict(dst, src, bias)

        def emit_attn_q1(qi, hb):
            """quad qi (groups 4qi..4qi+3 = pairs 2qi, 2qi+1): scores,
            transposes, fused exp/mask, V8 copy."""
            vb = 64 * hb
            psS = psSp.tile([128, 2, 256], F32)
            psV = psVp.tile([128, 4, 64], SD)
            for p in range(2):
                pr = 2 * qi + p
                nc.tensor.matmul(psS[:, p], K_bd[hb][:, pr], Q_bd[hb][:, pr],
                                 start=True, stop=True)
            for g in range(4):
                nc.tensor.transpose(psV[:, g, :], v_half(hb)[:, 4 * qi + g, :, :],
                                    idS[vb:vb + 64, vb:vb + 64])
            es = smx.tile([128, 4, 128], SD, name="es")
            nc.scalar.activation(es, psS.rearrange("p a (b c) -> p (a b) c", b=2),
                                 Exp, scale=0.125)
            nc.gpsimd.tensor_mul(es, es, maskT4)
            V8sb = v8q[qi % 2]
            if qi % 2 == 0:
                nc.vector.tensor_copy(V8sb[:, :, 0:64], psV)
            else:
                nc.scalar.copy(V8sb[:, :, 0:64], psV)
            return es, V8sb

        def emit_attn_q2(qi, es, V8sb, S2):
            psF = psFp.tile([128, 4, 16, 8], F32)
            for g in range(4):
                nc.tensor.matmul(psF[:, g], V8sb[:, g, :], es[:, g, :],
                                 start=True, stop=True)
            rZB = smx.tile([64, 4, 16, 8], F32, name="rZB")
            nc.vector.reciprocal(rZB, psF[64:128])
            for dlt in range(2):
                src = psF[0:64].rearrange("p g h t -> p h g t")[
                    :, 8 * dlt:8 * dlt + 8, :, :]
                rzs = rZB.rearrange("p g h t -> p h g t")[
                    :, 8 * dlt:8 * dlt + 8, :, :]
                dst = S2[64 * dlt:64 * dlt + 64].rearrange(
                    "p c (gb gq t) -> p c gb gq t", gb=4, gq=4)[:, :, qi % 4]
                nc.vector.tensor_mul(dst, src, rzs)

        def emit_outproj(S2, oiv, iss):
            for nh in range(2):
                psO = (psA.tile([128, 512], F32, name="pa") if SHWO
                       else psOp.tile([128, 512], F32))
                for c in range(8):
                    nc.tensor.matmul(psO, S2[:, c, :],
                                     wo_sb[:, c, 512 * nh:512 * nh + 512],
                                     start=(c == 0), stop=(c == 7))
                outsb = outp.tile([128, 512], F32, name="outsb")
                nc.scalar.copy(outsb, psO)
                nc.gpsimd.tensor_add(outsb, outsb,
                                     borep_sb[:, 512 * nh:512 * nh + 512])
                nc.sync.dma_start(
                    out=out_d[bass.ds(oiv + SS6 * iss, SS6),
                              bass.ds(512 * nh, 512)], in_=outsb)

        def emit_part(attn_oiv, attn_hb, proj_piv, proj_hb):
            """16 quads of attention woven with 24 projection chunks."""
            x_sb = emit_xload(proj_piv) if proj_piv is not None else None
            S2 = None
            dpo = None
            for qi in range(16):
                if attn_oiv is not None:
                    if qi % 4 == 0:
                        S2 = s2p.tile([128, 8, SS6], SD, name="S2")
                    pend = emit_attn_q1(qi, attn_hb)
                    if dpo is not None:
                        emit_outproj(dpo[0], attn_oiv, dpo[1])
                        dpo = None
                if x_sb is not None:
                    for co in range((3 * qi) // 2, (3 * (qi + 1)) // 2):
                        emit_proj_chunk(x_sb, co, proj_hb)
                if attn_oiv is not None:
                    emit_attn_q2(qi, pend[0], pend[1], S2)
                    if qi % 4 == 3:
                        dpo = (S2, qi // 4)
            if dpo is not None:
                emit_outproj(dpo[0], attn_oiv, dpo[1])

        assert tok % (2 * SB6) == 0 and tok >= 2 * SB6
        emit_part(None, None, 0, 0)
        if tok > 2 * SB6 and static_loop:
            for iv in range(0, tok - 2 * SB6, 2 * SB6):
                emit_part(iv, 0, iv + SB6, 1)
                emit_part(iv + SB6, 1, iv + 2 * SB6, 0)
        elif tok > 2 * SB6 and reps == 1:
            with tc.For_i(0, tok - 2 * SB6, 2 * SB6,
                          hint_engines=(mybir.EngineType.PE, mybir.EngineType.DVE,
                                        mybir.EngineType.Activation)) as iv:
                emit_part(iv, 0, iv + SB6, 1)
                emit_part(iv + SB6, 1, iv + 2 * SB6, 0)
        elif tok > 2 * SB6:
            with tc.For_i(0, reps, 1) as _rep:
                with tc.For_i(0, tok - 2 * SB6, 2 * SB6,
                              hint_engines=(mybir.EngineType.PE,
                                            mybir.EngineType.DVE,
                                            mybir.EngineType.Activation)) as iv:
                    emit_part(iv, 0, iv + SB6, 1)
                    emit_part(iv + SB6, 1, iv + 2 * SB6, 0)
        last = tok - 2 * SB6
        emit_part(last, 0, tok - SB6, 1)
        emit_part(tok - SB6, 1, None, None)
    nc.compile()
    return nc


def _round_f32r(a):
    """Round fp32 to the f32r grid (drop 12 mantissa bits, round-to-nearest)."""
    b = np.ascontiguousarray(a, dtype=np.float32).view(np.uint32)
    b = ((b + (1 << 11)) >> 12) << 12
    return b.view(np.float32)


def _wcast(a, mode):
    if mode == "bf16":
        return np.ascontiguousarray(a.astype(ml_dtypes.bfloat16))
    if mode == "f32r":
        return _round_f32r(np.ascontiguousarray(a, dtype=np.float32))
    return np.ascontiguousarray(a, dtype=np.float32)


def _host_prep(x, w_qkv, b_qkv, w_out, b_out, mode=KMODE):
    d = np.arange(D)
    perm_q = (192 * np.arange(H)[:, None] + d[None, :]).reshape(-1)
    perm = np.concatenate([perm_q, perm_q + 64, perm_q + 128])
    wqkvT = np.ascontiguousarray(w_qkv[perm, :].T, dtype=np.float32)
    bcols = np.ascontiguousarray(
        b_qkv[perm].reshape(24, 128).T, dtype=np.float32)
    # out-proj row perm: S2 row 128c+64dlt+d holds feature 64*(8dlt+c)+d
    co, dl = np.arange(8), np.arange(2)
    perm_o = (64 * (8 * dl[None, :, None] + co[:, None, None])
              + d[None, None, :]).reshape(-1)
    woutT = np.ascontiguousarray(w_out.T[perm_o, :], dtype=np.float32)
    borep = np.ascontiguousarray(
        np.broadcast_to(b_out[None, :], (128, C)), dtype=np.float32)
    maskB = np.tile((np.arange(128)[:, None] % 8
                     == np.arange(128)[None, :] % 8).astype(np.float32), (1, 4))
    # in-group mask [128 rows=(hk,tk), (hq, tq)]: keep tk==tq
    maskP = np.ascontiguousarray(
        (np.arange(128)[:, None] % 8 == np.arange(128)[None, :] % 8
         ).astype(ml_dtypes.bfloat16))
    borep16 = np.ascontiguousarray(borep.astype(ml_dtypes.bfloat16))
    maskP2 = np.ascontiguousarray(np.tile(maskP, (1, 2)))
    xT = np.ascontiguousarray(x.T, dtype=np.float32)
    if XBF:
        xT16 = np.ascontiguousarray(xT.astype(ml_dtypes.bfloat16))
    else:
        xT16 = _wcast(xT, mode)
    xT = _wcast(xT, mode)
    wqkvT = _wcast(wqkvT, mode)
    woutT = _wcast(woutT, mode)
    if mode == "bf16":
        maskB = np.ascontiguousarray(maskB.astype(ml_dtypes.bfloat16))
    return dict(xT=xT, xT16=xT16, wqkvT=wqkvT, bcols=bcols, woutT=woutT,
                borep=borep, borep16=borep16, maskB=maskB, maskP=maskP,
                maskP2=maskP2)


_cache = {}


def kernel(x, w_qkv, b_qkv, w_out, b_out, _trace=False, _tmpdir=None):
    x = np.asarray(x)
    n = x.shape[0]
    tok = n // N_CORES
    pipe = os.environ.get("PIPE", "6")
    hp = _host_prep(
        np.asarray(x), np.asarray(w_qkv), np.asarray(b_qkv),
        np.asarray(w_out), np.asarray(b_out),
        mode="bf16" if pipe in ("5", "6") else KMODE)
    key = (tok, KMODE, pipe)
    if key not in _cache:
        _cache[key] = {"6": build_pipe6, "5": build_pipe5,
                       "4": build_pipe4,
                       "1": build_pipe}.get(pipe, build)(tok)
    nc = _cache[key]
    if pipe in ("5", "6"):
        shared = dict(wqkvT=hp["wqkvT"], woutT=hp["woutT"], bcols=hp["bcols"],
                      borep=hp["borep16"], maskP=hp["maskP"])
        xT = hp["xT"]
    elif pipe == "4":
        xT = hp["xT16"]
        shared = dict(wqkvT=hp["wqkvT"], woutT=hp["woutT"], bcols=hp["bcols"],
                      borep=hp["borep16"], maskP=hp["maskP"])
    elif pipe == "1":
        xT = hp["xT16"]
        shared = dict(wqkvT=hp["wqkvT"], woutT=hp["woutT"], bcols=hp["bcols"],
                      borep=hp["borep16"], maskP=hp["maskP"])
    else:
        xT = hp["xT"]
        shared = dict(wqkvT=hp["wqkvT"], woutT=hp["woutT"], bcols=hp["bcols"],
                      borep=hp["borep"], maskB=hp["maskB"])
    in_maps = [dict(xT=np.ascontiguousarray(xT[:, i * tok:(i + 1) * tok]), **shared)
               for i in range(N_CORES)]
    res = run_bass_kernel_spmd(nc, in_maps, core_ids=list(range(N_CORES)),
                               trace=_trace, tmpdir=_tmpdir)
    out = np.concatenate([res.results[i]["out"] for i in range(N_CORES)], axis=0)
    kernel.last_results = res
    mod = sys.modules[__name__]
    mod.last_nc = nc
    mod.last_in_maps = in_maps
    mod.build_current = {"6": build_pipe6, "5": build_pipe5,
                         "4": build_pipe4,
                         "1": build_pipe}.get(pipe, build)
    mod.last_step = 1024 if pipe == "6" else 512
    return out



# revision 30
# speedup vs baseline: 1.0958x; 1.0958x over previous
"""Trainium2 Bass kernel for nn_MultiHeadAttention_72189810312078.

Computation (per token): qkv = x @ w_qkv.T + b_qkv; per-token attention over
the 16 heads with 16x16 score matrices; out = attn_out @ w_out.T + b_out.

Data-parallel over 8 NeuronCores (8192 tokens each). Host pre-transposes x
to xT [1024, N] so the channel (contraction) dim lands on SBUF partitions.

Active build: build_pipe6 (PIPE=6, all-bf16). HW-measured shape facts that
drive it (micro.py, this backend):
  - bf16 matmuls sustain ~0.38-0.39 ns/output-column at K=128 regardless of
    moving width (LDWEIGHTS does NOT serialize); f32r is ~10-20% slower
    (wider SBUF streams); K=64 matmuls run ~2x slower per column - so every
    matmul is shaped K=128.
  - fp8e4 DoubleRow gives 2.2x MAC rate but fails the 2e-2 gate (~4.4e-2);
    bf16 end-to-end lands at 4.3e-3.
Structure (per 512-token superblock, software-pipelined with the NEXT
superblock's projection via ping-pong staging):
  1) qkv projection: 24 feature chunks x 8 K-chunks, moving=512; bias fused
     into PSUM->staging evictions (DVE/ScalarE alternating). K staging is
     BLOCK-DIAG: even groups' d-dim on partitions 0:64, odd on 64:128; Q
     staging zero-padded block-diag (off-par halves zeroed once), so Q/K
     evictions split into even/odd partition-shifted writes.
  2) scores: ONE K=128 matmul per group-PAIR (block-diag stationary
     K_bd[:, pr], moving Q_bd[:, pr] 256 wide) -> both groups' [128,128]
     score blocks, no garbage columns.
  3) attention in QUADS (4 groups): ONE ScalarE exp [128,512-els], ONE
     GpSimd cross-token mask-mul, ONE V^T->SBUF copy, ONE DVE reciprocal
     per quad; 4 PE transposes build V8 [(hk,t) x (64 V^T | 64 ones)]; 4
     per-group attnV matmuls (K=128, N=128) whose ones-columns produce the
     softmax denominator rows in psF[64:128] for free; S2 pack fuses the
     normalization (tensor_mul by reciprocal) on DVE.
  4) out-projection per 128-token stage: 16 matmuls moving=512 against
     host-permuted w_out; dedicated PSUM bank (sharing the projection pool
     serializes the pipeline and costs 2x!).
PSUM banks (dep tracking is bank-granular): psA x2 (proj), psS x2 (scores),
psF x1 (attnV), psV x2 (transposes), psO x1 (out-proj) = 8.

Max rel err vs fp32 reference ~4.3e-3 (bf16 rounding).
Legacy builds kept for A/B: build (v1), build_pipe (v3 all-f32r),
build_pipe4 (v4), build_pipe5 (v5 bf16 SB=256).
"""

import os
import sys
from contextlib import ExitStack, nullcontext

sys.path.insert(0, "/opt/trn_rl_repo")

import numpy as np
import ml_dtypes

import concourse.bass as bass  # noqa: E402
import concourse.bacc as bacc  # noqa: E402
import concourse.tile as tile  # noqa: E402
from concourse import mybir  # noqa: E402
from concourse.bass_utils import run_bass_kernel_spmd  # noqa: E402
from concourse.masks import make_identity  # noqa: E402

F32 = mybir.dt.float32
F32R = mybir.dt.float32r
BF16 = mybir.dt.bfloat16

N_CORES = 8
H, D, C = 16, 64, 1024
SB = 256   # tokens per superblock (projection moving dim)
SS = 128   # tokens per attention sub-stage / out-projection block
NG = SB // 8   # token groups per superblock (32)

KMODE = os.environ.get("KMODE", "f32r")
GPS = int(os.environ.get("GPS", "3"))  # bitmask: 1=memset, 2=mask-mul, 4=bias
Exp = mybir.ActivationFunctionType.Exp
Copy = mybir.ActivationFunctionType.Copy
Ident = mybir.ActivationFunctionType.Identity


def _dtypes(mode):
    """-> (WD projection-weight dtype, SD attention staging, AD S2)."""
    if mode == "bf16":
        return BF16, BF16, BF16
    if mode == "f32r":
        return F32R, F32R, F32R
    return F32, F32, F32


XBF = os.environ.get("XBF", "0") == "1"   # x streamed in bf16 (moving operand)


def build(tok, mode=KMODE, static_loop=False):
    WD, SD, AD = _dtypes(mode)

    nc = bacc.Bacc("TRN2", target_bir_lowering=False, debug=False,
                   enable_asserts=True, num_devices=N_CORES)
    xT_d = nc.dram_tensor("xT", [C, tok], WD, kind="ExternalInput").ap()
    wqkvT_d = nc.dram_tensor("wqkvT", [C, 3 * C], WD, kind="ExternalInput").ap()
    woutT_d = nc.dram_tensor("woutT", [C, C], WD, kind="ExternalInput").ap()
    bcols_d = nc.dram_tensor("bcols", [128, 24], F32, kind="ExternalInput").ap()
    borep_d = nc.dram_tensor("borep", [128, C], F32, kind="ExternalInput").ap()
    maskB_d = nc.dram_tensor("maskB", [128, 512], SD, kind="ExternalInput").ap()
    out_d = nc.dram_tensor("out", [tok, C], F32, kind="ExternalOutput").ap()

    with tile.TileContext(nc) as tc, ExitStack() as ctx:
        consts = ctx.enter_context(tc.tile_pool(name="consts", bufs=1))
        xin = ctx.enter_context(tc.tile_pool(name="xin", bufs=2))
        stag = ctx.enter_context(tc.tile_pool(name="stag", bufs=1))
        smx = ctx.enter_context(tc.tile_pool(name="smx", bufs=2))
        s2p = ctx.enter_context(tc.tile_pool(name="s2p", bufs=2))
        outp = ctx.enter_context(tc.tile_pool(name="outp", bufs=2))
        psA = ctx.enter_context(tc.tile_pool(name="psA", bufs=2, space="PSUM"))
        psSp = ctx.enter_context(tc.tile_pool(name="psSp", bufs=2, space="PSUM"))
        psVp = ctx.enter_context(tc.tile_pool(name="psVp", bufs=1, space="PSUM"))
        psC2p = ctx.enter_context(tc.tile_pool(name="psC2p", bufs=1, space="PSUM"))
        psTp = ctx.enter_context(tc.tile_pool(name="psTp", bufs=1, space="PSUM"))
        psOp = ctx.enter_context(tc.tile_pool(name="psOp", bufs=1, space="PSUM"))

        # ---- constants ----
        wq_sb = consts.tile([128, 8, 3 * C], WD)
        nc.sync.dma_start(out=wq_sb, in_=wqkvT_d.rearrange("(ci p) f -> p ci f", p=128))
        wo_sb = consts.tile([128, 8, C], WD)
        nc.sync.dma_start(out=wo_sb, in_=woutT_d.rearrange("(ci p) f -> p ci f", p=128))
        bcols_sb = consts.tile([128, 24], F32)
        nc.sync.dma_start(out=bcols_sb, in_=bcols_d)
        borep_sb = consts.tile([128, C], F32)
        nc.sync.dma_start(out=borep_sb, in_=borep_d)
        maskB_sb = consts.tile([128, 512], SD)
        nc.sync.dma_start(out=maskB_sb, in_=maskB_d)
        idq = consts.tile([128, 128], F32)
        make_identity(nc, idq)
        if SD is F32:
            idS = idq
        else:
            idS = consts.tile([128, 128], SD)
            nc.vector.tensor_copy(idS, idq)
        if AD is F32:
            idr = idq
        elif AD is SD:
            idr = idS
        else:
            idr = consts.tile([128, 128], AD)
            nc.vector.tensor_copy(idr, idq)

        ecnt = 0  # evict-engine round robin

        def evict_copy(dst, src):
            nonlocal ecnt
            if ecnt % 2 == 0:
                nc.vector.tensor_copy(dst, src)
            else:
                nc.scalar.copy(dst, src)
            ecnt += 1

        xT_r = xT_d.rearrange("(ci p) t -> p ci t", p=128)
        if static_loop:
            loop_iter = [(nullcontext(iv), iv) for iv in range(0, tok, SB)]
        else:
            fc = tc.For_i(0, tok, SB,
                          hint_engines=(mybir.EngineType.PE,
                                        mybir.EngineType.DVE))
            loop_iter = [(fc, None)]
        for _ctx, _iv in loop_iter:
          with _ctx as _cv:
            iv = _iv if _iv is not None else _cv
            x_sb = xin.tile([128, 8, SB], WD)
            nc.sync.dma_start(out=x_sb, in_=xT_r[:, :, bass.ds(iv, SB)])

            # staging: T1 rows 64:128 = Q; T2 rows 64:128 = K, rows 0:64 = V
            T1 = stag.tile([128, NG, 16, 8], SD, name="T1")
            T2 = stag.tile([128, NG, 16, 8], SD, name="T2")

            # ---- qkv projection + scatter-evict (bias fused / on gpsimd) ----
            for co in range(24):
                psC1 = psA.tile([128, SB], F32)
                for ci in range(8):
                    nc.tensor.matmul(psC1, wq_sb[:, ci, co * 128:(co + 1) * 128],
                                     x_sb[:, ci, :], start=(ci == 0),
                                     stop=(ci == 7))
                kind, c = co // 8, co % 8
                for dlt in range(2):
                    src = psC1[64 * dlt:64 * dlt + 64, :].rearrange(
                        "p (g t) -> p g t", g=NG)
                    hslot = 2 * c + dlt
                    if kind == 0:
                        dst = T1[64:128, :, hslot, :]
                    elif kind == 1:
                        dst = T2[64:128, :, hslot, :]
                    else:
                        dst = T2[0:64, :, hslot, :]
                    bias = bcols_sb[64 * dlt:64 * dlt + 64, co:co + 1]
                    if dlt == 0:
                        # DVE evict with fused bias add
                        nc.vector.tensor_scalar_add(dst, src, bias)
                    elif GPS & 4:
                        # ACT plain evict, bias added SBUF-side on idle gpsimd
                        nc.scalar.copy(dst, src)
                        nc.gpsimd.tensor_scalar_add(dst, dst, bias)
                    else:
                        nc.vector.tensor_scalar_add(dst, src, bias)

            # ---- attention (8 batches of 4 groups) + out-proj per 128 tok ----
            for iss in range(2):
                S2 = s2p.tile([128, 8, SS], AD)
                for b4 in range(4 * iss, 4 * iss + 4):
                    psS = psSp.tile([128, 512], F32)
                    psV = psVp.tile([128, 4, 64], SD)
                    for j in range(4):
                        g = 4 * b4 + j
                        nc.tensor.matmul(psS[:, 128 * j:128 * j + 128],
                                         T2[64:128, g, :, :], T1[64:128, g, :, :],
                                         start=True, stop=True)
                        nc.tensor.transpose(psV[:, j, :], T2[0:64, g, :, :],
                                            idS[0:64, 0:64])
                    es4 = smx.tile([128, 512], SD)
                    nc.scalar.activation(es4, psS, Exp, scale=0.125)
                    if GPS & 2:
                        nc.gpsimd.tensor_mul(es4, es4, maskB_sb)
                    else:
                        nc.vector.tensor_mul(es4, es4, maskB_sb)
                    V8sb = smx.tile([128, 4, 66], SD)
                    nc.scalar.copy(V8sb[:, :, 0:64], psV)
                    if GPS & 1:
                        nc.gpsimd.memset(V8sb[:, :, 64:65], 1.0)
                    else:
                        nc.vector.memset(V8sb[:, :, 64:65], 1.0)
                    psC2 = psC2p.tile([128, 4, 66], F32)
                    for j in range(4):
                        nc.tensor.matmul(psC2[:, j, 0:65],
                                         es4[:, 128 * j:128 * j + 128],
                                         V8sb[:, j, 0:65], start=True, stop=True)
                    rec4 = smx.tile([128, 4], F32)
                    nc.vector.reciprocal(rec4, psC2[:, :, 64:65])
                    attno = smx.tile([128, 4, 64], AD)
                    for j in range(4):
                        if j % 2 == 0:
                            nc.scalar.activation(attno[:, j, :], psC2[:, j, 0:64],
                                                 Copy, scale=rec4[:, j:j + 1])
                        else:
                            nc.vector.tensor_scalar_mul(attno[:, j, :],
                                                        psC2[:, j, 0:64],
                                                        rec4[:, j:j + 1])
                    psT = psTp.tile([64, 4, 128], AD)
                    for j in range(4):
                        nc.tensor.transpose(psT[:, j, :], attno[:, j, :], idr)
                    # S2 pack: head slots 8*dlt..8*dlt+7 -> S2 rows 64*dlt+d
                    for dlt in range(2):
                        src = psT[:, :, 64 * dlt:64 * dlt + 64].rearrange(
                            "p j (h t) -> p j h t", h=8)
                        dst = S2[64 * dlt:64 * dlt + 64].rearrange(
                            "p c (gb gj t) -> p gj c gb t", gb=4, gj=4)[
                                :, :, :, b4 % 4, :]
                        evict_copy(dst, src)

                # ---- out-projection for this 128-token block ----
                outsb = outp.tile([128, C], F32)
                for nh in range(2):
                    psO = psOp.tile([128, 512], F32)
                    for c in range(8):
                        nc.tensor.matmul(psO, S2[:, c, :],
                                         wo_sb[:, c, 512 * nh:512 * nh + 512],
                                         start=(c == 0), stop=(c == 7))
                    nc.vector.tensor_add(outsb[:, 512 * nh:512 * nh + 512], psO,
                                         borep_sb[:, 512 * nh:512 * nh + 512])
                nc.sync.dma_start(out=out_d[bass.ds(iv + SS * iss, SS), :],
                                  in_=outsb)

    nc.compile()
    return nc


def build_pipe(tok, mode=KMODE, static_loop=False, reps=1):
    """Software-pipelined build v3: all-f32r matmuls (self-loading weights, no
    standalone LDWEIGHTS), attention via group-PAIR matmuls so every PE op has
    a 256-wide moving operand (f32r fast path):
      - scores: per pair (gA,gB), two matmuls K_g^T @ [Q_gA|Q_gB] (256 moving)
      - exp on ScalarE -> es (bf16); pair mask (kills cross-group and
        cross-token terms) on GpSimd
      - attnV flipped: psF[d(+Z row), QpairCols] = sum_X V8_X^T @ es_X with the
        V8 ones-column producing the softmax denominator row; S2 packed
        directly from psF (no output transpose)
      - denominators: Z row gathered to zbuf, PE-transposed to per-token
        column, reciprocal on DVE, applied as per-partition scale during the
        out-projection eviction on ScalarE.
    Projection of superblock k+1 overlaps attention of superblock k."""
    WD, SD, AD = _dtypes(mode)
    XD = BF16 if XBF else WD

    nc = bacc.Bacc("TRN2", target_bir_lowering=False, debug=False,
                   enable_asserts=True, num_devices=N_CORES)
    xT_d = nc.dram_tensor("xT", [C, tok], XD, kind="ExternalInput").ap()
    wqkvT_d = nc.dram_tensor("wqkvT", [C, 3 * C], WD, kind="ExternalInput").ap()
    woutT_d = nc.dram_tensor("woutT", [C, C], WD, kind="ExternalInput").ap()
    bcols_d = nc.dram_tensor("bcols", [128, 24], F32, kind="ExternalInput").ap()
    borep_d = nc.dram_tensor("borep", [128, C], BF16, kind="ExternalInput").ap()
    maskP_d = nc.dram_tensor("maskP", [128, 128], BF16, kind="ExternalInput").ap()
    out_d = nc.dram_tensor("out", [tok, C], F32, kind="ExternalOutput").ap()

    PB = [int(v) for v in os.environ.get("PB", "1,2,2,2,2,2").split(",")]
    PRJ = int(os.environ.get("PRJ", "3"))
    with tile.TileContext(nc) as tc, ExitStack() as ctx:
        consts = ctx.enter_context(tc.tile_pool(name="consts", bufs=1))
        xin = ctx.enter_context(tc.tile_pool(name="xin", bufs=PB[0]))
        stag = ctx.enter_context(tc.tile_pool(name="stag", bufs=1))
        smx = ctx.enter_context(tc.tile_pool(name="smx", bufs=PB[1]))
        s2p = ctx.enter_context(tc.tile_pool(name="s2p", bufs=PB[2]))
        outp = ctx.enter_context(tc.tile_pool(name="outp", bufs=PB[3]))
        psA = ctx.enter_context(tc.tile_pool(name="psA", bufs=PB[4], space="PSUM"))
        psSp = ctx.enter_context(tc.tile_pool(name="psSp", bufs=PB[5], space="PSUM"))
        psVp = ctx.enter_context(tc.tile_pool(name="psVp", bufs=1, space="PSUM"))
        psFp = ctx.enter_context(tc.tile_pool(name="psFp", bufs=2, space="PSUM"))
        psOp = ctx.enter_context(tc.tile_pool(name="psOp", bufs=1, space="PSUM"))

        wq_sb = consts.tile([128, 8, 3 * C], WD)
        nc.sync.dma_start(out=wq_sb, in_=wqkvT_d.rearrange("(ci p) f -> p ci f", p=128))
        wo_sb = consts.tile([128, 8, C], WD)
        nc.sync.dma_start(out=wo_sb, in_=woutT_d.rearrange("(ci p) f -> p ci f", p=128))
        bcols_sb = consts.tile([128, 24], F32)
        nc.sync.dma_start(out=bcols_sb, in_=bcols_d)
        borep_sb = consts.tile([128, C], BF16)
        nc.sync.dma_start(out=borep_sb, in_=borep_d)
        maskT_sb = consts.tile([128, 128], BF16)
        nc.sync.dma_start(out=maskT_sb, in_=maskP_d)
        idq = consts.tile([128, 128], F32)
        make_identity(nc, idq)
        if SD is F32:
            idS = idq
        else:
            idS = consts.tile([128, 128], SD)
            nc.vector.tensor_copy(idS, idq)

        # persistent ping-pong staging (half-set hb=0: Q/K upper, V lower)
        Q_AB = stag.tile([128, NG, 16, 8], SD, name="Q_AB")
        K_AB = stag.tile([128, NG, 16, 8], SD, name="K_AB")
        V_AB = stag.tile([128, NG, 16, 8], SD, name="V_AB")
        # persistent es pair tiles [128, X, gp, 16, 8]; the gp != X (cross
        # group) halves are zeroed once here and never written again, so the
        # attnV pair matmuls read zeros there without any recurring masking.
        es_pp = [stag.tile([128, 2, 2, 16, 8], SD, name=f"es{i}")
                 for i in range(2)]
        zeroC = consts.tile([128, 128], BF16)
        nc.vector.memset(zeroC, 0.0)
        onesC = consts.tile([128, 64], BF16)
        nc.vector.memset(onesC, 1.0)
        for e in es_pp:
            nc.vector.tensor_copy(e[:, 0, 1].rearrange("p h t -> p (h t)"), zeroC)
            nc.vector.tensor_copy(e[:, 1, 0].rearrange("p h t -> p (h t)"), zeroC)
        # persistent V8 pair tiles [128, X, 128]: cols 0:64 = V^T (rewritten
        # each pair), cols 64:128 = constant ones so the attnV matmul output
        # rows 64:128 replicate the softmax-denominator row across partitions
        v8_pp = [stag.tile([128, 2, 128], SD, name=f"v8_{i}") for i in range(2)]
        for e in v8_pp:
            nc.vector.tensor_copy(e[:, 0, 64:128], onesC)
            nc.vector.tensor_copy(e[:, 1, 64:128], onesC)

        xT_r = xT_d.rearrange("(ci p) t -> p ci t", p=128)

        def emit_xload(piv):
            x_sb = xin.tile([128, 8, SB], XD)
            nc.sync.dma_start(out=x_sb, in_=xT_r[:, :, bass.ds(piv, SB)])
            return x_sb

        def qk_half(T, hb):
            return T[64 * (1 - hb):64 * (1 - hb) + 64]

        def v_half(hb):
            return V_AB[64 * hb:64 * hb + 64]

        def emit_proj_chunk(x_sb, co, hb):
            psC1 = psA.tile([128, SB], F32)
            for ci in range(8):
                nc.tensor.matmul(psC1, wq_sb[:, ci, co * 128:(co + 1) * 128],
                                 x_sb[:, ci, :], start=(ci == 0), stop=(ci == 7))
            kind, c = co // 8, co % 8
            for dlt in range(2):
                src = psC1[64 * dlt:64 * dlt + 64, :].rearrange(
                    "p (g t) -> p g t", g=NG)
                hslot = 2 * c + dlt
                if kind == 0:
                    dst = qk_half(Q_AB, hb)[:, :, hslot, :]
                elif kind == 1:
                    dst = qk_half(K_AB, hb)[:, :, hslot, :]
                else:
                    dst = v_half(hb)[:, :, hslot, :]
                bias = bcols_sb[64 * dlt:64 * dlt + 64, co:co + 1]
                if (co + dlt) % 2 == 0:
                    nc.vector.tensor_scalar_add(dst, src, bias)
                else:
                    nc.scalar.activation(dst, src, Ident, bias=bias)

        def emit_attn_batch1(b4, hb):
            """scores (pair matmuls) + V transposes + exp + pair-mask for the
            two pairs of batch b4 (groups 4*b4..4*b4+3)."""
            vb = 64 * hb
            pend = []
            for p in range(2):
                gA = 4 * b4 + 2 * p
                psS = psSp.tile([128, 2, 256], F32)
                psV = psVp.tile([128, 2, 64], SD)
                qpair = qk_half(Q_AB, hb)[:, gA:gA + 2, :, :]
                for X in range(2):
                    nc.tensor.matmul(psS[:, X, :],
                                     qk_half(K_AB, hb)[:, gA + X, :, :],
                                     qpair, start=True, stop=True)
                for X in range(2):
                    nc.tensor.transpose(psV[:, X, :], v_half(hb)[:, gA + X, :, :],
                                        idS[vb:vb + 64, vb:vb + 64])
                es = es_pp[p]
                for X in range(2):
                    nc.scalar.activation(es[:, X, X],
                                         psS[:, X, 128 * X:128 * X + 128],
                                         Exp, scale=0.125)
                    nc.gpsimd.tensor_mul(es[:, X, X], es[:, X, X], maskT_sb)
                V8sb = v8_pp[p]
                if p == 0:
                    nc.vector.tensor_copy(V8sb[:, :, 0:64], psV)
                else:
                    nc.scalar.copy(V8sb[:, :, 0:64], psV)
                pend.append((es, V8sb))
            return pend

        def emit_attn_batch2(b4, pend, S2, pairs=(0, 1)):
            for p in pairs:
                es, V8sb = pend[p]
                psF = psFp.tile([128, 2, 16, 8], F32)
                esf = es.rearrange("q x gp h t -> q x (gp h t)")
                for X in range(2):
                    nc.tensor.matmul(psF, V8sb[:, X, :], esf[:, X, :],
                                     start=(X == 0), stop=(X == 1))
                # psF rows 64:128 hold the per-(head, token) softmax
                # denominator row replicated by the ones columns of V8
                rZB = smx.tile([64, 2, 16, 8], F32, name="rZB")
                nc.vector.reciprocal(rZB, psF[64:128])
                for dlt in range(2):
                    src = psF[0:64].rearrange("p gp h t -> p h gp t")[
                        :, 8 * dlt:8 * dlt + 8, :, :]
                    rzs = rZB.rearrange("p gp h t -> p h gp t")[
                        :, 8 * dlt:8 * dlt + 8, :, :]
                    dst = S2[64 * dlt:64 * dlt + 64].rearrange(
                        "p c (gb pp gp t) -> p c gb pp gp t",
                        gb=4, pp=2, gp=2)[:, :, b4 % 4, p, :, :]
                    nc.vector.tensor_mul(dst, src, rzs)

        def emit_outproj(S2, oiv, iss):
            for nh in range(2):
                psO = psOp.tile([128, 512], F32)
                for c in range(8):
                    nc.tensor.matmul(psO, S2[:, c, :],
                                     wo_sb[:, c, 512 * nh:512 * nh + 512],
                                     start=(c == 0), stop=(c == 7))
                outsb = outp.tile([128, 512], F32, name="outsb")
                nc.scalar.copy(outsb, psO)
                nc.gpsimd.tensor_add(outsb, outsb,
                                     borep_sb[:, 512 * nh:512 * nh + 512])
                nc.sync.dma_start(
                    out=out_d[bass.ds(oiv + SS * iss, SS),
                              bass.ds(512 * nh, 512)], in_=outsb)

        def emit_part(attn_oiv, attn_hb, proj_piv, proj_hb):
            """Weave attention of one superblock with projection of another.
            Either may be None (prologue/epilogue)."""
            x_sb = emit_xload(proj_piv) if proj_piv is not None else None
            S2 = None
            dpo = None   # deferred out-projection (S2, iss)
            for b4 in range(8):
                if attn_oiv is not None:
                    if b4 % 4 == 0:
                        S2 = s2p.tile([128, 8, SS], AD, name="S2")
                    pend = emit_attn_batch1(b4, attn_hb)
                    if dpo is not None:
                        emit_outproj(dpo[0], attn_oiv, dpo[1])
                        dpo = None
                if x_sb is not None:
                    for co in range(PRJ * b4, min(PRJ * b4 + PRJ, 24)):
                        emit_proj_chunk(x_sb, co, proj_hb)
                if attn_oiv is not None:
                    emit_attn_batch2(b4, pend, S2)
                    if b4 % 4 == 3:
                        dpo = (S2, b4 // 4)
            if dpo is not None:
                emit_outproj(dpo[0], attn_oiv, dpo[1])

        assert tok % (2 * SB) == 0 and tok >= 2 * SB
        emit_part(None, None, 0, 0)                      # prologue: proj sb0 -> A
        if tok > 2 * SB and static_loop:
            for iv in range(0, tok - 2 * SB, 2 * SB):
                emit_part(iv, 0, iv + SB, 1)             # attn A, proj -> B
                emit_part(iv + SB, 1, iv + 2 * SB, 0)    # attn B, proj -> A
        elif tok > 2 * SB and reps == 1:
            with tc.For_i(0, tok - 2 * SB, 2 * SB,
                          hint_engines=(mybir.EngineType.PE, mybir.EngineType.DVE,
                                        mybir.EngineType.Activation)) as iv:
                emit_part(iv, 0, iv + SB, 1)             # attn A, proj -> B
                emit_part(iv + SB, 1, iv + 2 * SB, 0)    # attn B, proj -> A
        elif tok > 2 * SB:
            with tc.For_i(0, reps, 1) as _rep:
                with tc.For_i(0, tok - 2 * SB, 2 * SB,
                              hint_engines=(mybir.EngineType.PE,
                                            mybir.EngineType.DVE,
                                            mybir.EngineType.Activation)) as iv:
                    emit_part(iv, 0, iv + SB, 1)         # attn A, proj -> B
                    emit_part(iv + SB, 1, iv + 2 * SB, 0)  # attn B, proj -> A
        last = tok - 2 * SB
        emit_part(last, 0, tok - SB, 1)                  # attn A, proj last -> B
        emit_part(tok - SB, 1, None, None)               # attn B
    nc.compile()
    return nc


def build_pipe4(tok, mode=KMODE, static_loop=False, reps=1):
    """v4: like build_pipe (v3) but
      - PSUM bank remap (dep tracking is bank-granular, 8 banks):
        psA 3 bufs (proj, deeper pipelining; v3 had 2), psSp 2, pvf 2
        (V^T transpose target + attnV psF SHARE one 1.5KB tile per pair --
        their accesses are a sequential chain within the pair so the shared
        bank adds no serialization), psOp 1.
      - exp and mask fused across the pair with a strided diag AP
        ([128, 2(stride 384), 128]): ONE ScalarE exp + ONE GpSimd mask-mul
        per pair instead of 2+2.
      - out-proj eviction fused: GpSimd tensor_add(outsb, psO, borep) reads
        PSUM directly (drops the separate ScalarE copy).
    """
    WD, SD, AD = _dtypes(mode)
    XD = BF16 if XBF else WD

    nc = bacc.Bacc("TRN2", target_bir_lowering=False, debug=False,
                   enable_asserts=True, num_devices=N_CORES)
    xT_d = nc.dram_tensor("xT", [C, tok], XD, kind="ExternalInput").ap()
    wqkvT_d = nc.dram_tensor("wqkvT", [C, 3 * C], WD, kind="ExternalInput").ap()
    woutT_d = nc.dram_tensor("woutT", [C, C], WD, kind="ExternalInput").ap()
    bcols_d = nc.dram_tensor("bcols", [128, 24], F32, kind="ExternalInput").ap()
    borep_d = nc.dram_tensor("borep", [128, C], BF16, kind="ExternalInput").ap()
    maskP_d = nc.dram_tensor("maskP", [128, 128], BF16, kind="ExternalInput").ap()
    out_d = nc.dram_tensor("out", [tok, C], F32, kind="ExternalOutput").ap()

    PRJ = int(os.environ.get("PRJ", "3"))
    NSLA = int(os.environ.get("NSLA", "3"))   # proj PSUM bufs
    XBUF = int(os.environ.get("XBUF", "2" if XBF else "1"))
    with tile.TileContext(nc) as tc, ExitStack() as ctx:
        consts = ctx.enter_context(tc.tile_pool(name="consts", bufs=1))
        xin = ctx.enter_context(tc.tile_pool(name="xin", bufs=XBUF))
        stag = ctx.enter_context(tc.tile_pool(name="stag", bufs=1))
        smx = ctx.enter_context(tc.tile_pool(name="smx", bufs=2))
        s2p = ctx.enter_context(tc.tile_pool(name="s2p", bufs=2))
        outp = ctx.enter_context(tc.tile_pool(name="outp", bufs=2))
        psA = ctx.enter_context(tc.tile_pool(name="psA", bufs=NSLA, space="PSUM"))
        psSp = ctx.enter_context(tc.tile_pool(name="psSp", bufs=2, space="PSUM"))
        pvf = ctx.enter_context(tc.tile_pool(name="pvf", bufs=2, space="PSUM"))
        psOp = ctx.enter_context(tc.tile_pool(name="psOp", bufs=1, space="PSUM"))

        wq_sb = consts.tile([128, 8, 3 * C], WD)
        nc.sync.dma_start(out=wq_sb, in_=wqkvT_d.rearrange("(ci p) f -> p ci f", p=128))
        wo_sb = consts.tile([128, 8, C], WD)
        nc.sync.dma_start(out=wo_sb, in_=woutT_d.rearrange("(ci p) f -> p ci f", p=128))
        bcols_sb = consts.tile([128, 24], F32)
        nc.sync.dma_start(out=bcols_sb, in_=bcols_d)
        borep_sb = consts.tile([128, C], BF16)
        nc.sync.dma_start(out=borep_sb, in_=borep_d)
        maskT_sb = consts.tile([128, 128], BF16)
        nc.sync.dma_start(out=maskT_sb, in_=maskP_d)
        maskT2 = maskT_sb.unsqueeze(1).broadcast_to([128, 2, 128])
        idq = consts.tile([128, 128], F32)
        make_identity(nc, idq)
        if SD is F32:
            idS = idq
        else:
            idS = consts.tile([128, 128], SD)
            nc.vector.tensor_copy(idS, idq)

        # persistent ping-pong staging (half-set hb=0: Q/K upper, V lower)
        Q_AB = stag.tile([128, NG, 16, 8], SD, name="Q_AB")
        K_AB = stag.tile([128, NG, 16, 8], SD, name="K_AB")
        V_AB = stag.tile([128, NG, 16, 8], SD, name="V_AB")
        # persistent es pair tiles [128, X, gp, 16, 8]; cross (gp != X) halves
        # zeroed once, never rewritten -> attnV pair matmuls read zeros there.
        es_pp = [stag.tile([128, 2, 2, 16, 8], SD, name=f"es{i}")
                 for i in range(2)]
        zeroC = consts.tile([128, 128], BF16)
        nc.vector.memset(zeroC, 0.0)
        onesC = consts.tile([128, 64], BF16)
        nc.vector.memset(onesC, 1.0)
        for e in es_pp:
            nc.vector.tensor_copy(e[:, 0, 1].rearrange("p h t -> p (h t)"), zeroC)
            nc.vector.tensor_copy(e[:, 1, 0].rearrange("p h t -> p (h t)"), zeroC)
        # persistent V8 pair tiles [128, X, 128]: cols 0:64 = V^T (rewritten
        # each pair), cols 64:128 = ones -> attnV rows 64:128 = softmax denom
        v8_pp = [stag.tile([128, 2, 128], SD, name=f"v8_{i}") for i in range(2)]
        for e in v8_pp:
            nc.vector.tensor_copy(e[:, 0, 64:128], onesC)
            nc.vector.tensor_copy(e[:, 1, 64:128], onesC)

        xT_r = xT_d.rearrange("(ci p) t -> p ci t", p=128)

        def emit_xload(piv):
            x_sb = xin.tile([128, 8, SB], XD)
            nc.sync.dma_start(out=x_sb, in_=xT_r[:, :, bass.ds(piv, SB)])
            return x_sb

        def qk_half(T, hb):
            return T[64 * (1 - hb):64 * (1 - hb) + 64]

        def v_half(hb):
            return V_AB[64 * hb:64 * hb + 64]

        def emit_proj_chunk(x_sb, co, hb):
            psC1 = psA.tile([128, SB], F32)
            for ci in range(8):
                nc.tensor.matmul(psC1, wq_sb[:, ci, co * 128:(co + 1) * 128],
                                 x_sb[:, ci, :], start=(ci == 0), stop=(ci == 7))
            kind, c = co // 8, co % 8
            for dlt in range(2):
                src = psC1[64 * dlt:64 * dlt + 64, :].rearrange(
                    "p (g t) -> p g t", g=NG)
                hslot = 2 * c + dlt
                if kind == 0:
                    dst = qk_half(Q_AB, hb)[:, :, hslot, :]
                elif kind == 1:
                    dst = qk_half(K_AB, hb)[:, :, hslot, :]
                else:
                    dst = v_half(hb)[:, :, hslot, :]
                bias = bcols_sb[64 * dlt:64 * dlt + 64, co:co + 1]
                if (co + dlt) % 2 == 0:
                    nc.vector.tensor_scalar_add(dst, src, bias)
                else:
                    nc.scalar.activation(dst, src, Ident, bias=bias)

        def diag2(ap4):
            """[128, 2, 256]-ish -> diag blocks [128, 2 (stride 384), 128]."""
            flat = ap4.rearrange("p x c -> p (x c)")
            return flat.rearrange("p (q r) -> p q r", q=4)[:, ::3, :]

        def emit_attn_batch1(b4, hb):
            """scores (pair matmuls) + V transposes + fused exp + pair-mask
            for the two pairs of batch b4 (groups 4*b4..4*b4+3)."""
            vb = 64 * hb
            pend = []
            for p in range(2):
                gA = 4 * b4 + 2 * p
                psS = psSp.tile([128, 2, 256], F32)
                vft = pvf.tile([128, 384], F32, name="vf")
                psV = vft[:, 256:384].rearrange("p (x v) -> p x v", x=2)
                if SD is not F32:
                    psV = psV.bitcast(SD)
                qpair = qk_half(Q_AB, hb)[:, gA:gA + 2, :, :]
                for X in range(2):
                    nc.tensor.matmul(psS[:, X, :],
                                     qk_half(K_AB, hb)[:, gA + X, :, :],
                                     qpair, start=True, stop=True)
                for X in range(2):
                    nc.tensor.transpose(psV[:, X, :], v_half(hb)[:, gA + X, :, :],
                                        idS[vb:vb + 64, vb:vb + 64])
                es = es_pp[p]
                es_diag = diag2(es.rearrange("p x g h t -> p x (g h t)"))
                nc.scalar.activation(es_diag, diag2(psS), Exp, scale=0.125)
                nc.gpsimd.tensor_mul(es_diag, es_diag, maskT2)
                V8sb = v8_pp[p]
                if p == 0:
                    nc.vector.tensor_copy(V8sb[:, :, 0:64], psV)
                else:
                    nc.scalar.copy(V8sb[:, :, 0:64], psV)
                pend.append((es, V8sb, vft))
            return pend

        def emit_attn_batch2(b4, pend, S2, pairs=(0, 1)):
            for p in pairs:
                es, V8sb, vft = pend[p]
                psF = vft[:, 0:256].rearrange("p (g h t) -> p g h t", g=2, h=16)
                esf = es.rearrange("q x gp h t -> q x (gp h t)")
                for X in range(2):
                    nc.tensor.matmul(psF, V8sb[:, X, :], esf[:, X, :],
                                     start=(X == 0), stop=(X == 1))
                # psF rows 64:128: per-(head, token) softmax denominator
                rZB = smx.tile([64, 2, 16, 8], F32, name="rZB")
                nc.vector.reciprocal(rZB, psF[64:128])
                for dlt in range(2):
                    src = psF[0:64].rearrange("p gp h t -> p h gp t")[
                        :, 8 * dlt:8 * dlt + 8, :, :]
                    rzs = rZB.rearrange("p gp h t -> p h gp t")[
                        :, 8 * dlt:8 * dlt + 8, :, :]
                    dst = S2[64 * dlt:64 * dlt + 64].rearrange(
                        "p c (gb pp gp t) -> p c gb pp gp t",
                        gb=4, pp=2, gp=2)[:, :, b4 % 4, p, :, :]
                    nc.vector.tensor_mul(dst, src, rzs)

        def emit_outproj(S2, oiv, iss):
            for nh in range(2):
                psO = psOp.tile([128, 512], F32)
                for c in range(8):
                    nc.tensor.matmul(psO, S2[:, c, :],
                                     wo_sb[:, c, 512 * nh:512 * nh + 512],
                                     start=(c == 0), stop=(c == 7))
                outsb = outp.tile([128, 512], F32, name="outsb")
                nc.scalar.copy(outsb, psO)
                nc.gpsimd.tensor_add(outsb, outsb,
                                     borep_sb[:, 512 * nh:512 * nh + 512])
                nc.sync.dma_start(
                    out=out_d[bass.ds(oiv + SS * iss, SS),
                              bass.ds(512 * nh, 512)], in_=outsb)

        def emit_part(attn_oiv, attn_hb, proj_piv, proj_hb):
            """Weave attention of one superblock with projection of another."""
            x_sb = emit_xload(proj_piv) if proj_piv is not None else None
            S2 = None
            dpo = None   # deferred out-projection (S2, iss)
            for b4 in range(8):
                if attn_oiv is not None:
                    if b4 % 4 == 0:
                        S2 = s2p.tile([128, 8, SS], AD, name="S2")
                    pend = emit_attn_batch1(b4, attn_hb)
                    if dpo is not None:
                        emit_outproj(dpo[0], attn_oiv, dpo[1])
                        dpo = None
                if x_sb is not None:
                    for co in range(PRJ * b4, min(PRJ * b4 + PRJ, 24)):
                        emit_proj_chunk(x_sb, co, proj_hb)
                if attn_oiv is not None:
                    emit_attn_batch2(b4, pend, S2)
                    if b4 % 4 == 3:
                        dpo = (S2, b4 // 4)
            if dpo is not None:
                emit_outproj(dpo[0], attn_oiv, dpo[1])

        assert tok % (2 * SB) == 0 and tok >= 2 * SB
        emit_part(None, None, 0, 0)                      # prologue: proj sb0 -> A
        if tok > 2 * SB and static_loop:
            for iv in range(0, tok - 2 * SB, 2 * SB):
                emit_part(iv, 0, iv + SB, 1)             # attn A, proj -> B
                emit_part(iv + SB, 1, iv + 2 * SB, 0)    # attn B, proj -> A
        elif tok > 2 * SB and reps == 1:
            with tc.For_i(0, tok - 2 * SB, 2 * SB,
                          hint_engines=(mybir.EngineType.PE, mybir.EngineType.DVE,
                                        mybir.EngineType.Activation)) as iv:
                emit_part(iv, 0, iv + SB, 1)             # attn A, proj -> B
                emit_part(iv + SB, 1, iv + 2 * SB, 0)    # attn B, proj -> A
        elif tok > 2 * SB:
            with tc.For_i(0, reps, 1) as _rep:
                with tc.For_i(0, tok - 2 * SB, 2 * SB,
                              hint_engines=(mybir.EngineType.PE,
                                            mybir.EngineType.DVE,
                                            mybir.EngineType.Activation)) as iv:
                    emit_part(iv, 0, iv + SB, 1)         # attn A, proj -> B
                    emit_part(iv + SB, 1, iv + 2 * SB, 0)  # attn B, proj -> A
        last = tok - 2 * SB
        emit_part(last, 0, tok - SB, 1)                  # attn A, proj last -> B
        emit_part(tok - SB, 1, None, None)               # attn B
    nc.compile()
    return nc


def build_pipe5(tok, mode="bf16", static_loop=False, reps=1):
    """v5: all-bf16 + K=128-everywhere matmul shapes (HW-measured: K=64
    matmuls run ~2x slower per output column; bf16 streams beat f32r and
    LDWEIGHTS does NOT serialize on this backend):

      - everything (x, weights, staging, es, S2) in bf16; PSUM f32.
      - scores via BLOCK-DIAG pairs: K staged with even groups' d-dim on
        partitions 0:64 and odd groups' on 64:128 (K_bd[128, pr, 16, 8]);
        Q staged zero-padded block-diag (Q_bd[128, pr, 2, 16, 8], the
        off-diagonal partition halves zeroed once at startup). ONE matmul
        per pair: psS[128, 2*128] = K_bd[:,pr]^T @ Q_bd[:,pr], K=128,
        moving 256 -> both groups' score blocks, no garbage columns.
      - exp: ONE ScalarE activation [128,256] psS -> es_pair bf16 (dense,
        no diag APs); ONE GpSimd mask-mul (broadcast [128,2,128]).
      - attnV per GROUP (K=128, N=128): psF_g = V8_g^T @ es_g with the
        V8 ones-columns producing the softmax denominator rows; psF pair
        halves live side by side in the pvf tile so the pair-granular
        reciprocal + S2 pack from v4 are unchanged.
      - PSUM banks: psA x3 (1 bank ea), psS x2, pvf x2 (psF pair + psV
        transposes share a 1.25KB tile), psO x1 = 8.
      - Q/K projection evictions split even/odd groups (partition-shifted
        writes, 4 small instrs per chunk instead of 2).
    """
    WD = SD = AD = BF16

    nc = bacc.Bacc("TRN2", target_bir_lowering=False, debug=False,
                   enable_asserts=True, num_devices=N_CORES)
    xT_d = nc.dram_tensor("xT", [C, tok], WD, kind="ExternalInput").ap()
    wqkvT_d = nc.dram_tensor("wqkvT", [C, 3 * C], WD, kind="ExternalInput").ap()
    woutT_d = nc.dram_tensor("woutT", [C, C], WD, kind="ExternalInput").ap()
    bcols_d = nc.dram_tensor("bcols", [128, 24], F32, kind="ExternalInput").ap()
    borep_d = nc.dram_tensor("borep", [128, C], BF16, kind="ExternalInput").ap()
    maskP_d = nc.dram_tensor("maskP", [128, 128], BF16, kind="ExternalInput").ap()
    out_d = nc.dram_tensor("out", [tok, C], F32, kind="ExternalOutput").ap()

    PRJ = int(os.environ.get("PRJ", "3"))
    NSLA = int(os.environ.get("NSLA", "3"))
    NPAIR = NG // 2  # 16 pairs per superblock
    with tile.TileContext(nc) as tc, ExitStack() as ctx:
        consts = ctx.enter_context(tc.tile_pool(name="consts", bufs=1))
        xin = ctx.enter_context(tc.tile_pool(name="xin", bufs=2))
        stag = ctx.enter_context(tc.tile_pool(name="stag", bufs=1))
        smx = ctx.enter_context(tc.tile_pool(name="smx", bufs=3))
        s2p = ctx.enter_context(tc.tile_pool(name="s2p", bufs=2))
        outp = ctx.enter_context(tc.tile_pool(name="outp", bufs=2))
        psA = ctx.enter_context(tc.tile_pool(name="psA", bufs=NSLA, space="PSUM"))
        psSp = ctx.enter_context(tc.tile_pool(name="psSp", bufs=2, space="PSUM"))
        pvf = ctx.enter_context(tc.tile_pool(name="pvf", bufs=2, space="PSUM"))
        psOp = ctx.enter_context(tc.tile_pool(name="psOp", bufs=1, space="PSUM"))

        wq_sb = consts.tile([128, 8, 3 * C], WD)
        nc.sync.dma_start(out=wq_sb, in_=wqkvT_d.rearrange("(ci p) f -> p ci f", p=128))
        wo_sb = consts.tile([128, 8, C], WD)
        nc.sync.dma_start(out=wo_sb, in_=woutT_d.rearrange("(ci p) f -> p ci f", p=128))
        bcols_sb = consts.tile([128, 24], F32)
        nc.sync.dma_start(out=bcols_sb, in_=bcols_d)
        borep_sb = consts.tile([128, C], BF16)
        nc.sync.dma_start(out=borep_sb, in_=borep_d)
        maskT_sb = consts.tile([128, 128], BF16)
        nc.sync.dma_start(out=maskT_sb, in_=maskP_d)
        maskT2 = maskT_sb.unsqueeze(1).broadcast_to([128, 2, 128])
        idq = consts.tile([128, 128], F32)
        make_identity(nc, idq)
        idS = consts.tile([128, 128], SD)
        nc.vector.tensor_copy(idS, idq)

        # staging: block-diag K/Q per ping-pong half (full 128 partitions),
        # V keeps the half-partition ping-pong of v3/v4.
        K_bd = [stag.tile([128, NPAIR, 16, 8], SD, name=f"Kbd{i}")
                for i in range(2)]
        Q_bd = [stag.tile([128, NPAIR, 2, 16, 8], SD, name=f"Qbd{i}")
                for i in range(2)]
        V_AB = stag.tile([128, NG, 16, 8], SD, name="V_AB")
        zeroC = consts.tile([128, 128], BF16)
        nc.vector.memset(zeroC, 0.0)
        onesC = consts.tile([128, 64], BF16)
        nc.vector.memset(onesC, 1.0)
        # zero the off-diagonal Q halves once (never rewritten)
        for qb in Q_bd:
            for par in range(2):
                z = qb[64 * (1 - par):64 * (1 - par) + 64, :, par]
                nc.vector.memset(z, 0.0)
        v8_pp = [stag.tile([128, 2, 128], SD, name=f"v8_{i}") for i in range(2)]
        for e in v8_pp:
            nc.vector.tensor_copy(e[:, 0, 64:128], onesC)
            nc.vector.tensor_copy(e[:, 1, 64:128], onesC)

        xT_r = xT_d.rearrange("(ci p) t -> p ci t", p=128)

        def emit_xload(piv):
            x_sb = xin.tile([128, 8, SB], WD)
            nc.sync.dma_start(out=x_sb, in_=xT_r[:, :, bass.ds(piv, SB)])
            return x_sb

        def v_half(hb):
            return V_AB[64 * hb:64 * hb + 64]

        ECNT = [0]

        def evict(dst, src, bias):
            if ECNT[0] % 2 == 0:
                nc.vector.tensor_scalar_add(dst, src, bias)
            else:
                nc.scalar.activation(dst, src, Ident, bias=bias)
            ECNT[0] += 1

        def emit_proj_chunk(x_sb, co, hb):
            psC1 = psA.tile([128, SB], F32)
            for ci in range(8):
                nc.tensor.matmul(psC1, wq_sb[:, ci, co * 128:(co + 1) * 128],
                                 x_sb[:, ci, :], start=(ci == 0), stop=(ci == 7))
            kind, c = co // 8, co % 8
            hslot = 2 * c  # +dlt below
            for dlt in range(2):
                bias = bcols_sb[64 * dlt:64 * dlt + 64, co:co + 1]
                srcg = psC1[64 * dlt:64 * dlt + 64, :].rearrange(
                    "p (pr par t) -> p pr par t", par=2, t=8)
                if kind == 2:
                    src = psC1[64 * dlt:64 * dlt + 64, :].rearrange(
                        "p (g t) -> p g t", g=NG)
                    evict(v_half(hb)[:, :, hslot + dlt, :], src, bias)
                    continue
                for par in range(2):
                    src = srcg[:, :, par, :]
                    if kind == 0:
                        dst = Q_bd[hb][64 * par:64 * par + 64, :, par,
                                       hslot + dlt, :]
                    else:
                        dst = K_bd[hb][64 * par:64 * par + 64, :,
                                       hslot + dlt, :]
                    evict(dst, src, bias)

        def emit_attn_batch1(b4, hb):
            """block-diag pair scores + V^T transposes + fused exp/mask."""
            vb = 64 * hb
            pend = []
            for p in range(2):
                pr = 2 * b4 + p
                gA = 4 * b4 + 2 * p
                psS = psSp.tile([128, 256], F32)
                vft = pvf.tile([128, 320], F32, name="vf")
                psV = vft[:, 256:320].bitcast(SD).rearrange(
                    "p (x v) -> p x v", x=2)
                nc.tensor.matmul(psS, K_bd[hb][:, pr], Q_bd[hb][:, pr],
                                 start=True, stop=True)
                for X in range(2):
                    nc.tensor.transpose(psV[:, X, :], v_half(hb)[:, gA + X, :, :],
                                        idS[vb:vb + 64, vb:vb + 64])
                es = smx.tile([128, 2, 128], SD, name="es")
                nc.scalar.activation(es, psS.rearrange("p (x c) -> p x c", x=2),
                                     Exp, scale=0.125)
                nc.gpsimd.tensor_mul(es, es, maskT2)
                V8sb = v8_pp[p]
                if p == 0:
                    nc.vector.tensor_copy(V8sb[:, :, 0:64], psV)
                else:
                    nc.scalar.copy(V8sb[:, :, 0:64], psV)
                pend.append((es, V8sb, vft))
            return pend

        def emit_attn_batch2(b4, pend, S2, pairs=(0, 1)):
            for p in pairs:
                es, V8sb, vft = pend[p]
                psF = vft[:, 0:256].rearrange("p (g h t) -> p g h t", g=2, h=16)
                for X in range(2):
                    nc.tensor.matmul(psF[:, X], V8sb[:, X, :], es[:, X, :],
                                     start=True, stop=True)
                rZB = smx.tile([64, 2, 16, 8], F32, name="rZB")
                nc.vector.reciprocal(rZB, psF[64:128])
                for dlt in range(2):
                    src = psF[0:64].rearrange("p gp h t -> p h gp t")[
                        :, 8 * dlt:8 * dlt + 8, :, :]
                    rzs = rZB.rearrange("p gp h t -> p h gp t")[
                        :, 8 * dlt:8 * dlt + 8, :, :]
                    dst = S2[64 * dlt:64 * dlt + 64].rearrange(
                        "p c (gb pp gp t) -> p c gb pp gp t",
                        gb=4, pp=2, gp=2)[:, :, b4 % 4, p, :, :]
                    nc.vector.tensor_mul(dst, src, rzs)

        def emit_outproj(S2, oiv, iss):
            for nh in range(2):
                psO = psOp.tile([128, 512], F32)
                for c in range(8):
                    nc.tensor.matmul(psO, S2[:, c, :],
                                     wo_sb[:, c, 512 * nh:512 * nh + 512],
                                     start=(c == 0), stop=(c == 7))
                outsb = outp.tile([128, 512], F32, name="outsb")
                nc.scalar.copy(outsb, psO)
                nc.gpsimd.tensor_add(outsb, outsb,
                                     borep_sb[:, 512 * nh:512 * nh + 512])
                nc.sync.dma_start(
                    out=out_d[bass.ds(oiv + SS * iss, SS),
                              bass.ds(512 * nh, 512)], in_=outsb)

        def emit_part(attn_oiv, attn_hb, proj_piv, proj_hb):
            x_sb = emit_xload(proj_piv) if proj_piv is not None else None
            S2 = None
            dpo = None
            for b4 in range(8):
                if attn_oiv is not None:
                    if b4 % 4 == 0:
                        S2 = s2p.tile([128, 8, SS], AD, name="S2")
                    pend = emit_attn_batch1(b4, attn_hb)
                    if dpo is not None:
                        emit_outproj(dpo[0], attn_oiv, dpo[1])
                        dpo = None
                if x_sb is not None:
                    for co in range(PRJ * b4, min(PRJ * b4 + PRJ, 24)):
                        emit_proj_chunk(x_sb, co, proj_hb)
                if attn_oiv is not None:
                    emit_attn_batch2(b4, pend, S2)
                    if b4 % 4 == 3:
                        dpo = (S2, b4 // 4)
            if dpo is not None:
                emit_outproj(dpo[0], attn_oiv, dpo[1])

        assert tok % (2 * SB) == 0 and tok >= 2 * SB
        emit_part(None, None, 0, 0)
        if tok > 2 * SB and static_loop:
            for iv in range(0, tok - 2 * SB, 2 * SB):
                emit_part(iv, 0, iv + SB, 1)
                emit_part(iv + SB, 1, iv + 2 * SB, 0)
        elif tok > 2 * SB and reps == 1:
            with tc.For_i(0, tok - 2 * SB, 2 * SB,
                          hint_engines=(mybir.EngineType.PE, mybir.EngineType.DVE,
                                        mybir.EngineType.Activation)) as iv:
                emit_part(iv, 0, iv + SB, 1)
                emit_part(iv + SB, 1, iv + 2 * SB, 0)
        elif tok > 2 * SB:
            with tc.For_i(0, reps, 1) as _rep:
                with tc.For_i(0, tok - 2 * SB, 2 * SB,
                              hint_engines=(mybir.EngineType.PE,
                                            mybir.EngineType.DVE,
                                            mybir.EngineType.Activation)) as iv:
                    emit_part(iv, 0, iv + SB, 1)
                    emit_part(iv + SB, 1, iv + 2 * SB, 0)
        last = tok - 2 * SB
        emit_part(last, 0, tok - SB, 1)
        emit_part(tok - SB, 1, None, None)
    nc.compile()
    return nc


def build_pipe6(tok, mode="bf16", static_loop=False, reps=1):
    """v6: v5's all-bf16 + K=128 shapes, restructured to minimize instruction
    count (HW shows ~100ns-class per-instruction sync/sequencer overhead that
    the cost model underestimates):
      - SB=512 token superblocks: projection matmuls go 512-wide (same
        per-column rate, HALF the instruction + LDWEIGHTS count), evictions
        double in size and halve in count.
      - attention in QUADS (4 groups): ONE exp [128,512], ONE mask-mul,
        ONE V^T->SBUF copy, ONE reciprocal per quad; 2 block-diag scores
        matmuls, 4 transposes, 4 attnV matmuls, 2 S2-pack muls.
      - PSUM: psA [128,512] x3 (shared by projection chunks AND the
        out-projection), psS-quad [128,2,256] x2, psF-quad [128,4,16,8] x2,
        psV-quad [128,4,64]bf16 x1 = 8 banks.
    """
    del mode
    SB6, SS6 = 512, 128
    NG6 = SB6 // 8          # 64 groups
    NPAIR6 = NG6 // 2       # 32 pairs
    SD = BF16

    nc = bacc.Bacc("TRN2", target_bir_lowering=False, debug=False,
                   enable_asserts=True, num_devices=N_CORES)
    xT_d = nc.dram_tensor("xT", [C, tok], SD, kind="ExternalInput").ap()
    wqkvT_d = nc.dram_tensor("wqkvT", [C, 3 * C], SD, kind="ExternalInput").ap()
    woutT_d = nc.dram_tensor("woutT", [C, C], SD, kind="ExternalInput").ap()
    bcols_d = nc.dram_tensor("bcols", [128, 24], F32, kind="ExternalInput").ap()
    borep_d = nc.dram_tensor("borep", [128, C], BF16, kind="ExternalInput").ap()
    maskP_d = nc.dram_tensor("maskP", [128, 128], BF16, kind="ExternalInput").ap()
    out_d = nc.dram_tensor("out", [tok, C], F32, kind="ExternalOutput").ap()

    NSLA = int(os.environ.get("NSLA", "2"))
    with tile.TileContext(nc) as tc, ExitStack() as ctx:
        consts = ctx.enter_context(tc.tile_pool(name="consts", bufs=1))
        xin = ctx.enter_context(tc.tile_pool(name="xin", bufs=2))
        stag = ctx.enter_context(tc.tile_pool(name="stag", bufs=1))
        smx = ctx.enter_context(tc.tile_pool(name="smx", bufs=3))
        s2p = ctx.enter_context(tc.tile_pool(name="s2p", bufs=2))
        outp = ctx.enter_context(tc.tile_pool(name="outp", bufs=2))
        psA = ctx.enter_context(tc.tile_pool(name="psA", bufs=NSLA, space="PSUM"))
        psSp = ctx.enter_context(tc.tile_pool(
            name="psSp", bufs=int(os.environ.get("NSLS", "2")), space="PSUM"))
        psFp = ctx.enter_context(tc.tile_pool(
            name="psFp", bufs=int(os.environ.get("NSLF", "1")), space="PSUM"))
        psVp = ctx.enter_context(tc.tile_pool(
            name="psVp", bufs=int(os.environ.get("NSLV", "2")), space="PSUM"))
        SHWO = os.environ.get("SHWO", "0") == "1"  # outproj shares psA pool
        psOp = None if SHWO else ctx.enter_context(
            tc.tile_pool(name="psOp", bufs=1, space="PSUM"))

        wq_sb = consts.tile([128, 8, 3 * C], SD)
        nc.sync.dma_start(out=wq_sb, in_=wqkvT_d.rearrange("(ci p) f -> p ci f", p=128))
        wo_sb = consts.tile([128, 8, C], SD)
        nc.sync.dma_start(out=wo_sb, in_=woutT_d.rearrange("(ci p) f -> p ci f", p=128))
        bcols_sb = consts.tile([128, 24], F32)
        nc.sync.dma_start(out=bcols_sb, in_=bcols_d)
        borep_sb = consts.tile([128, C], BF16)
        nc.sync.dma_start(out=borep_sb, in_=borep_d)
        maskT_sb = consts.tile([128, 128], BF16)
        nc.sync.dma_start(out=maskT_sb, in_=maskP_d)
        maskT4 = maskT_sb.unsqueeze(1).broadcast_to([128, 4, 128])
        idq = consts.tile([128, 128], F32)
        make_identity(nc, idq)
        idS = consts.tile([128, 128], SD)
        nc.vector.tensor_copy(idS, idq)

        K_bd = [stag.tile([128, NPAIR6, 16, 8], SD, name=f"Kbd{i}")
                for i in range(2)]
        Q_bd = [stag.tile([128, NPAIR6, 2, 16, 8], SD, name=f"Qbd{i}")
                for i in range(2)]
        V_AB = stag.tile([128, NG6, 16, 8], SD, name="V_AB")
        onesC = consts.tile([128, 64], BF16)
        nc.vector.memset(onesC, 1.0)
        for qb in Q_bd:
            for par in range(2):
                nc.vector.memset(qb[64 * (1 - par):64 * (1 - par) + 64, :, par],
                                 0.0)
        # persistent V8 quad tiles [128, 4(g), 128]: cols 64:128 ones
        v8q = [stag.tile([128, 4, 128], SD, name=f"v8q{i}") for i in range(2)]
        for e in v8q:
            for g in range(4):
                nc.vector.tensor_copy(e[:, g, 64:128], onesC)

        xT_r = xT_d.rearrange("(ci p) t -> p ci t", p=128)

        def emit_xload(piv):
            x_sb = xin.tile([128, 8, SB6], SD)
            nc.sync.dma_start(out=x_sb, in_=xT_r[:, :, bass.ds(piv, SB6)])
            return x_sb

        def v_half(hb):
            return V_AB[64 * hb:64 * hb + 64]

        ECNT = [0]

        def evict(dst, src, bias):
            if ECNT[0] % 2 == 0:
                nc.vector.tensor_scalar_add(dst, src, bias)
            else:
                nc.scalar.activation(dst, src, Ident, bias=bias)
            ECNT[0] += 1

        def emit_proj_chunk(x_sb, co, hb):
            psC1 = psA.tile([128, SB6], F32, name="pa")
            for ci in range(8):
                nc.tensor.matmul(psC1, wq_sb[:, ci, co * 128:(co + 1) * 128],
                                 x_sb[:, ci, :], start=(ci == 0), stop=(ci == 7))
            kind, c = co // 8, co % 8
            hslot = 2 * c
            for dlt in range(2):
                bias = bcols_sb[64 * dlt:64 * dlt + 64, co:co + 1]
                if kind == 2:
                    src = psC1[64 * dlt:64 * dlt + 64, :].rearrange(
                        "p (g t) -> p g t", g=NG6)
                    evict(v_half(hb)[:, :, hslot + dlt, :], src, bias)
                    continue
                srcg = psC1[64 * dlt:64 * dlt + 64, :].rearrange(
                    "p (pr par t) -> p pr par t", par=2, t=8)
                for par in range(2):
                    src = srcg[:, :, par, :]
                    if kind == 0:
                        dst = Q_bd[hb][64 * par:64 * par + 64, :, par,
                                       hslot + dlt, :]
                    else:
                        dst = K_bd[hb][64 * par:64 * par + 64, :,
                                       hslot + dlt, :]
                    evict(dst, src, bias)

        def emit_attn_q1(qi, hb):
            """quad qi (groups 4qi..4qi+3 = pairs 2qi, 2qi+1): scores,
            transposes, fused exp/mask, V8 copy."""
            vb = 64 * hb
            psS = psSp.tile([128, 2, 256], F32)
            psV = psVp.tile([128, 4, 64], SD)
            for p in range(2):
                pr = 2 * qi + p
                nc.tensor.matmul(psS[:, p], K_bd[hb][:, pr], Q_bd[hb][:, pr],
                                 start=True, stop=True)
            for g in range(4):
                nc.tensor.transpose(psV[:, g, :], v_half(hb)[:, 4 * qi + g, :, :],
                                    idS[vb:vb + 64, vb:vb + 64])
            es = smx.tile([128, 4, 128], SD, name="es")
            nc.scalar.activation(es, psS.rearrange("p a (b c) -> p (a b) c", b=2),
                                 Exp, scale=0.125)
            nc.gpsimd.tensor_mul(es, es, maskT4)
            V8sb = v8q[qi % 2]
            if qi % 2 == 0:
                nc.vector.tensor_copy(V8sb[:, :, 0:64], psV)
            else:
                nc.scalar.copy(V8sb[:, :, 0:64], psV)
            return es, V8sb

        def emit_attn_q2(qi, es, V8sb, S2):
            psF = psFp.tile([128, 4, 16, 8], F32)
            for g in range(4):
                nc.tensor.matmul(psF[:, g], V8sb[:, g, :], es[:, g, :],
                                 start=True, stop=True)
            rZB = smx.tile([64, 4, 16, 8], F32, name="rZB")
            nc.vector.reciprocal(rZB, psF[64:128])
            for dlt in range(2):
                src = psF[0:64].rearrange("p g h t -> p h g t")[
                    :, 8 * dlt:8 * dlt + 8, :, :]
                rzs = rZB.rearrange("p g h t -> p h g t")[
                    :, 8 * dlt:8 * dlt + 8, :, :]
                dst = S2[64 * dlt:64 * dlt + 64].rearrange(
                    "p c (gb gq t) -> p c gb gq t", gb=4, gq=4)[:, :, qi % 4]
                nc.vector.tensor_mul(dst, src, rzs)

        def emit_outproj(S2, oiv, iss):
            for nh in range(2):
                psO = (psA.tile([128, 512], F32, name="pa") if SHWO
                       else psOp.tile([128, 512], F32))
                for c in range(8):
                    nc.tensor.matmul(psO, S2[:, c, :],
                                     wo_sb[:, c, 512 * nh:512 * nh + 512],
                                     start=(c == 0), stop=(c == 7))
                outsb = outp.tile([128, 512], F32, name="outsb")
                nc.scalar.copy(outsb, psO)
                nc.gpsimd.tensor_add(outsb, outsb,
                                     borep_sb[:, 512 * nh:512 * nh + 512])
                nc.sync.dma_start(
                    out=out_d[bass.ds(oiv + SS6 * iss, SS6),
                              bass.ds(512 * nh, 512)], in_=outsb)

        def emit_part(attn_oiv, attn_hb, proj_piv, proj_hb):
            """16 quads of attention woven with 24 projection chunks."""
            x_sb = emit_xload(proj_piv) if proj_piv is not None else None
            S2 = None
            dpo = None
            for qi in range(16):
                if attn_oiv is not None:
                    if qi % 4 == 0:
                        S2 = s2p.tile([128, 8, SS6], SD, name="S2")
                    pend = emit_attn_q1(qi, attn_hb)
                    if dpo is not None:
                        emit_outproj(dpo[0], attn_oiv, dpo[1])
                        dpo = None
                if x_sb is not None:
                    for co in range((3 * qi) // 2, (3 * (qi + 1)) // 2):
                        emit_proj_chunk(x_sb, co, proj_hb)
                if attn_oiv is not None:
                    emit_attn_q2(qi, pend[0], pend[1], S2)
                    if qi % 4 == 3:
                        dpo = (S2, qi // 4)
            if dpo is not None:
                emit_outproj(dpo[0], attn_oiv, dpo[1])

        assert tok % (2 * SB6) == 0 and tok >= 2 * SB6
        emit_part(None, None, 0, 0)
        if tok > 2 * SB6 and static_loop:
            for iv in range(0, tok - 2 * SB6, 2 * SB6):
                emit_part(iv, 0, iv + SB6, 1)
                emit_part(iv + SB6, 1, iv + 2 * SB6, 0)
        elif tok > 2 * SB6 and reps == 1:
            with tc.For_i(0, tok - 2 * SB6, 2 * SB6,
                          hint_engines=(mybir.EngineType.PE, mybir.EngineType.DVE,
                                        mybir.EngineType.Activation)) as iv:
                emit_part(iv, 0, iv + SB6, 1)
                emit_part(iv + SB6, 1, iv + 2 * SB6, 0)
        elif tok > 2 * SB6:
            with tc.For_i(0, reps, 1) as _rep:
                with tc.For_i(0, tok - 2 * SB6, 2 * SB6,
                              hint_engines=(mybir.EngineType.PE,
                                            mybir.EngineType.DVE,
                                            mybir.EngineType.Activation)) as iv:
                    emit_part(iv, 0, iv + SB6, 1)
                    emit_part(iv + SB6, 1, iv + 2 * SB6, 0)
        last = tok - 2 * SB6
        emit_part(last, 0, tok - SB6, 1)
        emit_part(tok - SB6, 1, None, None)
    nc.compile()
    return nc


def _round_f32r(a):
    """Round fp32 to the f32r grid (drop 12 mantissa bits, round-to-nearest)."""
    b = np.ascontiguousarray(a, dtype=np.float32).view(np.uint32)
    b = ((b + (1 << 11)) >> 12) << 12
    return b.view(np.float32)


def _wcast(a, mode):
    if mode == "bf16":
        return np.ascontiguousarray(a.astype(ml_dtypes.bfloat16))
    if mode == "f32r":
        return _round_f32r(np.ascontiguousarray(a, dtype=np.float32))
    return np.ascontiguousarray(a, dtype=np.float32)


def _host_prep(x, w_qkv, b_qkv, w_out, b_out, mode=KMODE):
    d = np.arange(D)
    perm_q = (192 * np.arange(H)[:, None] + d[None, :]).reshape(-1)
    perm = np.concatenate([perm_q, perm_q + 64, perm_q + 128])
    wqkvT = np.ascontiguousarray(w_qkv[perm, :].T, dtype=np.float32)
    bcols = np.ascontiguousarray(
        b_qkv[perm].reshape(24, 128).T, dtype=np.float32)
    # out-proj row perm: S2 row 128c+64dlt+d holds feature 64*(8dlt+c)+d
    co, dl = np.arange(8), np.arange(2)
    perm_o = (64 * (8 * dl[None, :, None] + co[:, None, None])
              + d[None, None, :]).reshape(-1)
    woutT = np.ascontiguousarray(w_out.T[perm_o, :], dtype=np.float32)
    borep = np.ascontiguousarray(
        np.broadcast_to(b_out[None, :], (128, C)), dtype=np.float32)
    maskB = np.tile((np.arange(128)[:, None] % 8
                     == np.arange(128)[None, :] % 8).astype(np.float32), (1, 4))
    # in-group mask [128 rows=(hk,tk), (hq, tq)]: keep tk==tq
    maskP = np.ascontiguousarray(
        (np.arange(128)[:, None] % 8 == np.arange(128)[None, :] % 8
         ).astype(ml_dtypes.bfloat16))
    borep16 = np.ascontiguousarray(borep.astype(ml_dtypes.bfloat16))
    maskP2 = np.ascontiguousarray(np.tile(maskP, (1, 2)))
    xT = np.ascontiguousarray(x.T, dtype=np.float32)
    if XBF:
        xT16 = np.ascontiguousarray(xT.astype(ml_dtypes.bfloat16))
    else:
        xT16 = _wcast(xT, mode)
    xT = _wcast(xT, mode)
    wqkvT = _wcast(wqkvT, mode)
    woutT = _wcast(woutT, mode)
    if mode == "bf16":
        maskB = np.ascontiguousarray(maskB.astype(ml_dtypes.bfloat16))
    return dict(xT=xT, xT16=xT16, wqkvT=wqkvT, bcols=bcols, woutT=woutT,
                borep=borep, borep16=borep16, maskB=maskB, maskP=maskP,
                maskP2=maskP2)


_cache = {}


def kernel(x, w_qkv, b_qkv, w_out, b_out, _trace=False, _tmpdir=None):
    x = np.asarray(x)
    n = x.shape[0]
    tok = n // N_CORES
    pipe = os.environ.get("PIPE", "6")
    hp = _host_prep(
        np.asarray(x), np.asarray(w_qkv), np.asarray(b_qkv),
        np.asarray(w_out), np.asarray(b_out),
        mode="bf16" if pipe in ("5", "6") else KMODE)
    key = (tok, KMODE, pipe)
    if key not in _cache:
        _cache[key] = {"6": build_pipe6, "5": build_pipe5,
                       "4": build_pipe4,
                       "1": build_pipe}.get(pipe, build)(tok)
    nc = _cache[key]
    if pipe in ("5", "6"):
        shared = dict(wqkvT=hp["wqkvT"], woutT=hp["woutT"], bcols=hp["bcols"],
                      borep=hp["borep16"], maskP=hp["maskP"])
        xT = hp["xT"]
    elif pipe == "4":
        xT = hp["xT16"]
        shared = dict(wqkvT=hp["wqkvT"], woutT=hp["woutT"], bcols=hp["bcols"],
                      borep=hp["borep16"], maskP=hp["maskP"])
    elif pipe == "1":
        xT = hp["xT16"]
        shared = dict(wqkvT=hp["wqkvT"], woutT=hp["woutT"], bcols=hp["bcols"],
                      borep=hp["borep16"], maskP=hp["maskP"])
    else:
        xT = hp["xT"]
        shared = dict(wqkvT=hp["wqkvT"], woutT=hp["woutT"], bcols=hp["bcols"],
                      borep=hp["borep"], maskB=hp["maskB"])
    in_maps = [dict(xT=np.ascontiguousarray(xT[:, i * tok:(i + 1) * tok]), **shared)
               for i in range(N_CORES)]
    res = run_bass_kernel_spmd(nc, in_maps, core_ids=list(range(N_CORES)),
                               trace=_trace, tmpdir=_tmpdir)
    out = np.concatenate([res.results[i]["out"] for i in range(N_CORES)], axis=0)
    kernel.last_results = res
    mod = sys.modules[__name__]
    mod.last_nc = nc
    mod.last_in_maps = in_maps
    mod.build_current = {"6": build_pipe6, "5": build_pipe5,
                         "4": build_pipe4,
                         "1": build_pipe}.get(pipe, build)
    mod.last_step = 1024 if pipe == "6" else 512
    return out



# revision 32
# speedup vs baseline: 2.1000x; 1.9164x over previous
"""Trainium2 Bass kernel for nn_MultiHeadAttention_72189810312078.

Computation (per token): qkv = x @ w_qkv.T + b_qkv; per-token attention over
the 16 heads with 16x16 score matrices; out = attn_out @ w_out.T + b_out.

Data-parallel over 8 NeuronCores (8192 tokens each). Host pre-transposes x
to xT [1024, N] so the channel (contraction) dim lands on SBUF partitions.

Active build: build_pipe6 (PIPE=6, all-bf16). HW-measured shape facts that
drive it (micro.py, this backend):
  - bf16 matmuls sustain ~0.38-0.39 ns/output-column at K=128 regardless of
    moving width (LDWEIGHTS does NOT serialize); f32r is ~10-20% slower
    (wider SBUF streams); K=64 matmuls run ~2x slower per column - so every
    matmul is shaped K=128.
  - fp8e4 DoubleRow gives 2.2x MAC rate but fails the 2e-2 gate (~4.4e-2);
    bf16 end-to-end lands at 4.3e-3.
Structure (per 512-token superblock, software-pipelined with the NEXT
superblock's projection via ping-pong staging):
  1) qkv projection: 24 feature chunks x 8 K-chunks, moving=512; bias fused
     into PSUM->staging evictions (DVE/ScalarE alternating). K staging is
     BLOCK-DIAG: even groups' d-dim on partitions 0:64, odd on 64:128; Q
     staging zero-padded block-diag (off-par halves zeroed once), so Q/K
     evictions split into even/odd partition-shifted writes.
  2) scores: ONE K=128 matmul per group-PAIR (block-diag stationary
     K_bd[:, pr], moving Q_bd[:, pr] 256 wide) -> both groups' [128,128]
     score blocks, no garbage columns.
  3) attention in QUADS (4 groups): ONE ScalarE exp [128,512-els], ONE
     GpSimd cross-token mask-mul, ONE V^T->SBUF copy, ONE DVE reciprocal
     per quad; 4 PE transposes build V8 [(hk,t) x (64 V^T | 64 ones)]; 4
     per-group attnV matmuls (K=128, N=128) whose ones-columns produce the
     softmax denominator rows in psF[64:128] for free; S2 pack fuses the
     normalization (tensor_mul by reciprocal) on DVE.
  4) out-projection per 128-token stage: 16 matmuls moving=512 against
     host-permuted w_out; dedicated PSUM bank (sharing the projection pool
     serializes the pipeline and costs 2x!).
PSUM banks (dep tracking is bank-granular): psA x2 (proj), psS x2 (scores),
psF x1 (attnV), psV x2 (transposes), psO x1 (out-proj) = 8.

Max rel err vs fp32 reference ~4.3e-3 (bf16 rounding).
Legacy builds kept for A/B: build (v1), build_pipe (v3 all-f32r),
build_pipe4 (v4), build_pipe5 (v5 bf16 SB=256).
"""

import os
import sys
from contextlib import ExitStack, nullcontext

sys.path.insert(0, "/opt/trn_rl_repo")

import numpy as np
import ml_dtypes

import concourse.bass as bass  # noqa: E402
import concourse.bacc as bacc  # noqa: E402
import concourse.tile as tile  # noqa: E402
from concourse import mybir  # noqa: E402
from concourse.bass_utils import run_bass_kernel_spmd  # noqa: E402
from concourse.masks import make_identity  # noqa: E402

F32 = mybir.dt.float32
F32R = mybir.dt.float32r
BF16 = mybir.dt.bfloat16

N_CORES = 8
H, D, C = 16, 64, 1024
SB = 256   # tokens per superblock (projection moving dim)
SS = 128   # tokens per attention sub-stage / out-projection block
NG = SB // 8   # token groups per superblock (32)

KMODE = os.environ.get("KMODE", "f32r")
GPS = int(os.environ.get("GPS", "3"))  # bitmask: 1=memset, 2=mask-mul, 4=bias
Exp = mybir.ActivationFunctionType.Exp
Copy = mybir.ActivationFunctionType.Copy
Ident = mybir.ActivationFunctionType.Identity


def _dtypes(mode):
    """-> (WD projection-weight dtype, SD attention staging, AD S2)."""
    if mode == "bf16":
        return BF16, BF16, BF16
    if mode == "f32r":
        return F32R, F32R, F32R
    return F32, F32, F32


XBF = os.environ.get("XBF", "0") == "1"   # x streamed in bf16 (moving operand)


def build(tok, mode=KMODE, static_loop=False):
    WD, SD, AD = _dtypes(mode)

    nc = bacc.Bacc("TRN2", target_bir_lowering=False, debug=False,
                   enable_asserts=True, num_devices=N_CORES)
    xT_d = nc.dram_tensor("xT", [C, tok], WD, kind="ExternalInput").ap()
    wqkvT_d = nc.dram_tensor("wqkvT", [C, 3 * C], WD, kind="ExternalInput").ap()
    woutT_d = nc.dram_tensor("woutT", [C, C], WD, kind="ExternalInput").ap()
    bcols_d = nc.dram_tensor("bcols", [128, 24], F32, kind="ExternalInput").ap()
    borep_d = nc.dram_tensor("borep", [128, C], F32, kind="ExternalInput").ap()
    maskB_d = nc.dram_tensor("maskB", [128, 512], SD, kind="ExternalInput").ap()
    out_d = nc.dram_tensor("out", [tok, C], F32, kind="ExternalOutput").ap()

    with tile.TileContext(nc) as tc, ExitStack() as ctx:
        consts = ctx.enter_context(tc.tile_pool(name="consts", bufs=1))
        xin = ctx.enter_context(tc.tile_pool(name="xin", bufs=2))
        stag = ctx.enter_context(tc.tile_pool(name="stag", bufs=1))
        smx = ctx.enter_context(tc.tile_pool(name="smx", bufs=2))
        s2p = ctx.enter_context(tc.tile_pool(name="s2p", bufs=2))
        outp = ctx.enter_context(tc.tile_pool(name="outp", bufs=2))
        psA = ctx.enter_context(tc.tile_pool(name="psA", bufs=2, space="PSUM"))
        psSp = ctx.enter_context(tc.tile_pool(name="psSp", bufs=2, space="PSUM"))
        psVp = ctx.enter_context(tc.tile_pool(name="psVp", bufs=1, space="PSUM"))
        psC2p = ctx.enter_context(tc.tile_pool(name="psC2p", bufs=1, space="PSUM"))
        psTp = ctx.enter_context(tc.tile_pool(name="psTp", bufs=1, space="PSUM"))
        psOp = ctx.enter_context(tc.tile_pool(name="psOp", bufs=1, space="PSUM"))

        # ---- constants ----
        wq_sb = consts.tile([128, 8, 3 * C], WD)
        nc.sync.dma_start(out=wq_sb, in_=wqkvT_d.rearrange("(ci p) f -> p ci f", p=128))
        wo_sb = consts.tile([128, 8, C], WD)
        nc.sync.dma_start(out=wo_sb, in_=woutT_d.rearrange("(ci p) f -> p ci f", p=128))
        bcols_sb = consts.tile([128, 24], F32)
        nc.sync.dma_start(out=bcols_sb, in_=bcols_d)
        borep_sb = consts.tile([128, C], F32)
        nc.sync.dma_start(out=borep_sb, in_=borep_d)
        maskB_sb = consts.tile([128, 512], SD)
        nc.sync.dma_start(out=maskB_sb, in_=maskB_d)
        idq = consts.tile([128, 128], F32)
        make_identity(nc, idq)
        if SD is F32:
            idS = idq
        else:
            idS = consts.tile([128, 128], SD)
            nc.vector.tensor_copy(idS, idq)
        if AD is F32:
            idr = idq
        elif AD is SD:
            idr = idS
        else:
            idr = consts.tile([128, 128], AD)
            nc.vector.tensor_copy(idr, idq)

        ecnt = 0  # evict-engine round robin

        def evict_copy(dst, src):
            nonlocal ecnt
            if ecnt % 2 == 0:
                nc.vector.tensor_copy(dst, src)
            else:
                nc.scalar.copy(dst, src)
            ecnt += 1

        xT_r = xT_d.rearrange("(ci p) t -> p ci t", p=128)
        if static_loop:
            loop_iter = [(nullcontext(iv), iv) for iv in range(0, tok, SB)]
        else:
            fc = tc.For_i(0, tok, SB,
                          hint_engines=(mybir.EngineType.PE,
                                        mybir.EngineType.DVE))
            loop_iter = [(fc, None)]
        for _ctx, _iv in loop_iter:
          with _ctx as _cv:
            iv = _iv if _iv is not None else _cv
            x_sb = xin.tile([128, 8, SB], WD)
            nc.sync.dma_start(out=x_sb, in_=xT_r[:, :, bass.ds(iv, SB)])

            # staging: T1 rows 64:128 = Q; T2 rows 64:128 = K, rows 0:64 = V
            T1 = stag.tile([128, NG, 16, 8], SD, name="T1")
            T2 = stag.tile([128, NG, 16, 8], SD, name="T2")

            # ---- qkv projection + scatter-evict (bias fused / on gpsimd) ----
            for co in range(24):
                psC1 = psA.tile([128, SB], F32)
                for ci in range(8):
                    nc.tensor.matmul(psC1, wq_sb[:, ci, co * 128:(co + 1) * 128],
                                     x_sb[:, ci, :], start=(ci == 0),
                                     stop=(ci == 7))
                kind, c = co // 8, co % 8
                for dlt in range(2):
                    src = psC1[64 * dlt:64 * dlt + 64, :].rearrange(
                        "p (g t) -> p g t", g=NG)
                    hslot = 2 * c + dlt
                    if kind == 0:
                        dst = T1[64:128, :, hslot, :]
                    elif kind == 1:
                        dst = T2[64:128, :, hslot, :]
                    else:
                        dst = T2[0:64, :, hslot, :]
                    bias = bcols_sb[64 * dlt:64 * dlt + 64, co:co + 1]
                    if dlt == 0:
                        # DVE evict with fused bias add
                        nc.vector.tensor_scalar_add(dst, src, bias)
                    elif GPS & 4:
                        # ACT plain evict, bias added SBUF-side on idle gpsimd
                        nc.scalar.copy(dst, src)
                        nc.gpsimd.tensor_scalar_add(dst, dst, bias)
                    else:
                        nc.vector.tensor_scalar_add(dst, src, bias)

            # ---- attention (8 batches of 4 groups) + out-proj per 128 tok ----
            for iss in range(2):
                S2 = s2p.tile([128, 8, SS], AD)
                for b4 in range(4 * iss, 4 * iss + 4):
                    psS = psSp.tile([128, 512], F32)
                    psV = psVp.tile([128, 4, 64], SD)
                    for j in range(4):
                        g = 4 * b4 + j
                        nc.tensor.matmul(psS[:, 128 * j:128 * j + 128],
                                         T2[64:128, g, :, :], T1[64:128, g, :, :],
                                         start=True, stop=True)
                        nc.tensor.transpose(psV[:, j, :], T2[0:64, g, :, :],
                                            idS[0:64, 0:64])
                    es4 = smx.tile([128, 512], SD)
                    nc.scalar.activation(es4, psS, Exp, scale=0.125)
                    if GPS & 2:
                        nc.gpsimd.tensor_mul(es4, es4, maskB_sb)
                    else:
                        nc.vector.tensor_mul(es4, es4, maskB_sb)
                    V8sb = smx.tile([128, 4, 66], SD)
                    nc.scalar.copy(V8sb[:, :, 0:64], psV)
                    if GPS & 1:
                        nc.gpsimd.memset(V8sb[:, :, 64:65], 1.0)
                    else:
                        nc.vector.memset(V8sb[:, :, 64:65], 1.0)
                    psC2 = psC2p.tile([128, 4, 66], F32)
                    for j in range(4):
                        nc.tensor.matmul(psC2[:, j, 0:65],
                                         es4[:, 128 * j:128 * j + 128],
                                         V8sb[:, j, 0:65], start=True, stop=True)
                    rec4 = smx.tile([128, 4], F32)
                    nc.vector.reciprocal(rec4, psC2[:, :, 64:65])
                    attno = smx.tile([128, 4, 64], AD)
                    for j in range(4):
                        if j % 2 == 0:
                            nc.scalar.activation(attno[:, j, :], psC2[:, j, 0:64],
                                                 Copy, scale=rec4[:, j:j + 1])
                        else:
                            nc.vector.tensor_scalar_mul(attno[:, j, :],
                                                        psC2[:, j, 0:64],
                                                        rec4[:, j:j + 1])
                    psT = psTp.tile([64, 4, 128], AD)
                    for j in range(4):
                        nc.tensor.transpose(psT[:, j, :], attno[:, j, :], idr)
                    # S2 pack: head slots 8*dlt..8*dlt+7 -> S2 rows 64*dlt+d
                    for dlt in range(2):
                        src = psT[:, :, 64 * dlt:64 * dlt + 64].rearrange(
                            "p j (h t) -> p j h t", h=8)
                        dst = S2[64 * dlt:64 * dlt + 64].rearrange(
                            "p c (gb gj t) -> p gj c gb t", gb=4, gj=4)[
                                :, :, :, b4 % 4, :]
                        evict_copy(dst, src)

                # ---- out-projection for this 128-token block ----
                outsb = outp.tile([128, C], F32)
                for nh in range(2):
                    psO = psOp.tile([128, 512], F32)
                    for c in range(8):
                        nc.tensor.matmul(psO, S2[:, c, :],
                                         wo_sb[:, c, 512 * nh:512 * nh + 512],
                                         start=(c == 0), stop=(c == 7))
                    nc.vector.tensor_add(outsb[:, 512 * nh:512 * nh + 512], psO,
                                         borep_sb[:, 512 * nh:512 * nh + 512])
                nc.sync.dma_start(out=out_d[bass.ds(iv + SS * iss, SS), :],
                                  in_=outsb)

    nc.compile()
    return nc


def build_pipe(tok, mode=KMODE, static_loop=False, reps=1):
    """Software-pipelined build v3: all-f32r matmuls (self-loading weights, no
    standalone LDWEIGHTS), attention via group-PAIR matmuls so every PE op has
    a 256-wide moving operand (f32r fast path):
      - scores: per pair (gA,gB), two matmuls K_g^T @ [Q_gA|Q_gB] (256 moving)
      - exp on ScalarE -> es (bf16); pair mask (kills cross-group and
        cross-token terms) on GpSimd
      - attnV flipped: psF[d(+Z row), QpairCols] = sum_X V8_X^T @ es_X with the
        V8 ones-column producing the softmax denominator row; S2 packed
        directly from psF (no output transpose)
      - denominators: Z row gathered to zbuf, PE-transposed to per-token
        column, reciprocal on DVE, applied as per-partition scale during the
        out-projection eviction on ScalarE.
    Projection of superblock k+1 overlaps attention of superblock k."""
    WD, SD, AD = _dtypes(mode)
    XD = BF16 if XBF else WD

    nc = bacc.Bacc("TRN2", target_bir_lowering=False, debug=False,
                   enable_asserts=True, num_devices=N_CORES)
    xT_d = nc.dram_tensor("xT", [C, tok], XD, kind="ExternalInput").ap()
    wqkvT_d = nc.dram_tensor("wqkvT", [C, 3 * C], WD, kind="ExternalInput").ap()
    woutT_d = nc.dram_tensor("woutT", [C, C], WD, kind="ExternalInput").ap()
    bcols_d = nc.dram_tensor("bcols", [128, 24], F32, kind="ExternalInput").ap()
    borep_d = nc.dram_tensor("borep", [128, C], BF16, kind="ExternalInput").ap()
    maskP_d = nc.dram_tensor("maskP", [128, 128], BF16, kind="ExternalInput").ap()
    out_d = nc.dram_tensor("out", [tok, C], F32, kind="ExternalOutput").ap()

    PB = [int(v) for v in os.environ.get("PB", "1,2,2,2,2,2").split(",")]
    PRJ = int(os.environ.get("PRJ", "3"))
    with tile.TileContext(nc) as tc, ExitStack() as ctx:
        consts = ctx.enter_context(tc.tile_pool(name="consts", bufs=1))
        xin = ctx.enter_context(tc.tile_pool(name="xin", bufs=PB[0]))
        stag = ctx.enter_context(tc.tile_pool(name="stag", bufs=1))
        smx = ctx.enter_context(tc.tile_pool(name="smx", bufs=PB[1]))
        s2p = ctx.enter_context(tc.tile_pool(name="s2p", bufs=PB[2]))
        outp = ctx.enter_context(tc.tile_pool(name="outp", bufs=PB[3]))
        psA = ctx.enter_context(tc.tile_pool(name="psA", bufs=PB[4], space="PSUM"))
        psSp = ctx.enter_context(tc.tile_pool(name="psSp", bufs=PB[5], space="PSUM"))
        psVp = ctx.enter_context(tc.tile_pool(name="psVp", bufs=1, space="PSUM"))
        psFp = ctx.enter_context(tc.tile_pool(name="psFp", bufs=2, space="PSUM"))
        psOp = ctx.enter_context(tc.tile_pool(name="psOp", bufs=1, space="PSUM"))

        wq_sb = consts.tile([128, 8, 3 * C], WD)
        nc.sync.dma_start(out=wq_sb, in_=wqkvT_d.rearrange("(ci p) f -> p ci f", p=128))
        wo_sb = consts.tile([128, 8, C], WD)
        nc.sync.dma_start(out=wo_sb, in_=woutT_d.rearrange("(ci p) f -> p ci f", p=128))
        bcols_sb = consts.tile([128, 24], F32)
        nc.sync.dma_start(out=bcols_sb, in_=bcols_d)
        borep_sb = consts.tile([128, C], BF16)
        nc.sync.dma_start(out=borep_sb, in_=borep_d)
        maskT_sb = consts.tile([128, 128], BF16)
        nc.sync.dma_start(out=maskT_sb, in_=maskP_d)
        idq = consts.tile([128, 128], F32)
        make_identity(nc, idq)
        if SD is F32:
            idS = idq
        else:
            idS = consts.tile([128, 128], SD)
            nc.vector.tensor_copy(idS, idq)

        # persistent ping-pong staging (half-set hb=0: Q/K upper, V lower)
        Q_AB = stag.tile([128, NG, 16, 8], SD, name="Q_AB")
        K_AB = stag.tile([128, NG, 16, 8], SD, name="K_AB")
        V_AB = stag.tile([128, NG, 16, 8], SD, name="V_AB")
        # persistent es pair tiles [128, X, gp, 16, 8]; the gp != X (cross
        # group) halves are zeroed once here and never written again, so the
        # attnV pair matmuls read zeros there without any recurring masking.
        es_pp = [stag.tile([128, 2, 2, 16, 8], SD, name=f"es{i}")
                 for i in range(2)]
        zeroC = consts.tile([128, 128], BF16)
        nc.vector.memset(zeroC, 0.0)
        onesC = consts.tile([128, 64], BF16)
        nc.vector.memset(onesC, 1.0)
        for e in es_pp:
            nc.vector.tensor_copy(e[:, 0, 1].rearrange("p h t -> p (h t)"), zeroC)
            nc.vector.tensor_copy(e[:, 1, 0].rearrange("p h t -> p (h t)"), zeroC)
        # persistent V8 pair tiles [128, X, 128]: cols 0:64 = V^T (rewritten
        # each pair), cols 64:128 = constant ones so the attnV matmul output
        # rows 64:128 replicate the softmax-denominator row across partitions
        v8_pp = [stag.tile([128, 2, 128], SD, name=f"v8_{i}") for i in range(2)]
        for e in v8_pp:
            nc.vector.tensor_copy(e[:, 0, 64:128], onesC)
            nc.vector.tensor_copy(e[:, 1, 64:128], onesC)

        xT_r = xT_d.rearrange("(ci p) t -> p ci t", p=128)

        def emit_xload(piv):
            x_sb = xin.tile([128, 8, SB], XD)
            nc.sync.dma_start(out=x_sb, in_=xT_r[:, :, bass.ds(piv, SB)])
            return x_sb

        def qk_half(T, hb):
            return T[64 * (1 - hb):64 * (1 - hb) + 64]

        def v_half(hb):
            return V_AB[64 * hb:64 * hb + 64]

        def emit_proj_chunk(x_sb, co, hb):
            psC1 = psA.tile([128, SB], F32)
            for ci in range(8):
                nc.tensor.matmul(psC1, wq_sb[:, ci, co * 128:(co + 1) * 128],
                                 x_sb[:, ci, :], start=(ci == 0), stop=(ci == 7))
            kind, c = co // 8, co % 8
            for dlt in range(2):
                src = psC1[64 * dlt:64 * dlt + 64, :].rearrange(
                    "p (g t) -> p g t", g=NG)
                hslot = 2 * c + dlt
                if kind == 0:
                    dst = qk_half(Q_AB, hb)[:, :, hslot, :]
                elif kind == 1:
                    dst = qk_half(K_AB, hb)[:, :, hslot, :]
                else:
                    dst = v_half(hb)[:, :, hslot, :]
                bias = bcols_sb[64 * dlt:64 * dlt + 64, co:co + 1]
                if (co + dlt) % 2 == 0:
                    nc.vector.tensor_scalar_add(dst, src, bias)
                else:
                    nc.scalar.activation(dst, src, Ident, bias=bias)

        def emit_attn_batch1(b4, hb):
            """scores (pair matmuls) + V transposes + exp + pair-mask for the
            two pairs of batch b4 (groups 4*b4..4*b4+3)."""
            vb = 64 * hb
            pend = []
            for p in range(2):
                gA = 4 * b4 + 2 * p
                psS = psSp.tile([128, 2, 256], F32)
                psV = psVp.tile([128, 2, 64], SD)
                qpair = qk_half(Q_AB, hb)[:, gA:gA + 2, :, :]
                for X in range(2):
                    nc.tensor.matmul(psS[:, X, :],
                                     qk_half(K_AB, hb)[:, gA + X, :, :],
                                     qpair, start=True, stop=True)
                for X in range(2):
                    nc.tensor.transpose(psV[:, X, :], v_half(hb)[:, gA + X, :, :],
                                        idS[vb:vb + 64, vb:vb + 64])
                es = es_pp[p]
                for X in range(2):
                    nc.scalar.activation(es[:, X, X],
                                         psS[:, X, 128 * X:128 * X + 128],
                                         Exp, scale=0.125)
                    nc.gpsimd.tensor_mul(es[:, X, X], es[:, X, X], maskT_sb)
                V8sb = v8_pp[p]
                if p == 0:
                    nc.vector.tensor_copy(V8sb[:, :, 0:64], psV)
                else:
                    nc.scalar.copy(V8sb[:, :, 0:64], psV)
                pend.append((es, V8sb))
            return pend

        def emit_attn_batch2(b4, pend, S2, pairs=(0, 1)):
            for p in pairs:
                es, V8sb = pend[p]
                psF = psFp.tile([128, 2, 16, 8], F32)
                esf = es.rearrange("q x gp h t -> q x (gp h t)")
                for X in range(2):
                    nc.tensor.matmul(psF, V8sb[:, X, :], esf[:, X, :],
                                     start=(X == 0), stop=(X == 1))
                # psF rows 64:128 hold the per-(head, token) softmax
                # denominator row replicated by the ones columns of V8
                rZB = smx.tile([64, 2, 16, 8], F32, name="rZB")
                nc.vector.reciprocal(rZB, psF[64:128])
                for dlt in range(2):
                    src = psF[0:64].rearrange("p gp h t -> p h gp t")[
                        :, 8 * dlt:8 * dlt + 8, :, :]
                    rzs = rZB.rearrange("p gp h t -> p h gp t")[
                        :, 8 * dlt:8 * dlt + 8, :, :]
                    dst = S2[64 * dlt:64 * dlt + 64].rearrange(
                        "p c (gb pp gp t) -> p c gb pp gp t",
                        gb=4, pp=2, gp=2)[:, :, b4 % 4, p, :, :]
                    nc.vector.tensor_mul(dst, src, rzs)

        def emit_outproj(S2, oiv, iss):
            for nh in range(2):
                psO = psOp.tile([128, 512], F32)
                for c in range(8):
                    nc.tensor.matmul(psO, S2[:, c, :],
                                     wo_sb[:, c, 512 * nh:512 * nh + 512],
                                     start=(c == 0), stop=(c == 7))
                outsb = outp.tile([128, 512], F32, name="outsb")
                nc.scalar.copy(outsb, psO)
                nc.gpsimd.tensor_add(outsb, outsb,
                                     borep_sb[:, 512 * nh:512 * nh + 512])
                nc.sync.dma_start(
                    out=out_d[bass.ds(oiv + SS * iss, SS),
                              bass.ds(512 * nh, 512)], in_=outsb)

        def emit_part(attn_oiv, attn_hb, proj_piv, proj_hb):
            """Weave attention of one superblock with projection of another.
            Either may be None (prologue/epilogue)."""
            x_sb = emit_xload(proj_piv) if proj_piv is not None else None
            S2 = None
            dpo = None   # deferred out-projection (S2, iss)
            for b4 in range(8):
                if attn_oiv is not None:
                    if b4 % 4 == 0:
                        S2 = s2p.tile([128, 8, SS], AD, name="S2")
                    pend = emit_attn_batch1(b4, attn_hb)
                    if dpo is not None:
                        emit_outproj(dpo[0], attn_oiv, dpo[1])
                        dpo = None
                if x_sb is not None:
                    for co in range(PRJ * b4, min(PRJ * b4 + PRJ, 24)):
                        emit_proj_chunk(x_sb, co, proj_hb)
                if attn_oiv is not None:
                    emit_attn_batch2(b4, pend, S2)
                    if b4 % 4 == 3:
                        dpo = (S2, b4 // 4)
            if dpo is not None:
                emit_outproj(dpo[0], attn_oiv, dpo[1])

        assert tok % (2 * SB) == 0 and tok >= 2 * SB
        emit_part(None, None, 0, 0)                      # prologue: proj sb0 -> A
        if tok > 2 * SB and static_loop:
            for iv in range(0, tok - 2 * SB, 2 * SB):
                emit_part(iv, 0, iv + SB, 1)             # attn A, proj -> B
                emit_part(iv + SB, 1, iv + 2 * SB, 0)    # attn B, proj -> A
        elif tok > 2 * SB and reps == 1:
            with tc.For_i(0, tok - 2 * SB, 2 * SB,
                          hint_engines=(mybir.EngineType.PE, mybir.EngineType.DVE,
                                        mybir.EngineType.Activation)) as iv:
                emit_part(iv, 0, iv + SB, 1)             # attn A, proj -> B
                emit_part(iv + SB, 1, iv + 2 * SB, 0)    # attn B, proj -> A
        elif tok > 2 * SB:
            with tc.For_i(0, reps, 1) as _rep:
                with tc.For_i(0, tok - 2 * SB, 2 * SB,
                              hint_engines=(mybir.EngineType.PE,
                                            mybir.EngineType.DVE,
                                            mybir.EngineType.Activation)) as iv:
                    emit_part(iv, 0, iv + SB, 1)         # attn A, proj -> B
                    emit_part(iv + SB, 1, iv + 2 * SB, 0)  # attn B, proj -> A
        last = tok - 2 * SB
        emit_part(last, 0, tok - SB, 1)                  # attn A, proj last -> B
        emit_part(tok - SB, 1, None, None)               # attn B
    nc.compile()
    return nc


def build_pipe4(tok, mode=KMODE, static_loop=False, reps=1):
    """v4: like build_pipe (v3) but
      - PSUM bank remap (dep tracking is bank-granular, 8 banks):
        psA 3 bufs (proj, deeper pipelining; v3 had 2), psSp 2, pvf 2
        (V^T transpose target + attnV psF SHARE one 1.5KB tile per pair --
        their accesses are a sequential chain within the pair so the shared
        bank adds no serialization), psOp 1.
      - exp and mask fused across the pair with a strided diag AP
        ([128, 2(stride 384), 128]): ONE ScalarE exp + ONE GpSimd mask-mul
        per pair instead of 2+2.
      - out-proj eviction fused: GpSimd tensor_add(outsb, psO, borep) reads
        PSUM directly (drops the separate ScalarE copy).
    """
    WD, SD, AD = _dtypes(mode)
    XD = BF16 if XBF else WD

    nc = bacc.Bacc("TRN2", target_bir_lowering=False, debug=False,
                   enable_asserts=True, num_devices=N_CORES)
    xT_d = nc.dram_tensor("xT", [C, tok], XD, kind="ExternalInput").ap()
    wqkvT_d = nc.dram_tensor("wqkvT", [C, 3 * C], WD, kind="ExternalInput").ap()
    woutT_d = nc.dram_tensor("woutT", [C, C], WD, kind="ExternalInput").ap()
    bcols_d = nc.dram_tensor("bcols", [128, 24], F32, kind="ExternalInput").ap()
    borep_d = nc.dram_tensor("borep", [128, C], BF16, kind="ExternalInput").ap()
    maskP_d = nc.dram_tensor("maskP", [128, 128], BF16, kind="ExternalInput").ap()
    out_d = nc.dram_tensor("out", [tok, C], F32, kind="ExternalOutput").ap()

    PRJ = int(os.environ.get("PRJ", "3"))
    NSLA = int(os.environ.get("NSLA", "3"))   # proj PSUM bufs
    XBUF = int(os.environ.get("XBUF", "2" if XBF else "1"))
    with tile.TileContext(nc) as tc, ExitStack() as ctx:
        consts = ctx.enter_context(tc.tile_pool(name="consts", bufs=1))
        xin = ctx.enter_context(tc.tile_pool(name="xin", bufs=XBUF))
        stag = ctx.enter_context(tc.tile_pool(name="stag", bufs=1))
        smx = ctx.enter_context(tc.tile_pool(name="smx", bufs=2))
        s2p = ctx.enter_context(tc.tile_pool(name="s2p", bufs=2))
        outp = ctx.enter_context(tc.tile_pool(name="outp", bufs=2))
        psA = ctx.enter_context(tc.tile_pool(name="psA", bufs=NSLA, space="PSUM"))
        psSp = ctx.enter_context(tc.tile_pool(name="psSp", bufs=2, space="PSUM"))
        pvf = ctx.enter_context(tc.tile_pool(name="pvf", bufs=2, space="PSUM"))
        psOp = ctx.enter_context(tc.tile_pool(name="psOp", bufs=1, space="PSUM"))

        wq_sb = consts.tile([128, 8, 3 * C], WD)
        nc.sync.dma_start(out=wq_sb, in_=wqkvT_d.rearrange("(ci p) f -> p ci f", p=128))
        wo_sb = consts.tile([128, 8, C], WD)
        nc.sync.dma_start(out=wo_sb, in_=woutT_d.rearrange("(ci p) f -> p ci f", p=128))
        bcols_sb = consts.tile([128, 24], F32)
        nc.sync.dma_start(out=bcols_sb, in_=bcols_d)
        borep_sb = consts.tile([128, C], BF16)
        nc.sync.dma_start(out=borep_sb, in_=borep_d)
        maskT_sb = consts.tile([128, 128], BF16)
        nc.sync.dma_start(out=maskT_sb, in_=maskP_d)
        maskT2 = maskT_sb.unsqueeze(1).broadcast_to([128, 2, 128])
        idq = consts.tile([128, 128], F32)
        make_identity(nc, idq)
        if SD is F32:
            idS = idq
        else:
            idS = consts.tile([128, 128], SD)
            nc.vector.tensor_copy(idS, idq)

        # persistent ping-pong staging (half-set hb=0: Q/K upper, V lower)
        Q_AB = stag.tile([128, NG, 16, 8], SD, name="Q_AB")
        K_AB = stag.tile([128, NG, 16, 8], SD, name="K_AB")
        V_AB = stag.tile([128, NG, 16, 8], SD, name="V_AB")
        # persistent es pair tiles [128, X, gp, 16, 8]; cross (gp != X) halves
        # zeroed once, never rewritten -> attnV pair matmuls read zeros there.
        es_pp = [stag.tile([128, 2, 2, 16, 8], SD, name=f"es{i}")
                 for i in range(2)]
        zeroC = consts.tile([128, 128], BF16)
        nc.vector.memset(zeroC, 0.0)
        onesC = consts.tile([128, 64], BF16)
        nc.vector.memset(onesC, 1.0)
        for e in es_pp:
            nc.vector.tensor_copy(e[:, 0, 1].rearrange("p h t -> p (h t)"), zeroC)
            nc.vector.tensor_copy(e[:, 1, 0].rearrange("p h t -> p (h t)"), zeroC)
        # persistent V8 pair tiles [128, X, 128]: cols 0:64 = V^T (rewritten
        # each pair), cols 64:128 = ones -> attnV rows 64:128 = softmax denom
        v8_pp = [stag.tile([128, 2, 128], SD, name=f"v8_{i}") for i in range(2)]
        for e in v8_pp:
            nc.vector.tensor_copy(e[:, 0, 64:128], onesC)
            nc.vector.tensor_copy(e[:, 1, 64:128], onesC)

        xT_r = xT_d.rearrange("(ci p) t -> p ci t", p=128)

        def emit_xload(piv):
            x_sb = xin.tile([128, 8, SB], XD)
            nc.sync.dma_start(out=x_sb, in_=xT_r[:, :, bass.ds(piv, SB)])
            return x_sb

        def qk_half(T, hb):
            return T[64 * (1 - hb):64 * (1 - hb) + 64]

        def v_half(hb):
            return V_AB[64 * hb:64 * hb + 64]

        def emit_proj_chunk(x_sb, co, hb):
            psC1 = psA.tile([128, SB], F32)
            for ci in range(8):
                nc.tensor.matmul(psC1, wq_sb[:, ci, co * 128:(co + 1) * 128],
                                 x_sb[:, ci, :], start=(ci == 0), stop=(ci == 7))
            kind, c = co // 8, co % 8
            for dlt in range(2):
                src = psC1[64 * dlt:64 * dlt + 64, :].rearrange(
                    "p (g t) -> p g t", g=NG)
                hslot = 2 * c + dlt
                if kind == 0:
                    dst = qk_half(Q_AB, hb)[:, :, hslot, :]
                elif kind == 1:
                    dst = qk_half(K_AB, hb)[:, :, hslot, :]
                else:
                    dst = v_half(hb)[:, :, hslot, :]
                bias = bcols_sb[64 * dlt:64 * dlt + 64, co:co + 1]
                if (co + dlt) % 2 == 0:
                    nc.vector.tensor_scalar_add(dst, src, bias)
                else:
                    nc.scalar.activation(dst, src, Ident, bias=bias)

        def diag2(ap4):
            """[128, 2, 256]-ish -> diag blocks [128, 2 (stride 384), 128]."""
            flat = ap4.rearrange("p x c -> p (x c)")
            return flat.rearrange("p (q r) -> p q r", q=4)[:, ::3, :]

        def emit_attn_batch1(b4, hb):
            """scores (pair matmuls) + V transposes + fused exp + pair-mask
            for the two pairs of batch b4 (groups 4*b4..4*b4+3)."""
            vb = 64 * hb
            pend = []
            for p in range(2):
                gA = 4 * b4 + 2 * p
                psS = psSp.tile([128, 2, 256], F32)
                vft = pvf.tile([128, 384], F32, name="vf")
                psV = vft[:, 256:384].rearrange("p (x v) -> p x v", x=2)
                if SD is not F32:
                    psV = psV.bitcast(SD)
                qpair = qk_half(Q_AB, hb)[:, gA:gA + 2, :, :]
                for X in range(2):
                    nc.tensor.matmul(psS[:, X, :],
                                     qk_half(K_AB, hb)[:, gA + X, :, :],
                                     qpair, start=True, stop=True)
                for X in range(2):
                    nc.tensor.transpose(psV[:, X, :], v_half(hb)[:, gA + X, :, :],
                                        idS[vb:vb + 64, vb:vb + 64])
                es = es_pp[p]
                es_diag = diag2(es.rearrange("p x g h t -> p x (g h t)"))
                nc.scalar.activation(es_diag, diag2(psS), Exp, scale=0.125)
                nc.gpsimd.tensor_mul(es_diag, es_diag, maskT2)
                V8sb = v8_pp[p]
                if p == 0:
                    nc.vector.tensor_copy(V8sb[:, :, 0:64], psV)
                else:
                    nc.scalar.copy(V8sb[:, :, 0:64], psV)
                pend.append((es, V8sb, vft))
            return pend

        def emit_attn_batch2(b4, pend, S2, pairs=(0, 1)):
            for p in pairs:
                es, V8sb, vft = pend[p]
                psF = vft[:, 0:256].rearrange("p (g h t) -> p g h t", g=2, h=16)
                esf = es.rearrange("q x gp h t -> q x (gp h t)")
                for X in range(2):
                    nc.tensor.matmul(psF, V8sb[:, X, :], esf[:, X, :],
                                     start=(X == 0), stop=(X == 1))
                # psF rows 64:128: per-(head, token) softmax denominator
                rZB = smx.tile([64, 2, 16, 8], F32, name="rZB")
                nc.vector.reciprocal(rZB, psF[64:128])
                for dlt in range(2):
                    src = psF[0:64].rearrange("p gp h t -> p h gp t")[
                        :, 8 * dlt:8 * dlt + 8, :, :]
                    rzs = rZB.rearrange("p gp h t -> p h gp t")[
                        :, 8 * dlt:8 * dlt + 8, :, :]
                    dst = S2[64 * dlt:64 * dlt + 64].rearrange(
                        "p c (gb pp gp t) -> p c gb pp gp t",
                        gb=4, pp=2, gp=2)[:, :, b4 % 4, p, :, :]
                    nc.vector.tensor_mul(dst, src, rzs)

        def emit_outproj(S2, oiv, iss):
            for nh in range(2):
                psO = psOp.tile([128, 512], F32)
                for c in range(8):
                    nc.tensor.matmul(psO, S2[:, c, :],
                                     wo_sb[:, c, 512 * nh:512 * nh + 512],
                                     start=(c == 0), stop=(c == 7))
                outsb = outp.tile([128, 512], F32, name="outsb")
                nc.scalar.copy(outsb, psO)
                nc.gpsimd.tensor_add(outsb, outsb,
                                     borep_sb[:, 512 * nh:512 * nh + 512])
                nc.sync.dma_start(
                    out=out_d[bass.ds(oiv + SS * iss, SS),
                              bass.ds(512 * nh, 512)], in_=outsb)

        def emit_part(attn_oiv, attn_hb, proj_piv, proj_hb):
            """Weave attention of one superblock with projection of another."""
            x_sb = emit_xload(proj_piv) if proj_piv is not None else None
            S2 = None
            dpo = None   # deferred out-projection (S2, iss)
            for b4 in range(8):
                if attn_oiv is not None:
                    if b4 % 4 == 0:
                        S2 = s2p.tile([128, 8, SS], AD, name="S2")
                    pend = emit_attn_batch1(b4, attn_hb)
                    if dpo is not None:
                        emit_outproj(dpo[0], attn_oiv, dpo[1])
                        dpo = None
                if x_sb is not None:
                    for co in range(PRJ * b4, min(PRJ * b4 + PRJ, 24)):
                        emit_proj_chunk(x_sb, co, proj_hb)
                if attn_oiv is not None:
                    emit_attn_batch2(b4, pend, S2)
                    if b4 % 4 == 3:
                        dpo = (S2, b4 // 4)
            if dpo is not None:
                emit_outproj(dpo[0], attn_oiv, dpo[1])

        assert tok % (2 * SB) == 0 and tok >= 2 * SB
        emit_part(None, None, 0, 0)                      # prologue: proj sb0 -> A
        if tok > 2 * SB and static_loop:
            for iv in range(0, tok - 2 * SB, 2 * SB):
                emit_part(iv, 0, iv + SB, 1)             # attn A, proj -> B
                emit_part(iv + SB, 1, iv + 2 * SB, 0)    # attn B, proj -> A
        elif tok > 2 * SB and reps == 1:
            with tc.For_i(0, tok - 2 * SB, 2 * SB,
                          hint_engines=(mybir.EngineType.PE, mybir.EngineType.DVE,
                                        mybir.EngineType.Activation)) as iv:
                emit_part(iv, 0, iv + SB, 1)             # attn A, proj -> B
                emit_part(iv + SB, 1, iv + 2 * SB, 0)    # attn B, proj -> A
        elif tok > 2 * SB:
            with tc.For_i(0, reps, 1) as _rep:
                with tc.For_i(0, tok - 2 * SB, 2 * SB,
                              hint_engines=(mybir.EngineType.PE,
                                            mybir.EngineType.DVE,
                                            mybir.EngineType.Activation)) as iv:
                    emit_part(iv, 0, iv + SB, 1)         # attn A, proj -> B
                    emit_part(iv + SB, 1, iv + 2 * SB, 0)  # attn B, proj -> A
        last = tok - 2 * SB
        emit_part(last, 0, tok - SB, 1)                  # attn A, proj last -> B
        emit_part(tok - SB, 1, None, None)               # attn B
    nc.compile()
    return nc


def build_pipe5(tok, mode="bf16", static_loop=False, reps=1):
    """v5: all-bf16 + K=128-everywhere matmul shapes (HW-measured: K=64
    matmuls run ~2x slower per output column; bf16 streams beat f32r and
    LDWEIGHTS does NOT serialize on this backend):

      - everything (x, weights, staging, es, S2) in bf16; PSUM f32.
      - scores via BLOCK-DIAG pairs: K staged with even groups' d-dim on
        partitions 0:64 and odd groups' on 64:128 (K_bd[128, pr, 16, 8]);
        Q staged zero-padded block-diag (Q_bd[128, pr, 2, 16, 8], the
        off-diagonal partition halves zeroed once at startup). ONE matmul
        per pair: psS[128, 2*128] = K_bd[:,pr]^T @ Q_bd[:,pr], K=128,
        moving 256 -> both groups' score blocks, no garbage columns.
      - exp: ONE ScalarE activation [128,256] psS -> es_pair bf16 (dense,
        no diag APs); ONE GpSimd mask-mul (broadcast [128,2,128]).
      - attnV per GROUP (K=128, N=128): psF_g = V8_g^T @ es_g with the
        V8 ones-columns producing the softmax denominator rows; psF pair
        halves live side by side in the pvf tile so the pair-granular
        reciprocal + S2 pack from v4 are unchanged.
      - PSUM banks: psA x3 (1 bank ea), psS x2, pvf x2 (psF pair + psV
        transposes share a 1.25KB tile), psO x1 = 8.
      - Q/K projection evictions split even/odd groups (partition-shifted
        writes, 4 small instrs per chunk instead of 2).
    """
    WD = SD = AD = BF16

    nc = bacc.Bacc("TRN2", target_bir_lowering=False, debug=False,
                   enable_asserts=True, num_devices=N_CORES)
    xT_d = nc.dram_tensor("xT", [C, tok], WD, kind="ExternalInput").ap()
    wqkvT_d = nc.dram_tensor("wqkvT", [C, 3 * C], WD, kind="ExternalInput").ap()
    woutT_d = nc.dram_tensor("woutT", [C, C], WD, kind="ExternalInput").ap()
    bcols_d = nc.dram_tensor("bcols", [128, 24], F32, kind="ExternalInput").ap()
    borep_d = nc.dram_tensor("borep", [128, C], BF16, kind="ExternalInput").ap()
    maskP_d = nc.dram_tensor("maskP", [128, 128], BF16, kind="ExternalInput").ap()
    out_d = nc.dram_tensor("out", [tok, C], F32, kind="ExternalOutput").ap()

    PRJ = int(os.environ.get("PRJ", "3"))
    NSLA = int(os.environ.get("NSLA", "3"))
    NPAIR = NG // 2  # 16 pairs per superblock
    with tile.TileContext(nc) as tc, ExitStack() as ctx:
        consts = ctx.enter_context(tc.tile_pool(name="consts", bufs=1))
        xin = ctx.enter_context(tc.tile_pool(name="xin", bufs=2))
        stag = ctx.enter_context(tc.tile_pool(name="stag", bufs=1))
        smx = ctx.enter_context(tc.tile_pool(name="smx", bufs=3))
        s2p = ctx.enter_context(tc.tile_pool(name="s2p", bufs=2))
        outp = ctx.enter_context(tc.tile_pool(name="outp", bufs=2))
        psA = ctx.enter_context(tc.tile_pool(name="psA", bufs=NSLA, space="PSUM"))
        psSp = ctx.enter_context(tc.tile_pool(name="psSp", bufs=2, space="PSUM"))
        pvf = ctx.enter_context(tc.tile_pool(name="pvf", bufs=2, space="PSUM"))
        psOp = ctx.enter_context(tc.tile_pool(name="psOp", bufs=1, space="PSUM"))

        wq_sb = consts.tile([128, 8, 3 * C], WD)
        nc.sync.dma_start(out=wq_sb, in_=wqkvT_d.rearrange("(ci p) f -> p ci f", p=128))
        wo_sb = consts.tile([128, 8, C], WD)
        nc.sync.dma_start(out=wo_sb, in_=woutT_d.rearrange("(ci p) f -> p ci f", p=128))
        bcols_sb = consts.tile([128, 24], F32)
        nc.sync.dma_start(out=bcols_sb, in_=bcols_d)
        borep_sb = consts.tile([128, C], BF16)
        nc.sync.dma_start(out=borep_sb, in_=borep_d)
        maskT_sb = consts.tile([128, 128], BF16)
        nc.sync.dma_start(out=maskT_sb, in_=maskP_d)
        maskT2 = maskT_sb.unsqueeze(1).broadcast_to([128, 2, 128])
        idq = consts.tile([128, 128], F32)
        make_identity(nc, idq)
        idS = consts.tile([128, 128], SD)
        nc.vector.tensor_copy(idS, idq)

        # staging: block-diag K/Q per ping-pong half (full 128 partitions),
        # V keeps the half-partition ping-pong of v3/v4.
        K_bd = [stag.tile([128, NPAIR, 16, 8], SD, name=f"Kbd{i}")
                for i in range(2)]
        Q_bd = [stag.tile([128, NPAIR, 2, 16, 8], SD, name=f"Qbd{i}")
                for i in range(2)]
        V_AB = stag.tile([128, NG, 16, 8], SD, name="V_AB")
        zeroC = consts.tile([128, 128], BF16)
        nc.vector.memset(zeroC, 0.0)
        onesC = consts.tile([128, 64], BF16)
        nc.vector.memset(onesC, 1.0)
        # zero the off-diagonal Q halves once (never rewritten)
        for qb in Q_bd:
            for par in range(2):
                z = qb[64 * (1 - par):64 * (1 - par) + 64, :, par]
                nc.vector.memset(z, 0.0)
        v8_pp = [stag.tile([128, 2, 128], SD, name=f"v8_{i}") for i in range(2)]
        for e in v8_pp:
            nc.vector.tensor_copy(e[:, 0, 64:128], onesC)
            nc.vector.tensor_copy(e[:, 1, 64:128], onesC)

        xT_r = xT_d.rearrange("(ci p) t -> p ci t", p=128)

        def emit_xload(piv):
            x_sb = xin.tile([128, 8, SB], WD)
            nc.sync.dma_start(out=x_sb, in_=xT_r[:, :, bass.ds(piv, SB)])
            return x_sb

        def v_half(hb):
            return V_AB[64 * hb:64 * hb + 64]

        ECNT = [0]

        def evict(dst, src, bias):
            if ECNT[0] % 2 == 0:
                nc.vector.tensor_scalar_add(dst, src, bias)
            else:
                nc.scalar.activation(dst, src, Ident, bias=bias)
            ECNT[0] += 1

        def emit_proj_chunk(x_sb, co, hb):
            psC1 = psA.tile([128, SB], F32)
            for ci in range(8):
                nc.tensor.matmul(psC1, wq_sb[:, ci, co * 128:(co + 1) * 128],
                                 x_sb[:, ci, :], start=(ci == 0), stop=(ci == 7))
            kind, c = co // 8, co % 8
            hslot = 2 * c  # +dlt below
            for dlt in range(2):
                bias = bcols_sb[64 * dlt:64 * dlt + 64, co:co + 1]
                srcg = psC1[64 * dlt:64 * dlt + 64, :].rearrange(
                    "p (pr par t) -> p pr par t", par=2, t=8)
                if kind == 2:
                    src = psC1[64 * dlt:64 * dlt + 64, :].rearrange(
                        "p (g t) -> p g t", g=NG)
                    evict(v_half(hb)[:, :, hslot + dlt, :], src, bias)
                    continue
                for par in range(2):
                    src = srcg[:, :, par, :]
                    if kind == 0:
                        dst = Q_bd[hb][64 * par:64 * par + 64, :, par,
                                       hslot + dlt, :]
                    else:
                        dst = K_bd[hb][64 * par:64 * par + 64, :,
                                       hslot + dlt, :]
                    evict(dst, src, bias)

        def emit_attn_batch1(b4, hb):
            """block-diag pair scores + V^T transposes + fused exp/mask."""
            vb = 64 * hb
            pend = []
            for p in range(2):
                pr = 2 * b4 + p
                gA = 4 * b4 + 2 * p
                psS = psSp.tile([128, 256], F32)
                vft = pvf.tile([128, 320], F32, name="vf")
                psV = vft[:, 256:320].bitcast(SD).rearrange(
                    "p (x v) -> p x v", x=2)
                nc.tensor.matmul(psS, K_bd[hb][:, pr], Q_bd[hb][:, pr],
                                 start=True, stop=True)
                for X in range(2):
                    nc.tensor.transpose(psV[:, X, :], v_half(hb)[:, gA + X, :, :],
                                        idS[vb:vb + 64, vb:vb + 64])
                es = smx.tile([128, 2, 128], SD, name="es")
                nc.scalar.activation(es, psS.rearrange("p (x c) -> p x c", x=2),
                                     Exp, scale=0.125)
                nc.gpsimd.tensor_mul(es, es, maskT2)
                V8sb = v8_pp[p]
                if p == 0:
                    nc.vector.tensor_copy(V8sb[:, :, 0:64], psV)
                else:
                    nc.scalar.copy(V8sb[:, :, 0:64], psV)
                pend.append((es, V8sb, vft))
            return pend

        def emit_attn_batch2(b4, pend, S2, pairs=(0, 1)):
            for p in pairs:
                es, V8sb, vft = pend[p]
                psF = vft[:, 0:256].rearrange("p (g h t) -> p g h t", g=2, h=16)
                for X in range(2):
                    nc.tensor.matmul(psF[:, X], V8sb[:, X, :], es[:, X, :],
                                     start=True, stop=True)
                rZB = smx.tile([64, 2, 16, 8], F32, name="rZB")
                nc.vector.reciprocal(rZB, psF[64:128])
                for dlt in range(2):
                    src = psF[0:64].rearrange("p gp h t -> p h gp t")[
                        :, 8 * dlt:8 * dlt + 8, :, :]
                    rzs = rZB.rearrange("p gp h t -> p h gp t")[
                        :, 8 * dlt:8 * dlt + 8, :, :]
                    dst = S2[64 * dlt:64 * dlt + 64].rearrange(
                        "p c (gb pp gp t) -> p c gb pp gp t",
                        gb=4, pp=2, gp=2)[:, :, b4 % 4, p, :, :]
                    nc.vector.tensor_mul(dst, src, rzs)

        def emit_outproj(S2, oiv, iss):
            for nh in range(2):
                psO = psOp.tile([128, 512], F32)
                for c in range(8):
                    nc.tensor.matmul(psO, S2[:, c, :],
                                     wo_sb[:, c, 512 * nh:512 * nh + 512],
                                     start=(c == 0), stop=(c == 7))
                outsb = outp.tile([128, 512], F32, name="outsb")
                nc.scalar.copy(outsb, psO)
                nc.gpsimd.tensor_add(outsb, outsb,
                                     borep_sb[:, 512 * nh:512 * nh + 512])
                nc.sync.dma_start(
                    out=out_d[bass.ds(oiv + SS * iss, SS),
                              bass.ds(512 * nh, 512)], in_=outsb)

        def emit_part(attn_oiv, attn_hb, proj_piv, proj_hb):
            x_sb = emit_xload(proj_piv) if proj_piv is not None else None
            S2 = None
            dpo = None
            for b4 in range(8):
                if attn_oiv is not None:
                    if b4 % 4 == 0:
                        S2 = s2p.tile([128, 8, SS], AD, name="S2")
                    pend = emit_attn_batch1(b4, attn_hb)
                    if dpo is not None:
                        emit_outproj(dpo[0], attn_oiv, dpo[1])
                        dpo = None
                if x_sb is not None:
                    for co in range(PRJ * b4, min(PRJ * b4 + PRJ, 24)):
                        emit_proj_chunk(x_sb, co, proj_hb)
                if attn_oiv is not None:
                    emit_attn_batch2(b4, pend, S2)
                    if b4 % 4 == 3:
                        dpo = (S2, b4 // 4)
            if dpo is not None:
                emit_outproj(dpo[0], attn_oiv, dpo[1])

        assert tok % (2 * SB) == 0 and tok >= 2 * SB
        emit_part(None, None, 0, 0)
        if tok > 2 * SB and static_loop:
            for iv in range(0, tok - 2 * SB, 2 * SB):
                emit_part(iv, 0, iv + SB, 1)
                emit_part(iv + SB, 1, iv + 2 * SB, 0)
        elif tok > 2 * SB and reps == 1:
            with tc.For_i(0, tok - 2 * SB, 2 * SB,
                          hint_engines=(mybir.EngineType.PE, mybir.EngineType.DVE,
                                        mybir.EngineType.Activation)) as iv:
                emit_part(iv, 0, iv + SB, 1)
                emit_part(iv + SB, 1, iv + 2 * SB, 0)
        elif tok > 2 * SB:
            with tc.For_i(0, reps, 1) as _rep:
                with tc.For_i(0, tok - 2 * SB, 2 * SB,
                              hint_engines=(mybir.EngineType.PE,
                                            mybir.EngineType.DVE,
                                            mybir.EngineType.Activation)) as iv:
                    emit_part(iv, 0, iv + SB, 1)
                    emit_part(iv + SB, 1, iv + 2 * SB, 0)
        last = tok - 2 * SB
        emit_part(last, 0, tok - SB, 1)
        emit_part(tok - SB, 1, None, None)
    nc.compile()
    return nc


def build_pipe6(tok, mode="bf16", static_loop=False, reps=1):
    """v6: v5's all-bf16 + K=128 shapes, restructured to minimize instruction
    count (HW shows ~100ns-class per-instruction sync/sequencer overhead that
    the cost model underestimates):
      - SB=512 token superblocks: projection matmuls go 512-wide (same
        per-column rate, HALF the instruction + LDWEIGHTS count), evictions
        double in size and halve in count.
      - attention in QUADS (4 groups): ONE exp [128,512], ONE mask-mul,
        ONE V^T->SBUF copy, ONE reciprocal per quad; 2 block-diag scores
        matmuls, 4 transposes, 4 attnV matmuls, 2 S2-pack muls.
      - PSUM: psA [128,512] x3 (shared by projection chunks AND the
        out-projection), psS-quad [128,2,256] x2, psF-quad [128,4,16,8] x2,
        psV-quad [128,4,64]bf16 x1 = 8 banks.
    """
    del mode
    SB6, SS6 = 512, 128
    NG6 = SB6 // 8          # 64 groups
    NPAIR6 = NG6 // 2       # 32 pairs
    SD = BF16

    nc = bacc.Bacc("TRN2", target_bir_lowering=False, debug=False,
                   enable_asserts=True, num_devices=N_CORES)
    xT_d = nc.dram_tensor("xT", [C, tok], SD, kind="ExternalInput").ap()
    wqkvT_d = nc.dram_tensor("wqkvT", [C, 3 * C], SD, kind="ExternalInput").ap()
    woutT_d = nc.dram_tensor("woutT", [C, C], SD, kind="ExternalInput").ap()
    bcols_d = nc.dram_tensor("bcols", [128, 24], F32, kind="ExternalInput").ap()
    borep_d = nc.dram_tensor("borep", [128, C], BF16, kind="ExternalInput").ap()
    maskP_d = nc.dram_tensor("maskP", [128, 128], BF16, kind="ExternalInput").ap()
    out_d = nc.dram_tensor("out", [tok, C], F32, kind="ExternalOutput").ap()

    NSLA = int(os.environ.get("NSLA", "2"))
    with tile.TileContext(nc) as tc, ExitStack() as ctx:
        consts = ctx.enter_context(tc.tile_pool(name="consts", bufs=1))
        xin = ctx.enter_context(tc.tile_pool(name="xin", bufs=3))
        stag = ctx.enter_context(tc.tile_pool(name="stag", bufs=1))
        smx = ctx.enter_context(tc.tile_pool(name="smx", bufs=5))
        s2p = ctx.enter_context(tc.tile_pool(name="s2p", bufs=3))
        outp = ctx.enter_context(tc.tile_pool(name="outp", bufs=4))
        psA = ctx.enter_context(tc.tile_pool(name="psA", bufs=NSLA, space="PSUM"))
        psSp = ctx.enter_context(tc.tile_pool(
            name="psSp", bufs=int(os.environ.get("NSLS", "2")), space="PSUM"))
        psFp = ctx.enter_context(tc.tile_pool(
            name="psFp", bufs=int(os.environ.get("NSLF", "1")), space="PSUM"))
        psVp = ctx.enter_context(tc.tile_pool(
            name="psVp", bufs=int(os.environ.get("NSLV", "2")), space="PSUM"))
        SHWO = os.environ.get("SHWO", "0") == "1"  # outproj shares psA pool
        psOp = None if SHWO else ctx.enter_context(
            tc.tile_pool(name="psOp", bufs=1, space="PSUM"))

        wq_sb = consts.tile([128, 8, 3 * C], SD)
        nc.sync.dma_start(out=wq_sb, in_=wqkvT_d.rearrange("(ci p) f -> p ci f", p=128))
        wo_sb = consts.tile([128, 8, C], SD)
        nc.sync.dma_start(out=wo_sb, in_=woutT_d.rearrange("(ci p) f -> p ci f", p=128))
        bcols_sb = consts.tile([128, 24], F32)
        nc.sync.dma_start(out=bcols_sb, in_=bcols_d)
        borep_sb = consts.tile([128, C], BF16)
        nc.sync.dma_start(out=borep_sb, in_=borep_d)
        maskT_sb = consts.tile([128, 128], BF16)
        nc.sync.dma_start(out=maskT_sb, in_=maskP_d)
        maskT4 = maskT_sb.unsqueeze(1).broadcast_to([128, 4, 128])
        idq = consts.tile([128, 128], F32)
        make_identity(nc, idq)
        idS = consts.tile([128, 128], SD)
        nc.vector.tensor_copy(idS, idq)

        K_bd = [stag.tile([128, NPAIR6, 16, 8], SD, name=f"Kbd{i}")
                for i in range(2)]
        Q_bd = [stag.tile([128, NPAIR6, 2, 16, 8], SD, name=f"Qbd{i}")
                for i in range(2)]
        V_AB = stag.tile([128, NG6, 16, 8], SD, name="V_AB")
        onesC = consts.tile([128, 64], BF16)
        nc.vector.memset(onesC, 1.0)
        for qb in Q_bd:
            for par in range(2):
                nc.vector.memset(qb[64 * (1 - par):64 * (1 - par) + 64, :, par],
                                 0.0)
        # persistent V8 quad tiles [128, 4(g), 128]: cols 64:128 ones
        v8q = [stag.tile([128, 4, 128], SD, name=f"v8q{i}") for i in range(3)]
        for e in v8q:
            for g in range(4):
                nc.vector.tensor_copy(e[:, g, 64:128], onesC)

        xT_r = xT_d.rearrange("(ci p) t -> p ci t", p=128)

        def emit_xload(piv):
            x_sb = xin.tile([128, 8, SB6], SD)
            nc.sync.dma_start(out=x_sb, in_=xT_r[:, :, bass.ds(piv, SB6)])
            return x_sb

        def v_half(hb):
            return V_AB[64 * hb:64 * hb + 64]

        ECNT = [0]

        def evict(dst, src, bias):
            if ECNT[0] % 2 == 0:
                nc.vector.tensor_scalar_add(dst, src, bias)
            else:
                nc.scalar.activation(dst, src, Ident, bias=bias)
            ECNT[0] += 1

        def emit_proj_chunk(x_sb, co, hb):
            psC1 = psA.tile([128, SB6], F32, name="pa")
            for ci in range(8):
                nc.tensor.matmul(psC1, wq_sb[:, ci, co * 128:(co + 1) * 128],
                                 x_sb[:, ci, :], start=(ci == 0), stop=(ci == 7))
            kind, c = co // 8, co % 8
            hslot = 2 * c
            for dlt in range(2):
                bias = bcols_sb[64 * dlt:64 * dlt + 64, co:co + 1]
                if kind == 2:
                    src = psC1[64 * dlt:64 * dlt + 64, :].rearrange(
                        "p (g t) -> p g t", g=NG6)
                    evict(v_half(hb)[:, :, hslot + dlt, :], src, bias)
                    continue
                srcg = psC1[64 * dlt:64 * dlt + 64, :].rearrange(
                    "p (pr par t) -> p pr par t", par=2, t=8)
                for par in range(2):
                    src = srcg[:, :, par, :]
                    if kind == 0:
                        dst = Q_bd[hb][64 * par:64 * par + 64, :, par,
                                       hslot + dlt, :]
                    else:
                        dst = K_bd[hb][64 * par:64 * par + 64, :,
                                       hslot + dlt, :]
                    evict(dst, src, bias)

        def emit_attn_q1(qi, hb):
            """quad qi (groups 4qi..4qi+3 = pairs 2qi, 2qi+1): scores,
            transposes, fused exp/mask, V8 copy."""
            vb = 64 * hb
            psS = psSp.tile([128, 2, 256], F32)
            psV = psVp.tile([128, 4, 64], SD)
            for p in range(2):
                pr = 2 * qi + p
                nc.tensor.matmul(psS[:, p], K_bd[hb][:, pr], Q_bd[hb][:, pr],
                                 start=True, stop=True)
            for g in range(4):
                nc.tensor.transpose(psV[:, g, :], v_half(hb)[:, 4 * qi + g, :, :],
                                    idS[vb:vb + 64, vb:vb + 64])
            es = smx.tile([128, 4, 128], SD, name="es")
            nc.scalar.activation(es, psS.rearrange("p a (b c) -> p (a b) c", b=2),
                                 Exp, scale=0.125)
            nc.gpsimd.tensor_mul(es, es, maskT4)
            V8sb = v8q[qi % 3]
            if qi % 2 == 0:
                nc.vector.tensor_copy(V8sb[:, :, 0:64], psV)
            else:
                nc.scalar.copy(V8sb[:, :, 0:64], psV)
            return es, V8sb

        def emit_attn_q2(qi, es, V8sb, S2):
            psF = psFp.tile([128, 4, 16, 8], F32)
            for g in range(4):
                nc.tensor.matmul(psF[:, g], V8sb[:, g, :], es[:, g, :],
                                 start=True, stop=True)
            rZB = smx.tile([64, 4, 16, 8], F32, name="rZB")
            nc.vector.reciprocal(rZB, psF[64:128])
            for dlt in range(2):
                src = psF[0:64].rearrange("p g h t -> p h g t")[
                    :, 8 * dlt:8 * dlt + 8, :, :]
                rzs = rZB.rearrange("p g h t -> p h g t")[
                    :, 8 * dlt:8 * dlt + 8, :, :]
                dst = S2[64 * dlt:64 * dlt + 64].rearrange(
                    "p c (gb gq t) -> p c gb gq t", gb=4, gq=4)[:, :, qi % 4]
                nc.vector.tensor_mul(dst, src, rzs)

        def emit_outproj(S2, oiv, iss):
            for nh in range(2):
                psO = (psA.tile([128, 512], F32, name="pa") if SHWO
                       else psOp.tile([128, 512], F32))
                for c in range(8):
                    nc.tensor.matmul(psO, S2[:, c, :],
                                     wo_sb[:, c, 512 * nh:512 * nh + 512],
                                     start=(c == 0), stop=(c == 7))
                outsb = outp.tile([128, 512], F32, name="outsb")
                nc.scalar.copy(outsb, psO)
                nc.gpsimd.tensor_add(outsb, outsb,
                                     borep_sb[:, 512 * nh:512 * nh + 512])
                nc.sync.dma_start(
                    out=out_d[bass.ds(oiv + SS6 * iss, SS6),
                              bass.ds(512 * nh, 512)], in_=outsb)

        def emit_part(attn_oiv, attn_hb, proj_piv, proj_hb):
            """16 quads of attention woven with 24 projection chunks. The
            attnV/normalize phase (q2) of quad qi is DEFERRED until after
            quad qi+1's scores and projection filler, so the cross-engine
            exp->mask chain latency never stalls the PE before attnV."""
            x_sb = emit_xload(proj_piv) if proj_piv is not None else None
            S2 = None
            dpo = None
            prev = None   # deferred q2: (qi, es, V8sb, S2 of that quad)
            for qi in range(16):
                if attn_oiv is not None:
                    if qi % 4 == 0:
                        S2 = s2p.tile([128, 8, SS6], SD, name="S2")
                    pend = emit_attn_q1(qi, attn_hb)
                    if dpo is not None:
                        emit_outproj(dpo[0], attn_oiv, dpo[1])
                        dpo = None
                if x_sb is not None:
                    for co in range((3 * qi) // 2, (3 * (qi + 1)) // 2):
                        emit_proj_chunk(x_sb, co, proj_hb)
                if attn_oiv is not None:
                    if prev is not None:
                        emit_attn_q2(prev[0], prev[1], prev[2], prev[3])
                        if prev[0] % 4 == 3:
                            dpo = (prev[3], prev[0] // 4)
                    prev = (qi, pend[0], pend[1], S2)
            if attn_oiv is not None and prev is not None:
                emit_attn_q2(prev[0], prev[1], prev[2], prev[3])
                if prev[0] % 4 == 3:
                    dpo = (prev[3], prev[0] // 4)
            if dpo is not None:
                emit_outproj(dpo[0], attn_oiv, dpo[1])

        assert tok % (2 * SB6) == 0 and tok >= 2 * SB6
        emit_part(None, None, 0, 0)
        if tok > 2 * SB6 and static_loop:
            for iv in range(0, tok - 2 * SB6, 2 * SB6):
                emit_part(iv, 0, iv + SB6, 1)
                emit_part(iv + SB6, 1, iv + 2 * SB6, 0)
        elif tok > 2 * SB6 and reps == 1:
            with tc.For_i(0, tok - 2 * SB6, 2 * SB6,
                          hint_engines=(mybir.EngineType.PE, mybir.EngineType.DVE,
                                        mybir.EngineType.Activation)) as iv:
                emit_part(iv, 0, iv + SB6, 1)
                emit_part(iv + SB6, 1, iv + 2 * SB6, 0)
        elif tok > 2 * SB6:
            with tc.For_i(0, reps, 1) as _rep:
                with tc.For_i(0, tok - 2 * SB6, 2 * SB6,
                              hint_engines=(mybir.EngineType.PE,
                                            mybir.EngineType.DVE,
                                            mybir.EngineType.Activation)) as iv:
                    emit_part(iv, 0, iv + SB6, 1)
                    emit_part(iv + SB6, 1, iv + 2 * SB6, 0)
        last = tok - 2 * SB6
        emit_part(last, 0, tok - SB6, 1)
        emit_part(tok - SB6, 1, None, None)
    nc.compile()
    return nc


def _round_f32r(a):
    """Round fp32 to the f32r grid (drop 12 mantissa bits, round-to-nearest)."""
    b = np.ascontiguousarray(a, dtype=np.float32).view(np.uint32)
    b = ((b + (1 << 11)) >> 12) << 12
    return b.view(np.float32)


def _wcast(a, mode):
    if mode == "bf16":
        return np.ascontiguousarray(a.astype(ml_dtypes.bfloat16))
    if mode == "f32r":
        return _round_f32r(np.ascontiguousarray(a, dtype=np.float32))
    return np.ascontiguousarray(a, dtype=np.float32)


def _host_prep(x, w_qkv, b_qkv, w_out, b_out, mode=KMODE):
    d = np.arange(D)
    perm_q = (192 * np.arange(H)[:, None] + d[None, :]).reshape(-1)
    perm = np.concatenate([perm_q, perm_q + 64, perm_q + 128])
    wqkvT = np.ascontiguousarray(w_qkv[perm, :].T, dtype=np.float32)
    bcols = np.ascontiguousarray(
        b_qkv[perm].reshape(24, 128).T, dtype=np.float32)
    # out-proj row perm: S2 row 128c+64dlt+d holds feature 64*(8dlt+c)+d
    co, dl = np.arange(8), np.arange(2)
    perm_o = (64 * (8 * dl[None, :, None] + co[:, None, None])
              + d[None, None, :]).reshape(-1)
    woutT = np.ascontiguousarray(w_out.T[perm_o, :], dtype=np.float32)
    borep = np.ascontiguousarray(
        np.broadcast_to(b_out[None, :], (128, C)), dtype=np.float32)
    maskB = np.tile((np.arange(128)[:, None] % 8
                     == np.arange(128)[None, :] % 8).astype(np.float32), (1, 4))
    # in-group mask [128 rows=(hk,tk), (hq, tq)]: keep tk==tq
    maskP = np.ascontiguousarray(
        (np.arange(128)[:, None] % 8 == np.arange(128)[None, :] % 8
         ).astype(ml_dtypes.bfloat16))
    borep16 = np.ascontiguousarray(borep.astype(ml_dtypes.bfloat16))
    maskP2 = np.ascontiguousarray(np.tile(maskP, (1, 2)))
    xT = np.ascontiguousarray(x.T, dtype=np.float32)
    if XBF:
        xT16 = np.ascontiguousarray(xT.astype(ml_dtypes.bfloat16))
    else:
        xT16 = _wcast(xT, mode)
    xT = _wcast(xT, mode)
    wqkvT = _wcast(wqkvT, mode)
    woutT = _wcast(woutT, mode)
    if mode == "bf16":
        maskB = np.ascontiguousarray(maskB.astype(ml_dtypes.bfloat16))
    return dict(xT=xT, xT16=xT16, wqkvT=wqkvT, bcols=bcols, woutT=woutT,
                borep=borep, borep16=borep16, maskB=maskB, maskP=maskP,
                maskP2=maskP2)


_cache = {}


def kernel(x, w_qkv, b_qkv, w_out, b_out, _trace=False, _tmpdir=None):
    x = np.asarray(x)
    n = x.shape[0]
    tok = n // N_CORES
    pipe = os.environ.get("PIPE", "6")
    hp = _host_prep(
        np.asarray(x), np.asarray(w_qkv), np.asarray(b_qkv),
        np.asarray(w_out), np.asarray(b_out),
        mode="bf16" if pipe in ("5", "6") else KMODE)
    key = (tok, KMODE, pipe)
    if key not in _cache:
        _cache[key] = {"6": build_pipe6, "5": build_pipe5,
                       "4": build_pipe4,
                       "1": build_pipe}.get(pipe, build)(tok)
    nc = _cache[key]
    if pipe in ("5", "6"):
        shared = dict(wqkvT=hp["wqkvT"], woutT=hp["woutT"], bcols=hp["bcols"],
                      borep=hp["borep16"], maskP=hp["maskP"])
        xT = hp["xT"]
    elif pipe == "4":
        xT = hp["xT16"]
        shared = dict(wqkvT=hp["wqkvT"], woutT=hp["woutT"], bcols=hp["bcols"],
                      borep=hp["borep16"], maskP=hp["maskP"])
    elif pipe == "1":
        xT = hp["xT16"]
        shared = dict(wqkvT=hp["wqkvT"], woutT=hp["woutT"], bcols=hp["bcols"],
                      borep=hp["borep16"], maskP=hp["maskP"])
    else:
        xT = hp["xT"]
        shared = dict(wqkvT=hp["wqkvT"], woutT=hp["woutT"], bcols=hp["bcols"],
                      borep=hp["borep"], maskB=hp["maskB"])
    in_maps = [dict(xT=np.ascontiguousarray(xT[:, i * tok:(i + 1) * tok]), **shared)
               for i in range(N_CORES)]
    res = run_bass_kernel_spmd(nc, in_maps, core_ids=list(range(N_CORES)),
                               trace=_trace, tmpdir=_tmpdir)
    out = np.concatenate([res.results[i]["out"] for i in range(N_CORES)], axis=0)
    kernel.last_results = res
    mod = sys.modules[__name__]
    mod.last_nc = nc
    mod.last_in_maps = in_maps
    mod.build_current = {"6": build_pipe6, "5": build_pipe5,
                         "4": build_pipe4,
                         "1": build_pipe}.get(pipe, build)
    mod.last_step = 1024 if pipe == "6" else 512
    return out

